# revision 1
# baseline (speedup 1.0000x reference)
"""Trainium2 Bass kernel for nn_NodeModel (GNN message passing).

Reference computation:
    h   = relu(concat(x[row], edge_attr) @ W1 + b1) @ W2 + b2     # edge MLP
    agg = scatter_mean(h, col, N)                                  # per-dest mean
    out = relu(concat(x, agg) @ W3 + b3) @ W4 + b4                 # node MLP

Distribution strategy (8 cores, no collectives needed):
  - Sort edges by destination node; split destination nodes into 8
    block-aligned, edge-balanced shards.  Each core owns one node shard and
    ALL edges targeting it, so per-node sums are complete locally.
  - x is replicated; each core gathers x[row] for its edges with indirect
    DMA on-device.
  - Edge MLP runs with weights stationary and activations kept transposed
    [feat, edge]; h2 rows are staged to DRAM.
  - Scatter-mean per 128-node block: indirect-gather the block's h2 rows,
    build a one-hot selection matrix with is_equal against an iota, and
    matmul-accumulate S^T @ h2 in PSUM; scale by 1/count.
  - Node MLP on the local shard; outputs are concatenated on host.

All matmuls run in float32r (TF32-like, full PE rate); accumulation fp32.
"""

import math
import sys
from contextlib import ExitStack

sys.path.insert(0, "/opt/trn_rl_repo")

import numpy as np

import concourse.bass as bass
import concourse.tile as tile
from concourse import bacc, mybir
from concourse.bass_utils import run_bass_kernel_spmd

NCORES = 8
P = 128
FN = 512    # node feature dim
FE = 128    # edge feature dim
HID = 1280  # edge-MLP hidden/output dim
F32 = mybir.dt.float32
F32R = mybir.dt.float32r
I32 = mybir.dt.int32
RELU = mybir.ActivationFunctionType.Relu

_prog_cache = {}


def _build(EC, NB, KB, NX):
    """Build the SPMD program for one core.

    EC: edge chunks (128 edges each) per core, multiple of 4.
    NB: node blocks (128 nodes each) per core, multiple of 4.
    KB: max edge chunks per node block (scatter schedule width).
    NX: number of rows of the replicated x (gather source).
    """
    EP = EC * P
    NBP = NB * P
    SC = EC // 4   # superchunks of 512 edges
    NSB = NB // 4  # superblocks of 512 nodes

    nc = bacc.Bacc("TRN2", target_bir_lowering=False, debug=False,
                   num_devices=NCORES)

    x_d = nc.dram_tensor("x", [NX, FN], F32R, kind="ExternalInput")
    rows_d = nc.dram_tensor("rows", [P, EC], I32, kind="ExternalInput")
    eaT_d = nc.dram_tensor("eaT", [FE, EP], F32R, kind="ExternalInput")
    W1_d = nc.dram_tensor("W1", [FN + FE, HID], F32R, kind="ExternalInput")
    W2_d = nc.dram_tensor("W2", [HID, HID], F32R, kind="ExternalInput")
    W3_d = nc.dram_tensor("W3", [FN + HID, FN + FE], F32R, kind="ExternalInput")
    W4_d = nc.dram_tensor("W4", [FN + FE, FN], F32R, kind="ExternalInput")
    b1_d = nc.dram_tensor("b1", [P, HID // P], F32, kind="ExternalInput")
    b2_d = nc.dram_tensor("b2", [P, HID // P], F32, kind="ExternalInput")
    b3_d = nc.dram_tensor("b3", [P, (FN + FE) // P], F32, kind="ExternalInput")
    b4_d = nc.dram_tensor("b4", [P, FN // P], F32, kind="ExternalInput")
    gid_d = nc.dram_tensor("gid", [P, NB * KB], I32, kind="ExternalInput")
    colb_d = nc.dram_tensor("colb", [P, NB * KB], F32, kind="ExternalInput")
    invc_d = nc.dram_tensor("invc", [P, NB], F32, kind="ExternalInput")
    xsT_d = nc.dram_tensor("xsT", [FN, NBP], F32R, kind="ExternalInput")
    iota_d = nc.dram_tensor("iota", [P, P], F32, kind="ExternalInput")
    ident_d = nc.dram_tensor("ident", [P, P], F32R, kind="ExternalInput")
    out_d = nc.dram_tensor("out", [NBP, FN], F32, kind="ExternalOutput")
    h2_d = nc.dram_tensor("h2buf", [EP, HID], F32R)  # internal staging

    with tile.TileContext(nc) as tc, ExitStack() as ctx:
        cpool = ctx.enter_context(tc.tile_pool(name="const", bufs=1))

        identt = cpool.tile([P, P], F32R)
        nc.sync.dma_start(identt[:], ident_d.ap()[:])
        iotat = cpool.tile([P, P], F32)
        nc.sync.dma_start(iotat[:], iota_d.ap()[:])
        b1t = cpool.tile([P, HID // P], F32)
        nc.sync.dma_start(b1t[:], b1_d.ap()[:])
        b2t = cpool.tile([P, HID // P], F32)
        nc.sync.dma_start(b2t[:], b2_d.ap()[:])
        b3t = cpool.tile([P, (FN + FE) // P], F32)
        nc.sync.dma_start(b3t[:], b3_d.ap()[:])
        b4t = cpool.tile([P, FN // P], F32)
        nc.sync.dma_start(b4t[:], b4_d.ap()[:])
        rowst = cpool.tile([P, EC], I32)
        nc.sync.dma_start(rowst[:], rows_d.ap()[:])
        gidt = cpool.tile([P, NB * KB], I32)
        nc.sync.dma_start(gidt[:], gid_d.ap()[:])
        colbt = cpool.tile([P, NB * KB], F32)
        nc.sync.dma_start(colbt[:], colb_d.ap()[:])
        invct = cpool.tile([P, NB], F32)
        nc.sync.dma_start(invct[:], invc_d.ap()[:])

        # ---------------- Phase E: edge MLP ----------------
        # Transposes run in PE transpose-mode, which does NOT count as
        # PE activity for the HAM clock gate: a burst of back-to-back
        # transposes >3.4us re-throttles the PE to 1.2 GHz.  All
        # transposes are therefore interleaved between matmul groups,
        # and gathers are pipelined one superchunk ahead.
        with ExitStack() as ectx:
            wpool = ectx.enter_context(tc.tile_pool(name="wE", bufs=1))
            W1t = wpool.tile([P, 5, HID], F32R)
            W1r = W1_d.ap().rearrange("(ko ki) m -> ki ko m", ki=P)
            for k in range(5):
                nc.sync.dma_start(W1t[:, k, :], W1r[:, k, :])
            W2t = wpool.tile([P, 10, HID], F32R)
            W2r = W2_d.ap().rearrange("(ko ki) m -> ki ko m", ki=P)
            for k in range(10):
                nc.sync.dma_start(W2t[:, k, :], W2r[:, k, :])

            ptp = ectx.enter_context(
                tc.tile_pool(name="ptp", bufs=2, space="PSUM"))
            xgp = ectx.enter_context(tc.tile_pool(name="xg", bufs=2))
            xgTp = ectx.enter_context(tc.tile_pool(name="xgT", bufs=2))
            eap = ectx.enter_context(tc.tile_pool(name="ea", bufs=2))
            h1p = ectx.enter_context(tc.tile_pool(name="h1T", bufs=1))
            h2Tp = ectx.enter_context(tc.tile_pool(name="h2T", bufs=1))
            h2op = ectx.enter_context(tc.tile_pool(name="h2o", bufs=4))
            mmp = ectx.enter_context(
                tc.tile_pool(name="mmE", bufs=4, space="PSUM"))

            def issue_gather(sc):
                xgt = xgp.tile([P, 4, FN], F32R)
                for k in range(4):
                    nc.gpsimd.indirect_dma_start(
                        out=xgt[:, k, :], out_offset=None, in_=x_d.ap()[:],
                        in_offset=bass.IndirectOffsetOnAxis(
                            ap=rowst[:, sc * 4 + k:sc * 4 + k + 1], axis=0))
                eat = eap.tile([P, 512], F32R)
                nc.sync.dma_start(
                    eat[:], eaT_d.ap()[:, sc * 512:(sc + 1) * 512])
                return xgt, eat

            def entry_T(xgt, xgTt, f, k):
                pt = ptp.tile([P, P], F32R)
                nc.tensor.transpose(
                    pt[:], xgt[:, k, f * P:(f + 1) * P], identt[:])
                nc.vector.tensor_copy(xgTt[:, f, k * P:(k + 1) * P], pt[:])

            # prologue: superchunk 0 input + its entry transposes
            xg_cur, ea_cur = issue_gather(0)
            xgT_cur = xgTp.tile([P, 4, 512], F32R)
            for f in range(4):
                for k in range(4):
                    entry_T(xg_cur, xgT_cur, f, k)

            for sc in range(SC):
                if sc + 1 < SC:
                    xg_next, ea_next = issue_gather(sc + 1)
                    xgT_next = xgTp.tile([P, 4, 512], F32R)
                else:
                    xg_next = ea_next = xgT_next = None

                h1Tt = h1p.tile([P, 10, 512], F32R)
                for of in range(10):
                    ps = mmp.tile([P, 512], F32)
                    for k in range(5):
                        rhs = xgT_cur[:, k, :] if k < 4 else ea_cur[:]
                        nc.tensor.matmul(
                            ps[:], W1t[:, k, of * P:(of + 1) * P], rhs,
                            start=(k == 0), stop=(k == 4))
                    nc.scalar.activation(h1Tt[:, of, :], ps[:], RELU,
                                         bias=b1t[:, of:of + 1])
                h2Tt = h2Tp.tile([P, 10, 512], F32R)
                h2ot = [h2op.tile([P, HID], F32R, name=f"h2o_{sc}_{k}", tag="h2o")
                         for k in range(4)]
                for of in range(10):
                    ps = mmp.tile([P, 512], F32)
                    for k in range(10):
                        nc.tensor.matmul(
                            ps[:], W2t[:, k, of * P:(of + 1) * P],
                            h1Tt[:, k, :], start=(k == 0), stop=(k == 9))
                    nc.scalar.activation(
                        h2Tt[:, of, :], ps[:],
                        mybir.ActivationFunctionType.Identity,
                        bias=b2t[:, of:of + 1])
                    # interleave: this of-chunk's exit transposes
                    for k in range(4):
                        pt = ptp.tile([P, P], F32R)
                        nc.tensor.transpose(
                            pt[:], h2Tt[:, of, k * P:(k + 1) * P], identt[:])
                        nc.vector.tensor_copy(
                            h2ot[k][:, of * P:(of + 1) * P], pt[:])
                    # interleave: next superchunk's entry transposes
                    if xgT_next is not None and of < 8:
                        for k in range(2):
                            entry_T(xg_next, xgT_next, of // 2, (of % 2) * 2 + k)
                for k in range(4):
                    r0 = sc * 512 + k * P
                    nc.sync.dma_start(h2_d.ap()[r0:r0 + P, :], h2ot[k][:])
                xg_cur, ea_cur, xgT_cur = xg_next, ea_next, xgT_next

        # ---------------- Phases S+N: scatter-mean + node MLP ----------------
        with ExitStack() as sctx:
            wpool2 = sctx.enter_context(tc.tile_pool(name="wN", bufs=1))
            W3t = wpool2.tile([P, 14, FN + FE], F32R)
            nc.sync.dma_start(
                W3t[:], W3_d.ap().rearrange("(ko ki) m -> ki ko m", ki=P))
            W4t = wpool2.tile([P, 5, FN], F32R)
            nc.sync.dma_start(
                W4t[:], W4_d.ap().rearrange("(ko ki) m -> ki ko m", ki=P))

            h2gp = sctx.enter_context(tc.tile_pool(name="h2g", bufs=7))
            Sp = sctx.enter_context(tc.tile_pool(name="Smat", bufs=3))
            aggp = sctx.enter_context(tc.tile_pool(name="agg", bufs=2))
            aggTp = sctx.enter_context(tc.tile_pool(name="aggT", bufs=2))
            xsp = sctx.enter_context(tc.tile_pool(name="xs", bufs=2))
            h3p = sctx.enter_context(tc.tile_pool(name="h3T", bufs=1))
            oTp = sctx.enter_context(tc.tile_pool(name="oT", bufs=2))
            ogp = sctx.enter_context(tc.tile_pool(name="og", bufs=4))
            smp = sctx.enter_context(
                tc.tile_pool(name="smp", bufs=1, space="PSUM"))
            mmp2 = sctx.enter_context(
                tc.tile_pool(name="mmN", bufs=2, space="PSUM"))
            ptp = sctx.enter_context(
                tc.tile_pool(name="ptp2", bufs=2, space="PSUM"))

            nj = (HID + 511) // 512  # psum 512-slices of the scatter output

            # Rolling gather lookahead: block b's h2-row gathers (slow,
            # gpsimd SW-DGE) are issued one block ahead of its scatter
            # matmuls so the PE never waits on them.  Pad slots carry an
            # out-of-bounds id and are silently skipped by the DMA
            # (bounds_check), so padding costs no gather bandwidth.
            pend_gs = {}

            def gather_S(b):
                lst = []
                for k in range(KB):
                    c = b * KB + k
                    h2g = h2gp.tile([P, HID], F32R, name=f"h2g_{b}_{k}",
                                    tag="h2g")
                    nc.gpsimd.indirect_dma_start(
                        out=h2g[:], out_offset=None, in_=h2_d.ap()[:],
                        in_offset=bass.IndirectOffsetOnAxis(
                            ap=gidt[:, c:c + 1], axis=0),
                        bounds_check=EP - 1, oob_is_err=False)
                    St = Sp.tile([P, P], F32R, name=f"S_{b}_{k}", tag="S")
                    nc.vector.tensor_tensor(
                        St[:], colbt[:, c:c + 1].to_broadcast([P, P]),
                        iotat[:], op=mybir.AluOpType.is_equal)
                    lst.append((h2g, St))
                pend_gs[b] = lst

            gather_S(0)
            gather_S(1)

            def load_xst(s):
                xst = xsp.tile([P, 4, 512], F32R, name=f"xst_{s}", tag="xst")
                nc.sync.dma_start(
                    xst[:],
                    xsT_d.ap().rearrange("(fo fi) n -> fi fo n", fi=P)
                    [:, :, s * 512:(s + 1) * 512])
                return xst

            xst_cur = load_xst(0)
            for s in range(NSB):
                aggTt = aggTp.tile([P, 10, 512], F32R)
                # pending aggT transposes: (agg_tile, bb) emitted lazily so
                # they interleave with the next block's scatter matmuls
                pend = []

                def emit_aggT(n):
                    for _ in range(n):
                        if not pend:
                            return
                        agg, bb2, f = pend.pop(0)
                        pt = ptp.tile([P, P], F32R)
                        nc.tensor.transpose(
                            pt[:], agg[:, f * P:(f + 1) * P], identt[:])
                        nc.vector.tensor_copy(
                            aggTt[:, f, bb2 * P:(bb2 + 1) * P], pt[:])

                for bb in range(4):
                    b = s * 4 + bb
                    if b + 2 < NB:
                        gather_S(b + 2)
                    pss = smp.tile([P, HID], F32)
                    for k, (h2g, St) in enumerate(pend_gs.pop(b)):
                        for j in range(nj):
                            lo, hi = j * 512, min((j + 1) * 512, HID)
                            nc.tensor.matmul(
                                pss[:, lo:hi], St[:], h2g[:, lo:hi],
                                start=(k == 0), stop=(k == KB - 1))
                        q = 10 // KB
                        emit_aggT(10 - (KB - 1) * q if k == KB - 1 else q)
                    agg = aggp.tile([P, HID], F32R)
                    nc.scalar.activation(
                        agg[:], pss[:], mybir.ActivationFunctionType.Copy,
                        bias=0.0, scale=invct[:, b:b + 1])
                    pend.extend((agg, bb, f) for f in range(10))

                xst = xst_cur
                xst_cur = load_xst(s + 1) if s + 1 < NSB else None
                h3Tt = h3p.tile([P, 5, 512], F32R)
                for of in range(5):
                    ps = mmp2.tile([P, 512], F32)
                    for k in range(4):
                        nc.tensor.matmul(
                            ps[:], W3t[:, k, of * P:(of + 1) * P],
                            xst[:, k, :], start=(k == 0), stop=False)
                        emit_aggT(3)  # last block's transposes, staggered
                    for f in range(10):
                        nc.tensor.matmul(
                            ps[:], W3t[:, 4 + f, of * P:(of + 1) * P],
                            aggTt[:, f, :], start=False, stop=(f == 9))
                    nc.scalar.activation(h3Tt[:, of, :], ps[:], RELU,
                                         bias=b3t[:, of:of + 1])
                emit_aggT(100)  # drain any stragglers (non-standard KB)
                oTt = oTp.tile([P, 4, 512], F32R)
                ogs = [ogp.tile([P, FN], F32, name=f"og_{s}_{g}", tag="og")
                       for g in range(4)]
                for of in range(4):
                    ps = mmp2.tile([P, 512], F32)
                    for k in range(5):
                        nc.tensor.matmul(
                            ps[:], W4t[:, k, of * P:(of + 1) * P],
                            h3Tt[:, k, :], start=(k == 0), stop=(k == 4))
                    nc.scalar.activation(
                        oTt[:, of, :], ps[:],
                        mybir.ActivationFunctionType.Identity,
                        bias=b4t[:, of:of + 1])
                    for g in range(4):
                        pt = ptp.tile([P, P], F32R)
                        nc.tensor.transpose(
                            pt[:], oTt[:, of, g * P:(g + 1) * P], identt[:])
                        nc.vector.tensor_copy(
                            ogs[g][:, of * P:(of + 1) * P],
                            pt[:].bitcast(F32))
                for g in range(4):
                    r0 = s * 512 + g * P
                    nc.sync.dma_start(out_d.ap()[r0:r0 + P, :], ogs[g][:])

    nc.compile()
    return nc


def _prepare(x, row, col, ea):
    """Host-side sharding: sort edges by destination, split nodes into 8
    block-aligned edge-balanced shards, build per-core arrays."""
    N = x.shape[0]
    E = ea.shape[0]
    order = np.argsort(col, kind="stable")
    scol = col[order]
    srow = row[order]
    NBLK = (N + P - 1) // P
    NTOT = NBLK * P

    bounds = [0]
    for p in range(1, NCORES):
        if E > 0:
            t = int(scol[min((p * E) // NCORES, E - 1)])
        else:
            t = (p * NTOT) // NCORES
        b = int(round(t / P)) * P
        b = max(b, bounds[-1] + P)
        b = min(b, NTOT - P * (NCORES - p))
        bounds.append(b)
    bounds.append(NTOT)
    for p in range(1, NCORES + 1):
        assert bounds[p] > bounds[p - 1], f"degenerate shard bounds {bounds}"

    e_split = np.searchsorted(scol, bounds)
    Ec = np.diff(e_split)
    EC = max(4, math.ceil(int(Ec.max()) / P))
    EC = ((EC + 3) // 4) * 4
    EP = EC * P
    nblk = [(bounds[p + 1] - bounds[p]) // P for p in range(NCORES)]
    NB = max(4, ((max(nblk) + 3) // 4) * 4)
    NBP = NB * P
    blkdeg = np.bincount(scol // P, minlength=NBLK)
    KB = max(1, math.ceil(int(blkdeg.max()) / P))

    xpadT = np.zeros((FN, NTOT + NBP), np.float32)
    xpadT[:, :N] = x.T

    cores = []
    for p in range(NCORES):
        s, e = int(e_split[p]), int(e_split[p + 1])
        n0 = bounds[p]
        ne = e - s
        tmp = np.zeros(EP, np.int32)
        tmp[:ne] = srow[s:e]
        rows_t = np.ascontiguousarray(tmp.reshape(EC, P).T)
        eaT = np.zeros((FE, EP), np.float32)
        eaT[:, :ne] = ea[order[s:e]].T
        lcol = (scol[s:e] - n0).astype(np.int64)
        bstart = np.searchsorted(lcol, np.arange(NB + 1) * P)
        gid = np.full((NB, KB, P), 1 << 30, np.int32)
        gid.reshape(NB * KB, P)[:7] = 0
        colb = np.full((NB, KB, P), -1.0, np.float32)
        for b in range(NB):
            sb, eb = int(bstart[b]), int(bstart[b + 1])
            cnt = eb - sb
            assert cnt <= KB * P
            gid[b].reshape(-1)[:cnt] = np.arange(sb, eb, dtype=np.int32)
            colb[b].reshape(-1)[:cnt] = (lcol[sb:eb] - b * P)
        gid_t = np.ascontiguousarray(gid.reshape(NB * KB, P).T)
        colb_t = np.ascontiguousarray(colb.reshape(NB * KB, P).T)
        deg = np.bincount(lcol, minlength=NBP)[:NBP]
        invc_t = np.ascontiguousarray(
            (1.0 / np.maximum(deg, 1.0)).astype(np.float32).reshape(NB, P).T)
        xsT = np.ascontiguousarray(xpadT[:, n0:n0 + NBP])
        cores.append(dict(rows=rows_t, eaT=eaT, gid=gid_t, colb=colb_t,
                          invc=invc_t, xsT=xsT))
    return cores, bounds, EC, NB, KB


def _run(inputs, trace=False):
    x = np.ascontiguousarray(np.asarray(inputs["x"], dtype=np.float32))
    ei = np.asarray(inputs["edge_index"])
    ea = np.ascontiguousarray(np.asarray(inputs["edge_attr"], dtype=np.float32))
    row = ei[0].astype(np.int64)
    col = ei[1].astype(np.int64)
    W1 = np.ascontiguousarray(np.asarray(inputs["W1"], np.float32))
    W2 = np.ascontiguousarray(np.asarray(inputs["W2"], np.float32))
    W3 = np.ascontiguousarray(np.asarray(inputs["W3"], np.float32))
    W4 = np.ascontiguousarray(np.asarray(inputs["W4"], np.float32))
    b1 = np.asarray(inputs["b1"], np.float32)
    b2 = np.asarray(inputs["b2"], np.float32)
    b3 = np.asarray(inputs["b3"], np.float32)
    b4 = np.asarray(inputs["b4"], np.float32)
    N = x.shape[0]

    cores, bounds, EC, NB, KB = _prepare(x, row, col, ea)

    key = (EC, NB, KB, N)
    if key not in _prog_cache:
        _prog_cache[key] = _build(EC, NB, KB, N)
    nc = _prog_cache[key]

    b1t = np.ascontiguousarray(b1.reshape(HID // P, P).T)
    b2t = np.ascontiguousarray(b2.reshape(HID // P, P).T)
    b3t = np.ascontiguousarray(b3.reshape((FN + FE) // P, P).T)
    b4t = np.ascontiguousarray(b4.reshape(FN // P, P).T)
    iota = np.ascontiguousarray(
        np.broadcast_to(np.arange(P, dtype=np.float32), (P, P)))
    ident = np.eye(P, dtype=np.float32)

    in_maps = []
    for p in range(NCORES):
        c = cores[p]
        in_maps.append({
            "x": x, "rows": c["rows"], "eaT": c["eaT"],
            "W1": W1, "W2": W2, "W3": W3, "W4": W4,
            "b1": b1t, "b2": b2t, "b3": b3t, "b4": b4t,
            "gid": c["gid"], "colb": c["colb"], "invc": c["invc"],
            "xsT": c["xsT"], "iota": iota, "ident": ident,
        })

    res = run_bass_kernel_spmd(nc, in_maps, list(range(NCORES)), trace=trace)

    out = np.empty((N, FN), np.float32)
    for p in range(NCORES):
        n0, n1 = bounds[p], min(bounds[p + 1], N)
        if n1 > n0:
            out[n0:n1] = res.results[p]["out"][:n1 - n0]
    return out, res


def kernel(**inputs) -> np.ndarray:
    out, _ = _run(inputs, trace=False)
    return out



# revision 8
# speedup vs baseline: 1.1308x; 1.1308x over previous
"""Trainium2 Bass kernel for nn_NodeModel (GNN message passing).

Reference computation:
    h   = relu(concat(x[row], edge_attr) @ W1 + b1) @ W2 + b2     # edge MLP
    agg = scatter_mean(h, col, N)                                  # per-dest mean
    out = relu(concat(x, agg) @ W3 + b3) @ W4 + b4                 # node MLP

Distribution strategy (8 cores, no collectives needed):
  - Sort edges by destination node; split destination nodes into 8
    block-aligned, edge-balanced shards.  Each core owns one node shard and
    ALL edges targeting it, so per-node sums are complete locally.
  - x is replicated; each core gathers x[row] for its edges with indirect
    DMA on-device.
  - All matmul operands are bf16 (fp32 PSUM accumulation): halves HBM
    traffic vs fp32r and speeds PE transposes 1.5x.
  - Edge MLP runs with weights stationary and activations kept transposed
    [feat, edge]; h2 rows (pre-scaled by 1/count of their destination) are
    staged to DRAM in bf16.
  - Scatter-mean per 128-node block: indirect-gather the block's h2 rows,
    build a one-hot selection matrix with is_equal against an iota, and
    matmul-accumulate h2^T @ S in PSUM -> aggT directly in [hid, node]
    layout (no per-block transposes; the mean's 1/count is pre-applied
    per-edge in the h2 exit copy).
  - Node MLP on the local shard; output stays transposed [feat, node] and
    is un-transposed on host.
"""

import math
import sys
from contextlib import ExitStack

sys.path.insert(0, "/opt/trn_rl_repo")

import ml_dtypes
import numpy as np

import concourse.bass as bass
import concourse.tile as tile
from concourse import bacc, mybir
from concourse.bass_utils import run_bass_kernel_spmd

NCORES = 8
P = 128
FN = 512    # node feature dim
FE = 128    # edge feature dim
HID = 1280  # edge-MLP hidden/output dim
F32 = mybir.dt.float32
BF16 = mybir.dt.bfloat16
F32R = mybir.dt.float32r
I32 = mybir.dt.int32
RELU = mybir.ActivationFunctionType.Relu
IDENT = mybir.ActivationFunctionType.Identity
NPBF16 = ml_dtypes.bfloat16

_prog_cache = {}


def _build(EC, NB, KB, NX):
    """Build the SPMD program for one core.

    EC: edge chunks (128 edges each) per core, multiple of 4.
    NB: node blocks (128 nodes each) per core, multiple of 4.
    KB: max edge chunks per node block (scatter schedule width).
    NX: number of rows of the replicated x (gather source).
    """
    EP = EC * P
    SC = EC // 4   # superchunks of 512 edges
    NSB = NB // 4  # superblocks of 512 nodes

    nc = bacc.Bacc("TRN2", target_bir_lowering=False, debug=False,
                   num_devices=NCORES)

    x_d = nc.dram_tensor("x", [NX, FN], BF16, kind="ExternalInput")
    rows_d = nc.dram_tensor("rows", [P, EC], I32, kind="ExternalInput")
    eaT_d = nc.dram_tensor("eaT", [FE, EP], BF16, kind="ExternalInput")
    W1_d = nc.dram_tensor("W1", [FN + FE, HID], BF16, kind="ExternalInput")
    W2_d = nc.dram_tensor("W2", [HID, HID], BF16, kind="ExternalInput")
    W3_d = nc.dram_tensor("W3", [FN + HID, FN + FE], BF16, kind="ExternalInput")
    W4_d = nc.dram_tensor("W4", [FN + FE, FN], BF16, kind="ExternalInput")
    b1_d = nc.dram_tensor("b1", [P, HID // P], F32, kind="ExternalInput")
    b2_d = nc.dram_tensor("b2", [P, HID // P], F32, kind="ExternalInput")
    b3_d = nc.dram_tensor("b3", [P, (FN + FE) // P], F32, kind="ExternalInput")
    b4_d = nc.dram_tensor("b4", [P, FN // P], F32, kind="ExternalInput")
    gid_d = nc.dram_tensor("gid", [P, NB * KB], I32, kind="ExternalInput")
    colb_d = nc.dram_tensor("colb", [P, NB * KB], BF16, kind="ExternalInput")
    invce_d = nc.dram_tensor("invce", [P, EC], BF16, kind="ExternalInput")
    xsT_d = nc.dram_tensor("xsT", [FN, NB * P], BF16, kind="ExternalInput")
    iota_d = nc.dram_tensor("iota", [P, P], BF16, kind="ExternalInput")
    ident_d = nc.dram_tensor("ident", [P, P], BF16, kind="ExternalInput")
    outT_d = nc.dram_tensor("outT", [FN, NB * P], F32, kind="ExternalOutput")
    h2_d = nc.dram_tensor("h2buf", [EP, HID], BF16)  # internal staging

    with tile.TileContext(nc) as tc, ExitStack() as ctx:
        cpool = ctx.enter_context(tc.tile_pool(name="const", bufs=1))

        identt = cpool.tile([P, P], BF16)
        nc.sync.dma_start(identt[:], ident_d.ap()[:])
        iotat = cpool.tile([P, P], BF16)
        nc.sync.dma_start(iotat[:], iota_d.ap()[:])
        b1t = cpool.tile([P, HID // P], F32)
        nc.sync.dma_start(b1t[:], b1_d.ap()[:])
        b2t = cpool.tile([P, HID // P], F32)
        nc.sync.dma_start(b2t[:], b2_d.ap()[:])
        b3t = cpool.tile([P, (FN + FE) // P], F32)
        nc.sync.dma_start(b3t[:], b3_d.ap()[:])
        b4t = cpool.tile([P, FN // P], F32)
        nc.sync.dma_start(b4t[:], b4_d.ap()[:])
        rowst = cpool.tile([P, EC], I32)
        nc.sync.dma_start(rowst[:], rows_d.ap()[:])
        gidt = cpool.tile([P, NB * KB], I32)
        nc.sync.dma_start(gidt[:], gid_d.ap()[:])
        colbt = cpool.tile([P, NB * KB], BF16)
        nc.sync.dma_start(colbt[:], colb_d.ap()[:])
        invcet = cpool.tile([P, EC], BF16)
        nc.sync.dma_start(invcet[:], invce_d.ap()[:])

        # All weights loaded up front (W3/W4 early kills the phase-boundary
        # stall seen in the fp32r baseline's trace).
        wpool = ctx.enter_context(tc.tile_pool(name="wts", bufs=1))
        W1t = wpool.tile([P, 5, HID], BF16)
        W1r = W1_d.ap().rearrange("(ko ki) m -> ki ko m", ki=P)
        for k in range(5):
            nc.sync.dma_start(W1t[:, k, :], W1r[:, k, :])
        W2t = wpool.tile([P, 10, HID], BF16)
        W2r = W2_d.ap().rearrange("(ko ki) m -> ki ko m", ki=P)
        for k in range(10):
            nc.sync.dma_start(W2t[:, k, :], W2r[:, k, :])
        W3t = wpool.tile([P, 14, FN + FE], BF16)
        nc.sync.dma_start(
            W3t[:], W3_d.ap().rearrange("(ko ki) m -> ki ko m", ki=P))
        W4t = wpool.tile([P, 5, FN], BF16)
        nc.sync.dma_start(
            W4t[:], W4_d.ap().rearrange("(ko ki) m -> ki ko m", ki=P))

        # ---------------- Phase E: edge MLP ----------------
        # Transposes run in PE transpose-mode, which does NOT count as
        # PE activity for the HAM clock gate: a burst of back-to-back
        # transposes >3.4us re-throttles the PE to 1.2 GHz.  All
        # transposes are therefore interleaved between matmul groups,
        # and gathers are pipelined one superchunk ahead.
        with ExitStack() as ectx:
            ptp = ectx.enter_context(
                tc.tile_pool(name="ptp", bufs=2, space="PSUM"))
            xgp = ectx.enter_context(tc.tile_pool(name="xg", bufs=2))
            xgTp = ectx.enter_context(tc.tile_pool(name="xgT", bufs=2))
            eap = ectx.enter_context(tc.tile_pool(name="ea", bufs=2))
            h1p = ectx.enter_context(tc.tile_pool(name="h1T", bufs=1))
            h2Tp = ectx.enter_context(tc.tile_pool(name="h2T", bufs=1))
            h2op = ectx.enter_context(tc.tile_pool(name="h2o", bufs=4))
            mmp = ectx.enter_context(
                tc.tile_pool(name="mmE", bufs=2, space="PSUM"))

            def issue_gather(sc):
                xgt = xgp.tile([P, 4, FN], BF16)
                for k in range(4):
                    nc.gpsimd.indirect_dma_start(
                        out=xgt[:, k, :], out_offset=None, in_=x_d.ap()[:],
                        in_offset=bass.IndirectOffsetOnAxis(
                            ap=rowst[:, sc * 4 + k:sc * 4 + k + 1], axis=0))
                eat = eap.tile([P, 512], BF16)
                nc.sync.dma_start(
                    eat[:], eaT_d.ap()[:, sc * 512:(sc + 1) * 512])
                return xgt, eat

            def entry_T(xgt, xgTt, f, k):
                pt = ptp.tile([P, P], BF16)
                nc.tensor.transpose(
                    pt[:], xgt[:, k, f * P:(f + 1) * P], identt[:])
                nc.vector.tensor_copy(xgTt[:, f, k * P:(k + 1) * P], pt[:])

            # prologue: superchunk 0 input + its entry transposes
            xg_cur, ea_cur = issue_gather(0)
            xgT_cur = xgTp.tile([P, 4, 512], BF16)
            for f in range(4):
                for k in range(4):
                    entry_T(xg_cur, xgT_cur, f, k)

            for sc in range(SC):
                if sc + 1 < SC:
                    xg_next, ea_next = issue_gather(sc + 1)
                    xgT_next = xgTp.tile([P, 4, 512], BF16)
                else:
                    xg_next = ea_next = xgT_next = None

                h1Tt = h1p.tile([P, 10, 512], BF16)
                for of in range(10):
                    ps = mmp.tile([P, 512], F32)
                    for k in range(5):
                        rhs = xgT_cur[:, k, :] if k < 4 else ea_cur[:]
                        nc.tensor.matmul(
                            ps[:], W1t[:, k, of * P:(of + 1) * P], rhs,
                            start=(k == 0), stop=(k == 4))
                    nc.scalar.activation(h1Tt[:, of, :], ps[:], RELU,
                                         bias=b1t[:, of:of + 1])
                h2Tt = h2Tp.tile([P, 10, 512], BF16)
                h2ot = [h2op.tile([P, HID], BF16, name=f"h2o_{sc}_{k}", tag="h2o")
                         for k in range(4)]
                for of in range(10):
                    ps = mmp.tile([P, 512], F32)
                    for k in range(10):
                        nc.tensor.matmul(
                            ps[:], W2t[:, k, of * P:(of + 1) * P],
                            h1Tt[:, k, :], start=(k == 0), stop=(k == 9))
                    nc.scalar.activation(
                        h2Tt[:, of, :], ps[:], IDENT, bias=b2t[:, of:of + 1])
                    # interleave: this of-chunk's exit transposes, with the
                    # per-edge 1/count of the destination folded into the
                    # PSUM->SBUF copy so the scatter can accumulate raw sums.
                    for k in range(4):
                        c = sc * 4 + k
                        pt = ptp.tile([P, P], BF16)
                        nc.tensor.transpose(
                            pt[:], h2Tt[:, of, k * P:(k + 1) * P], identt[:])
                        nc.vector.tensor_tensor(
                            h2ot[k][:, of * P:(of + 1) * P], pt[:],
                            invcet[:, c:c + 1].to_broadcast([P, P]),
                            op=mybir.AluOpType.mult)
                    # interleave: next superchunk's entry transposes
                    if xgT_next is not None and of < 8:
                        for k in range(2):
                            entry_T(xg_next, xgT_next, of // 2, (of % 2) * 2 + k)
                for k in range(4):
                    r0 = sc * 512 + k * P
                    nc.sync.dma_start(h2_d.ap()[r0:r0 + P, :], h2ot[k][:])
                xg_cur, ea_cur, xgT_cur = xg_next, ea_next, xgT_next

        # ---------------- Phases S+N: scatter-mean + node MLP ----------------
        with ExitStack() as sctx:
            h2gp = sctx.enter_context(tc.tile_pool(name="h2g", bufs=7))
            Sp = sctx.enter_context(tc.tile_pool(name="Smat", bufs=3))
            aggTp = sctx.enter_context(tc.tile_pool(name="aggT", bufs=2))
            xsp = sctx.enter_context(tc.tile_pool(name="xs", bufs=2))
            h3p = sctx.enter_context(tc.tile_pool(name="h3T", bufs=1))
            oTp = sctx.enter_context(tc.tile_pool(name="oT", bufs=2))
            smp = sctx.enter_context(
                tc.tile_pool(name="smp", bufs=2, space="PSUM"))
            mmp2 = sctx.enter_context(
                tc.tile_pool(name="mmN", bufs=2, space="PSUM"))

            # Rolling gather lookahead: block b's h2-row gathers (slow,
            # gpsimd SW-DGE) are issued two blocks ahead of its scatter
            # matmuls so the PE never waits on them.  Pad slots carry an
            # out-of-bounds id and are silently skipped by the DMA
            # (bounds_check), so padding costs no gather bandwidth.
            pend_gs = {}

            def gather_S(b):
                lst = []
                for k in range(KB):
                    c = b * KB + k
                    h2g = h2gp.tile([P, HID], BF16, name=f"h2g_{b}_{k}",
                                    tag="h2g")
                    nc.gpsimd.indirect_dma_start(
                        out=h2g[:], out_offset=None, in_=h2_d.ap()[:],
                        in_offset=bass.IndirectOffsetOnAxis(
                            ap=gidt[:, c:c + 1], axis=0),
                        bounds_check=EP - 1, oob_is_err=False)
                    St = Sp.tile([P, P], BF16, name=f"S_{b}_{k}", tag="S")
                    nc.vector.tensor_tensor(
                        St[:], colbt[:, c:c + 1].to_broadcast([P, P]),
                        iotat[:], op=mybir.AluOpType.is_equal)
                    lst.append((h2g, St))
                pend_gs[b] = lst

            gather_S(0)
            gather_S(1)

            def load_xst(s):
                xst = xsp.tile([P, 4, 512], BF16, name=f"xst_{s}", tag="xst")
                nc.sync.dma_start(
                    xst[:],
                    xsT_d.ap().rearrange("(fo fi) n -> fi fo n", fi=P)
                    [:, :, s * 512:(s + 1) * 512])
                return xst

            xst_cur = load_xst(0)
            outTr = outT_d.ap().rearrange("(fo fi) n -> fi fo n", fi=P)
            for s in range(NSB):
                # scatter: accumulate aggT[hid, node] directly in PSUM with
                # the gathered h2 rows stationary and the one-hot S moving.
                aggTsb = aggTp.tile([P, 10, 512], BF16)
                for bb in range(4):
                    b = s * 4 + bb
                    if b + 2 < NB:
                        gather_S(b + 2)
                    pss = smp.tile([P, 10, P], F32)
                    gs = pend_gs.pop(b)
                    # j-major so each 128-wide accumulation group's matmuls
                    # are consecutive: open groups must not interleave within
                    # a PSUM bank.
                    for j in range(10):
                        for k, (h2g, St) in enumerate(gs):
                            nc.tensor.matmul(
                                pss[:, j, :], h2g[:, j * P:(j + 1) * P], St[:],
                                start=(k == 0), stop=(k == KB - 1))
                    nc.vector.tensor_copy(
                        aggTsb[:, :, bb * P:(bb + 1) * P], pss[:])

                xst = xst_cur
                xst_cur = load_xst(s + 1) if s + 1 < NSB else None
                h3Tt = h3p.tile([P, 5, 512], BF16)
                for of in range(5):
                    ps = mmp2.tile([P, 512], F32)
                    for k in range(4):
                        nc.tensor.matmul(
                            ps[:], W3t[:, k, of * P:(of + 1) * P],
                            xst[:, k, :], start=(k == 0), stop=False)
                    for f in range(10):
                        nc.tensor.matmul(
                            ps[:], W3t[:, 4 + f, of * P:(of + 1) * P],
                            aggTsb[:, f, :], start=False, stop=(f == 9))
                    nc.scalar.activation(h3Tt[:, of, :], ps[:], RELU,
                                         bias=b3t[:, of:of + 1])
                oTt = oTp.tile([P, 4, 512], F32)
                for of in range(4):
                    ps = mmp2.tile([P, 512], F32)
                    for k in range(5):
                        nc.tensor.matmul(
                            ps[:], W4t[:, k, of * P:(of + 1) * P],
                            h3Tt[:, k, :], start=(k == 0), stop=(k == 4))
                    nc.scalar.activation(
                        oTt[:, of, :], ps[:], IDENT, bias=b4t[:, of:of + 1])
                nc.sync.dma_start(
                    outTr[:, :, s * 512:(s + 1) * 512], oTt[:])

    nc.compile()
    return nc


def _prepare(x, row, col, ea):
    """Host-side sharding: sort edges by destination, split nodes into 8
    block-aligned edge-balanced shards, build per-core arrays."""
    N = x.shape[0]
    E = ea.shape[0]
    order = np.argsort(col, kind="stable")
    scol = col[order]
    srow = row[order]
    NBLK = (N + P - 1) // P
    NTOT = NBLK * P

    bounds = [0]
    for p in range(1, NCORES):
        if E > 0:
            t = int(scol[min((p * E) // NCORES, E - 1)])
        else:
            t = (p * NTOT) // NCORES
        b = int(round(t / P)) * P
        b = max(b, bounds[-1] + P)
        b = min(b, NTOT - P * (NCORES - p))
        bounds.append(b)
    bounds.append(NTOT)
    for p in range(1, NCORES + 1):
        assert bounds[p] > bounds[p - 1], f"degenerate shard bounds {bounds}"

    e_split = np.searchsorted(scol, bounds)
    Ec = np.diff(e_split)
    EC = max(4, math.ceil(int(Ec.max()) / P))
    EC = ((EC + 3) // 4) * 4
    EP = EC * P
    nblk = [(bounds[p + 1] - bounds[p]) // P for p in range(NCORES)]
    NB = max(4, ((max(nblk) + 3) // 4) * 4)
    NBP = NB * P
    blkdeg = np.bincount(scol // P, minlength=NBLK)
    KB = max(1, math.ceil(int(blkdeg.max()) / P))

    xbf = np.zeros((NTOT, FN), NPBF16)
    xbf[:N] = x.astype(NPBF16)
    xpadT = np.zeros((FN, NTOT + NBP), NPBF16)
    xpadT[:, :N] = xbf[:N].T

    cnt_all = np.bincount(scol, minlength=NTOT)

    cores = []
    for p in range(NCORES):
        s, e = int(e_split[p]), int(e_split[p + 1])
        n0 = bounds[p]
        ne = e - s
        tmp = np.zeros(EP, np.int32)
        tmp[:ne] = srow[s:e]
        rows_t = np.ascontiguousarray(tmp.reshape(EC, P).T)
        eaT = np.zeros((FE, EP), NPBF16)
        eaT[:, :ne] = ea[order[s:e]].T.astype(NPBF16)
        lcol = (scol[s:e] - n0).astype(np.int64)
        bstart = np.searchsorted(lcol, np.arange(NB + 1) * P)
        gid = np.full((NB, KB, P), 1 << 30, np.int32)
        gid.reshape(NB * KB, P)[:7] = 0
        colb = np.full((NB, KB, P), -1.0, np.float32)
        for b in range(NB):
            sb, eb = int(bstart[b]), int(bstart[b + 1])
            cnt = eb - sb
            assert cnt <= KB * P
            gid[b].reshape(-1)[:cnt] = np.arange(sb, eb, dtype=np.int32)
            colb[b].reshape(-1)[:cnt] = (lcol[sb:eb] - b * P)
        gid_t = np.ascontiguousarray(gid.reshape(NB * KB, P).T)
        colb_t = np.ascontiguousarray(
            colb.reshape(NB * KB, P).T.astype(NPBF16))
        # per-edge 1/count of the destination node (0 for pad edges so
        # their staged h2 rows are exactly zero)
        invce = np.zeros(EP, np.float32)
        invce[:ne] = 1.0 / np.maximum(cnt_all[scol[s:e]], 1.0)
        invce_t = np.ascontiguousarray(invce.reshape(EC, P).T.astype(NPBF16))
        xsT = np.ascontiguousarray(xpadT[:, n0:n0 + NBP])
        cores.append(dict(rows=rows_t, eaT=eaT, gid=gid_t, colb=colb_t,
                          invce=invce_t, xsT=xsT))
    return cores, bounds, EC, NB, KB, xbf


def _run(inputs, trace=False):
    x = np.ascontiguousarray(np.asarray(inputs["x"], dtype=np.float32))
    ei = np.asarray(inputs["edge_index"])
    ea = np.ascontiguousarray(np.asarray(inputs["edge_attr"], dtype=np.float32))
    row = ei[0].astype(np.int64)
    col = ei[1].astype(np.int64)
    W1 = np.asarray(inputs["W1"], np.float32).astype(NPBF16)
    W2 = np.asarray(inputs["W2"], np.float32).astype(NPBF16)
    W3 = np.asarray(inputs["W3"], np.float32).astype(NPBF16)
    W4 = np.asarray(inputs["W4"], np.float32).astype(NPBF16)
    b1 = np.asarray(inputs["b1"], np.float32)
    b2 = np.asarray(inputs["b2"], np.float32)
    b3 = np.asarray(inputs["b3"], np.float32)
    b4 = np.asarray(inputs["b4"], np.float32)
    N = x.shape[0]

    cores, bounds, EC, NB, KB, xbf = _prepare(x, row, col, ea)

    key = (EC, NB, KB, xbf.shape[0])
    if key not in _prog_cache:
        _prog_cache[key] = _build(EC, NB, KB, xbf.shape[0])
    nc = _prog_cache[key]

    b1t = np.ascontiguousarray(b1.reshape(HID // P, P).T)
    b2t = np.ascontiguousarray(b2.reshape(HID // P, P).T)
    b3t = np.ascontiguousarray(b3.reshape((FN + FE) // P, P).T)
    b4t = np.ascontiguousarray(b4.reshape(FN // P, P).T)
    iota = np.ascontiguousarray(
        np.broadcast_to(np.arange(P, dtype=np.float32), (P, P))).astype(NPBF16)
    ident = np.eye(P, dtype=np.float32).astype(NPBF16)

    in_maps = []
    for p in range(NCORES):
        c = cores[p]
        in_maps.append({
            "x": xbf, "rows": c["rows"], "eaT": c["eaT"],
            "W1": W1, "W2": W2, "W3": W3, "W4": W4,
            "b1": b1t, "b2": b2t, "b3": b3t, "b4": b4t,
            "gid": c["gid"], "colb": c["colb"], "invce": c["invce"],
            "xsT": c["xsT"], "iota": iota, "ident": ident,
        })

    res = run_bass_kernel_spmd(nc, in_maps, list(range(NCORES)), trace=trace)

    out = np.empty((N, FN), np.float32)
    for p in range(NCORES):
        n0, n1 = bounds[p], min(bounds[p + 1], N)
        if n1 > n0:
            out[n0:n1] = res.results[p]["outT"].T[:n1 - n0]
    return out, res


def kernel(**inputs) -> np.ndarray:
    out, _ = _run(inputs, trace=False)
    return out


# revision 18
# speedup vs baseline: 1.1321x; 1.0011x over previous
"""Trainium2 Bass kernel for nn_NodeModel (GNN message passing).

Reference computation:
    h   = relu(concat(x[row], edge_attr) @ W1 + b1) @ W2 + b2     # edge MLP
    agg = scatter_mean(h, col, N)                                  # per-dest mean
    out = relu(concat(x, agg) @ W3 + b3) @ W4 + b4                 # node MLP

Distribution strategy (8 cores, no collectives needed):
  - Sort edges by destination node; split destination nodes into 8
    block-aligned, edge-balanced shards.  Each core owns one node shard and
    ALL edges targeting it, so per-node sums are complete locally.
  - x is replicated; each core gathers x[row] for its edges with indirect
    DMA on-device.
  - All matmul operands are bf16 (fp32 PSUM accumulation): halves HBM
    traffic vs fp32r and speeds PE transposes 1.5x.
  - Edge MLP runs with weights stationary and activations kept transposed
    [feat, edge]; h2 rows (pre-scaled by 1/count of their destination) are
    staged to DRAM in bf16.
  - Scatter-mean per 128-node block: indirect-gather the block's h2 rows,
    build a one-hot selection matrix with is_equal against an iota, and
    matmul-accumulate h2^T @ S in PSUM -> aggT directly in [hid, node]
    layout (the mean's 1/count is pre-applied per-edge in the h2 exit copy).
  - The scatter + node-MLP work is INTERLEAVED into the edge phase: since
    edges are sorted by destination, node block b only needs the first
    bcut[b] edge superchunks.  A static schedule (max over cores, so the
    SPMD program is uniform) runs each superblock as soon as its edges are
    done, overlapping the h2 gather-back DMA with edge-MLP compute.
  - Node-MLP output stays transposed [feat, node]; un-transposed on host.
"""

import math
import sys
from contextlib import ExitStack

sys.path.insert(0, "/opt/trn_rl_repo")

import ml_dtypes
import numpy as np

import concourse.bass as bass
import concourse.tile as tile
from concourse import bacc, mybir
from concourse.bass_utils import run_bass_kernel_spmd

NCORES = 8
P = 128
FN = 512    # node feature dim
FE = 128    # edge feature dim
HID = 1280  # edge-MLP hidden/output dim
F32 = mybir.dt.float32
BF16 = mybir.dt.bfloat16
I32 = mybir.dt.int32
RELU = mybir.ActivationFunctionType.Relu
IDENT = mybir.ActivationFunctionType.Identity
NPBF16 = ml_dtypes.bfloat16

_prog_cache = {}


def _build(EC, NB, KB, NX, bcut):
    """Build the SPMD program for one core.

    EC: edge chunks (128 edges each) per core, multiple of 4.
    NB: node blocks (128 nodes each) per core, multiple of 4.
    KB: max edge chunks per node block (scatter schedule width).
    NX: number of rows of the replicated x (gather source).
    bcut: per node block, the number of edge superchunks that must be
          complete before its h2 rows exist (max over cores -> uniform).
    """
    EP = EC * P
    SC = EC // 4   # superchunks of 512 edges
    NSB = NB // 4  # superblocks of 512 nodes
    LOOKAHEAD = max(2, 12 // KB)  # h2-gather prefetch blocks (SBUF-bounded)

    nc = bacc.Bacc("TRN2", target_bir_lowering=False, debug=False,
                   num_devices=NCORES)

    x_d = nc.dram_tensor("x", [NX, FN], BF16, kind="ExternalInput")
    rows_d = nc.dram_tensor("rows", [P, EC], I32, kind="ExternalInput")
    eaT_d = nc.dram_tensor("eaT", [FE, EP], BF16, kind="ExternalInput")
    W1_d = nc.dram_tensor("W1", [FN + FE, HID], BF16, kind="ExternalInput")
    W2_d = nc.dram_tensor("W2", [HID, HID], BF16, kind="ExternalInput")
    W3_d = nc.dram_tensor("W3", [FN + HID, FN + FE], BF16, kind="ExternalInput")
    W4_d = nc.dram_tensor("W4", [FN + FE, FN], BF16, kind="ExternalInput")
    b1_d = nc.dram_tensor("b1", [P, HID // P], F32, kind="ExternalInput")
    b2_d = nc.dram_tensor("b2", [P, HID // P], F32, kind="ExternalInput")
    b3_d = nc.dram_tensor("b3", [P, (FN + FE) // P], F32, kind="ExternalInput")
    b4_d = nc.dram_tensor("b4", [P, FN // P], F32, kind="ExternalInput")
    gid_d = nc.dram_tensor("gid", [P, NB * KB], I32, kind="ExternalInput")
    colb_d = nc.dram_tensor("colb", [P, NB * KB], BF16, kind="ExternalInput")
    invce_d = nc.dram_tensor("invce", [P, EC], BF16, kind="ExternalInput")
    xsT_d = nc.dram_tensor("xsT", [FN, NB * P], BF16, kind="ExternalInput")
    iota_d = nc.dram_tensor("iota", [P, P], BF16, kind="ExternalInput")
    ident_d = nc.dram_tensor("ident", [P, P], BF16, kind="ExternalInput")
    outT_d = nc.dram_tensor("outT", [FN, NB * P], F32, kind="ExternalOutput")
    h2_d = nc.dram_tensor("h2buf", [EP, HID], BF16)  # internal staging

    with tile.TileContext(nc) as tc, ExitStack() as ctx:
        cpool = ctx.enter_context(tc.tile_pool(name="const", bufs=1))

        identt = cpool.tile([P, P], BF16)
        nc.sync.dma_start(identt[:], ident_d.ap()[:])
        iotat = cpool.tile([P, P], BF16)
        nc.sync.dma_start(iotat[:], iota_d.ap()[:])
        b1t = cpool.tile([P, HID // P], F32)
        nc.sync.dma_start(b1t[:], b1_d.ap()[:])
        b2t = cpool.tile([P, HID // P], F32)
        nc.sync.dma_start(b2t[:], b2_d.ap()[:])
        b3t = cpool.tile([P, (FN + FE) // P], F32)
        nc.sync.dma_start(b3t[:], b3_d.ap()[:])
        b4t = cpool.tile([P, FN // P], F32)
        nc.sync.dma_start(b4t[:], b4_d.ap()[:])
        rowst = cpool.tile([P, EC], I32)
        nc.sync.dma_start(rowst[:], rows_d.ap()[:])
        gidt = cpool.tile([P, NB * KB], I32)
        nc.sync.dma_start(gidt[:], gid_d.ap()[:])
        colbt = cpool.tile([P, NB * KB], BF16)
        nc.sync.dma_start(colbt[:], colb_d.ap()[:])
        invcet = cpool.tile([P, EC], BF16)
        nc.sync.dma_start(invcet[:], invce_d.ap()[:])

        # Weights: W1 split per contraction chunk so the first matmul only
        # waits for 1/5 of it; W3/W4 up front so the node MLP never stalls.
        wpool = ctx.enter_context(tc.tile_pool(name="wts", bufs=1))
        W1r = W1_d.ap().rearrange("(ko ki) m -> ki ko m", ki=P)
        W1t = []
        for k in range(5):
            w = wpool.tile([P, HID], BF16, name=f"W1_{k}", tag=f"W1_{k}")
            nc.sync.dma_start(w[:], W1r[:, k, :])
            W1t.append(w)
        W2t = wpool.tile([P, 10, HID], BF16)
        W2r = W2_d.ap().rearrange("(ko ki) m -> ki ko m", ki=P)
        for k in range(10):
            nc.sync.dma_start(W2t[:, k, :], W2r[:, k, :])
        W3t = wpool.tile([P, 14, FN + FE], BF16)
        nc.sync.dma_start(
            W3t[:], W3_d.ap().rearrange("(ko ki) m -> ki ko m", ki=P))
        W4t = wpool.tile([P, 5, FN], BF16)
        nc.sync.dma_start(
            W4t[:], W4_d.ap().rearrange("(ko ki) m -> ki ko m", ki=P))

        # ---- pools (all coexist: phases are interleaved) ----
        # PSUM is 8 banks of 2 KB, allocated per tile name at bank
        # granularity: mm 2x[P,512]f32 = 2 banks; smp holds the scatter
        # half-accumulator [P,5,P]f32 (2 banks) and the 4-slot transpose
        # staging tile [P,512]bf16 (1 bank) -> 2 + 2*(2+1) = 8.
        mmp = ctx.enter_context(tc.tile_pool(name="mm", bufs=2, space="PSUM"))
        smp = ctx.enter_context(tc.tile_pool(name="smp", bufs=2, space="PSUM"))

        def pt4():
            return smp.tile([P, 512], BF16, name="pt4", tag="pt4")
        xgp = ctx.enter_context(tc.tile_pool(name="xg", bufs=2))
        xgTp = ctx.enter_context(tc.tile_pool(name="xgT", bufs=2))
        eap = ctx.enter_context(tc.tile_pool(name="ea", bufs=2))
        h1p = ctx.enter_context(tc.tile_pool(name="h1T", bufs=1))
        h2Tp = ctx.enter_context(tc.tile_pool(name="h2T", bufs=1))
        h2op = ctx.enter_context(tc.tile_pool(name="h2o", bufs=4))
        h2gp = ctx.enter_context(
            tc.tile_pool(name="h2g", bufs=(LOOKAHEAD + 1) * KB))
        Sp = ctx.enter_context(
            tc.tile_pool(name="Smat", bufs=(LOOKAHEAD + 1) * KB))
        aggTp = ctx.enter_context(tc.tile_pool(name="aggT", bufs=2))
        xsp = ctx.enter_context(tc.tile_pool(name="xs", bufs=2))
        h3p = ctx.enter_context(tc.tile_pool(name="h3T", bufs=1))
        oTp = ctx.enter_context(tc.tile_pool(name="oT", bufs=1))

        # ---------------- edge-phase helpers ----------------
        def issue_gather(sc):
            xgt = xgp.tile([P, 4, FN], BF16)
            for k in range(4):
                nc.gpsimd.indirect_dma_start(
                    out=xgt[:, k, :], out_offset=None, in_=x_d.ap()[:],
                    in_offset=bass.IndirectOffsetOnAxis(
                        ap=rowst[:, sc * 4 + k:sc * 4 + k + 1], axis=0))
            eat = eap.tile([P, 512], BF16)
            nc.sync.dma_start(
                eat[:], eaT_d.ap()[:, sc * 512:(sc + 1) * 512])
            return xgt, eat

        def entry_T2(xgt, xgTt, pairs):
            """Transpose up to 4 (f, k) entries through one pt4 staging tile."""
            pt = pt4()
            for i, (f, k) in enumerate(pairs):
                nc.tensor.transpose(
                    pt[:, i * P:(i + 1) * P],
                    xgt[:, k, f * P:(f + 1) * P], identt[:])
                nc.vector.tensor_copy(
                    xgTt[:, f, k * P:(k + 1) * P], pt[:, i * P:(i + 1) * P])

        # ---------------- scatter/node-phase helpers ----------------
        pend_gs = {}
        state = dict(g_next=0, b_next=0, s_next=0, sc_done=0,
                     xg_cur=None, ea_cur=None, xgT_cur=None, xst_cur=None)

        def gather_S(b):
            ext = max(bcut[b], 1) * 512  # h2 rows that exist by then
            lst = []
            for k in range(KB):
                c = b * KB + k
                # pad slots carry id 0 (not OOB-skip): every partition of the
                # tile gets written with finite data, so the zero one-hot
                # columns can never multiply stale NaN bit patterns.
                h2g = h2gp.tile([P, HID], BF16, name=f"h2g_{b}_{k}",
                                tag="h2g")
                nc.gpsimd.indirect_dma_start(
                    out=h2g[:], out_offset=None, in_=h2_d.ap()[:ext],
                    in_offset=bass.IndirectOffsetOnAxis(
                        ap=gidt[:, c:c + 1], axis=0))
                St = Sp.tile([P, P], BF16, name=f"S_{b}_{k}", tag="S")
                nc.vector.tensor_tensor(
                    St[:], colbt[:, c:c + 1].to_broadcast([P, P]),
                    iotat[:], op=mybir.AluOpType.is_equal)
                lst.append((h2g, St))
            pend_gs[b] = lst

        def try_gathers():
            while (state["g_next"] < NB
                   and bcut[state["g_next"]] <= state["sc_done"]
                   and state["g_next"] < state["b_next"] + LOOKAHEAD):
                gather_S(state["g_next"])
                state["g_next"] += 1

        def load_xst(s):
            xst = xsp.tile([P, 4, 512], BF16, name=f"xst_{s}", tag="xst")
            nc.sync.dma_start(
                xst[:],
                xsT_d.ap().rearrange("(fo fi) n -> fi fo n", fi=P)
                [:, :, s * 512:(s + 1) * 512])
            return xst

        outTr = outT_d.ap().rearrange("(fo fi) n -> fi fo n", fi=P)

        def emit_superblock(s):
            # scatter: accumulate aggT[hid, node] directly in PSUM with the
            # gathered h2 rows stationary and the one-hot S moving.
            aggTsb = aggTp.tile([P, 10, 512], BF16)
            for bb in range(4):
                b = s * 4 + bb
                gs = pend_gs.pop(b)
                # j-major: each 128-wide accumulation group's matmuls stay
                # consecutive (open groups must not interleave in a bank).
                for half in range(2):
                    pss = smp.tile([P, 5, P], F32, name="pss", tag="pss")
                    for j5 in range(5):
                        j = half * 5 + j5
                        for k, (h2g, St) in enumerate(gs):
                            nc.tensor.matmul(
                                pss[:, j5, :], h2g[:, j * P:(j + 1) * P],
                                St[:], start=(k == 0), stop=(k == KB - 1))
                    nc.vector.tensor_copy(
                        aggTsb[:, half * 5:(half + 1) * 5,
                               bb * P:(bb + 1) * P], pss[:])
                state["b_next"] = b + 1
                try_gathers()

            xst = state["xst_cur"]
            state["xst_cur"] = load_xst(s + 1) if s + 1 < NSB else None
            h3Tt = h3p.tile([P, 5, 512], BF16)
            for of in range(5):
                ps = mmp.tile([P, 512], F32)
                for k in range(4):
                    nc.tensor.matmul(
                        ps[:], W3t[:, k, of * P:(of + 1) * P],
                        xst[:, k, :], start=(k == 0), stop=False)
                for f in range(10):
                    nc.tensor.matmul(
                        ps[:], W3t[:, 4 + f, of * P:(of + 1) * P],
                        aggTsb[:, f, :], start=False, stop=(f == 9))
                nc.scalar.activation(h3Tt[:, of, :], ps[:], RELU,
                                     bias=b3t[:, of:of + 1])
            oTt = oTp.tile([P, 4, 512], F32)
            for of in range(4):
                ps = mmp.tile([P, 512], F32)
                for k in range(5):
                    nc.tensor.matmul(
                        ps[:], W4t[:, k, of * P:(of + 1) * P],
                        h3Tt[:, k, :], start=(k == 0), stop=(k == 4))
                nc.scalar.activation(
                    oTt[:, of, :], ps[:], IDENT, bias=b4t[:, of:of + 1])
            nc.sync.dma_start(outTr[:, :, s * 512:(s + 1) * 512], oTt[:])

        # ---------------- interleaved main loop ----------------
        # prologue: superchunk 0 input + its entry transposes + first xst
        xg_cur, ea_cur = issue_gather(0)
        xgT_cur = xgTp.tile([P, 4, 512], BF16)
        for f in range(4):
            entry_T2(xg_cur, xgT_cur, [(f, k) for k in range(4)])
        state["xst_cur"] = load_xst(0)

        for sc in range(SC):
            if sc + 1 < SC:
                xg_next, ea_next = issue_gather(sc + 1)
                xgT_next = xgTp.tile([P, 4, 512], BF16)
            else:
                xg_next = ea_next = xgT_next = None

            h1Tt = h1p.tile([P, 10, 512], BF16)
            for of in range(10):
                ps = mmp.tile([P, 512], F32)
                for k in range(5):
                    rhs = xgT_cur[:, k, :] if k < 4 else ea_cur[:]
                    nc.tensor.matmul(
                        ps[:], W1t[k][:, of * P:(of + 1) * P], rhs,
                        start=(k == 0), stop=(k == 4))
                nc.scalar.activation(h1Tt[:, of, :], ps[:], RELU,
                                     bias=b1t[:, of:of + 1])
            h2Tt = h2Tp.tile([P, 10, 512], BF16)
            h2ot = [h2op.tile([P, HID], BF16, name=f"h2o_{sc}_{k}", tag="h2o")
                     for k in range(4)]
            for of in range(10):
                ps = mmp.tile([P, 512], F32)
                for k in range(10):
                    nc.tensor.matmul(
                        ps[:], W2t[:, k, of * P:(of + 1) * P],
                        h1Tt[:, k, :], start=(k == 0), stop=(k == 9))
                nc.scalar.activation(
                    h2Tt[:, of, :], ps[:], IDENT, bias=b2t[:, of:of + 1])
                # interleave: this of-chunk's exit transposes, with the
                # per-edge 1/count of the destination folded into the
                # PSUM->SBUF copy so the scatter can accumulate raw sums.
                pt = pt4()
                for k in range(4):
                    c = sc * 4 + k
                    nc.tensor.transpose(
                        pt[:, k * P:(k + 1) * P],
                        h2Tt[:, of, k * P:(k + 1) * P], identt[:])
                    nc.vector.tensor_tensor(
                        h2ot[k][:, of * P:(of + 1) * P],
                        pt[:, k * P:(k + 1) * P],
                        invcet[:, c:c + 1].to_broadcast([P, P]),
                        op=mybir.AluOpType.mult)
                # interleave: next superchunk's entry transposes
                if xgT_next is not None and of < 8:
                    entry_T2(xg_next, xgT_next,
                             [(of // 2, (of % 2) * 2 + k) for k in range(2)])
            for k in range(4):
                r0 = sc * 512 + k * P
                nc.sync.dma_start(h2_d.ap()[r0:r0 + P, :], h2ot[k][:])
            xg_cur, ea_cur, xgT_cur = xg_next, ea_next, xgT_next

            state["sc_done"] = sc + 1
            try_gathers()
            while (state["s_next"] < NSB
                   and bcut[4 * (state["s_next"] + 1) - 1] <= state["sc_done"]):
                emit_superblock(state["s_next"])
                state["s_next"] += 1

        while state["s_next"] < NSB:
            emit_superblock(state["s_next"])
            state["s_next"] += 1

    nc.compile()
    return nc


def _prepare(x, row, col, ea):
    """Host-side sharding: sort edges by destination, split nodes into 8
    block-aligned edge-balanced shards, build per-core arrays."""
    N = x.shape[0]
    E = ea.shape[0]
    order = np.argsort(col, kind="stable")
    scol = col[order]
    srow = row[order]
    NBLK = (N + P - 1) // P
    NTOT = NBLK * P

    bounds = [0]
    for p in range(1, NCORES):
        if E > 0:
            t = int(scol[min((p * E) // NCORES, E - 1)])
        else:
            t = (p * NTOT) // NCORES
        b = int(round(t / P)) * P
        b = max(b, bounds[-1] + P)
        b = min(b, NTOT - P * (NCORES - p))
        bounds.append(b)
    bounds.append(NTOT)
    for p in range(1, NCORES + 1):
        assert bounds[p] > bounds[p - 1], f"degenerate shard bounds {bounds}"

    e_split = np.searchsorted(scol, bounds)
    Ec = np.diff(e_split)
    EC = max(4, math.ceil(int(Ec.max()) / P))
    EC = ((EC + 3) // 4) * 4
    EP = EC * P
    nblk = [(bounds[p + 1] - bounds[p]) // P for p in range(NCORES)]
    NB = max(4, ((max(nblk) + 3) // 4) * 4)
    NBP = NB * P
    blkdeg = np.bincount(scol // P, minlength=NBLK)
    KB = max(1, math.ceil(int(blkdeg.max()) / P))

    xbf = np.zeros((NTOT, FN), NPBF16)
    xbf[:N] = x.astype(NPBF16)
    xpadT = np.zeros((FN, NTOT + NBP), NPBF16)
    xpadT[:, :N] = xbf[:N].T

    cnt_all = np.bincount(scol, minlength=NTOT)

    cores = []
    bstarts = []
    for p in range(NCORES):
        s, e = int(e_split[p]), int(e_split[p + 1])
        n0 = bounds[p]
        ne = e - s
        tmp = np.zeros(EP, np.int32)
        tmp[:ne] = srow[s:e]
        rows_t = np.ascontiguousarray(tmp.reshape(EC, P).T)
        eaT = np.zeros((FE, EP), NPBF16)
        eaT[:, :ne] = ea[order[s:e]].T.astype(NPBF16)
        lcol = (scol[s:e] - n0).astype(np.int64)
        bstart = np.searchsorted(lcol, np.arange(NB + 1) * P)
        bstarts.append(bstart)
        gid = np.zeros((NB, KB, P), np.int32)
        colb = np.full((NB, KB, P), -1.0, np.float32)
        for b in range(NB):
            sb, eb = int(bstart[b]), int(bstart[b + 1])
            cnt = eb - sb
            assert cnt <= KB * P
            gid[b].reshape(-1)[:cnt] = np.arange(sb, eb, dtype=np.int32)
            colb[b].reshape(-1)[:cnt] = (lcol[sb:eb] - b * P)
        gid_t = np.ascontiguousarray(gid.reshape(NB * KB, P).T)
        colb_t = np.ascontiguousarray(
            colb.reshape(NB * KB, P).T.astype(NPBF16))
        # per-edge 1/count of the destination node (0 for pad edges so
        # their staged h2 rows are exactly zero)
        invce = np.zeros(EP, np.float32)
        invce[:ne] = 1.0 / np.maximum(cnt_all[scol[s:e]], 1.0)
        invce_t = np.ascontiguousarray(invce.reshape(EC, P).T.astype(NPBF16))
        xsT = np.ascontiguousarray(xpadT[:, n0:n0 + NBP])
        cores.append(dict(rows=rows_t, eaT=eaT, gid=gid_t, colb=colb_t,
                          invce=invce_t, xsT=xsT))

    # uniform (max-over-cores) superchunk cut per node block: block b's h2
    # rows are complete once bcut[b] edge superchunks have run on every core
    bcut = tuple(
        int(max(math.ceil(bstarts[p][b + 1] / 512) for p in range(NCORES)))
        for b in range(NB))
    return cores, bounds, EC, NB, KB, xbf, bcut


def _run(inputs, trace=False):
    x = np.ascontiguousarray(np.asarray(inputs["x"], dtype=np.float32))
    ei = np.asarray(inputs["edge_index"])
    ea = np.ascontiguousarray(np.asarray(inputs["edge_attr"], dtype=np.float32))
    row = ei[0].astype(np.int64)
    col = ei[1].astype(np.int64)
    W1 = np.asarray(inputs["W1"], np.float32).astype(NPBF16)
    W2 = np.asarray(inputs["W2"], np.float32).astype(NPBF16)
    W3 = np.asarray(inputs["W3"], np.float32).astype(NPBF16)
    W4 = np.asarray(inputs["W4"], np.float32).astype(NPBF16)
    b1 = np.asarray(inputs["b1"], np.float32)
    b2 = np.asarray(inputs["b2"], np.float32)
    b3 = np.asarray(inputs["b3"], np.float32)
    b4 = np.asarray(inputs["b4"], np.float32)
    N = x.shape[0]

    cores, bounds, EC, NB, KB, xbf, bcut = _prepare(x, row, col, ea)

    key = (EC, NB, KB, xbf.shape[0], bcut)
    if key not in _prog_cache:
        _prog_cache[key] = _build(EC, NB, KB, xbf.shape[0], bcut)
    nc = _prog_cache[key]

    b1t = np.ascontiguousarray(b1.reshape(HID // P, P).T)
    b2t = np.ascontiguousarray(b2.reshape(HID // P, P).T)
    b3t = np.ascontiguousarray(b3.reshape((FN + FE) // P, P).T)
    b4t = np.ascontiguousarray(b4.reshape(FN // P, P).T)
    iota = np.ascontiguousarray(
        np.broadcast_to(np.arange(P, dtype=np.float32), (P, P))).astype(NPBF16)
    ident = np.eye(P, dtype=np.float32).astype(NPBF16)

    in_maps = []
    for p in range(NCORES):
        c = cores[p]
        in_maps.append({
            "x": xbf, "rows": c["rows"], "eaT": c["eaT"],
            "W1": W1, "W2": W2, "W3": W3, "W4": W4,
            "b1": b1t, "b2": b2t, "b3": b3t, "b4": b4t,
            "gid": c["gid"], "colb": c["colb"], "invce": c["invce"],
            "xsT": c["xsT"], "iota": iota, "ident": ident,
        })

    res = run_bass_kernel_spmd(nc, in_maps, list(range(NCORES)), trace=trace)

    out = np.empty((N, FN), np.float32)
    for p in range(NCORES):
        n0, n1 = bounds[p], min(bounds[p + 1], N)
        if n1 > n0:
            out[n0:n1] = res.results[p]["outT"].T[:n1 - n0]
    return out, res


def kernel(**inputs) -> np.ndarray:
    out, _ = _run(inputs, trace=False)
    return out


# revision 30
# speedup vs baseline: 1.3614x; 1.2026x over previous
"""Trainium2 Bass kernel for nn_NodeModel (GNN message passing).

Reference computation:
    h   = relu(concat(x[row], edge_attr) @ W1 + b1) @ W2 + b2     # edge MLP
    agg = scatter_mean(h, col, N)                                  # per-dest mean
    out = relu(concat(x, agg) @ W3 + b3) @ W4 + b4                 # node MLP

Distribution strategy (8 cores, no collectives needed):
  - Sort edges by destination node; split destination nodes into 8
    block-aligned, edge-balanced shards.  Each core owns one node shard and
    ALL edges targeting it, so per-node sums are complete locally.
  - x is replicated; each core gathers x[row] for its edges with indirect
    DMA on-device.
  - All matmul operands are bf16 (fp32 PSUM accumulation): halves HBM
    traffic vs fp32r and speeds PE transposes 1.5x.
  - Edge MLP runs with weights stationary and activations kept transposed
    [feat, edge]; h2 rows (pre-scaled by 1/count of their destination) are
    staged to DRAM in bf16.
  - Scatter-mean per 128-node block: indirect-gather the block's h2 rows,
    build a one-hot selection matrix with is_equal against an iota, and
    matmul-accumulate h2^T @ S in PSUM -> aggT directly in [hid, node]
    layout (the mean's 1/count is pre-applied per-edge in the h2 exit copy).
  - The scatter + node-MLP work is INTERLEAVED into the edge phase: since
    edges are sorted by destination, node block b only needs the first
    bcut[b] edge superchunks.  A static schedule (max over cores, so the
    SPMD program is uniform) runs each superblock as soon as its edges are
    done, overlapping the h2 gather-back DMA with edge-MLP compute.
  - Node-MLP output stays transposed [feat, node]; un-transposed on host.
"""

import math
import sys
from contextlib import ExitStack

sys.path.insert(0, "/opt/trn_rl_repo")

import ml_dtypes
import numpy as np

import concourse.bass as bass
import concourse.tile as tile
from concourse import bacc, mybir
from concourse.bass_utils import run_bass_kernel_spmd

NCORES = 8
P = 128
FN = 512    # node feature dim
FE = 128    # edge feature dim
HID = 1280  # edge-MLP hidden/output dim
F32 = mybir.dt.float32
BF16 = mybir.dt.bfloat16
I32 = mybir.dt.int32
RELU = mybir.ActivationFunctionType.Relu
IDENT = mybir.ActivationFunctionType.Identity
NPBF16 = ml_dtypes.bfloat16

_prog_cache = {}


def _build(EC, NB, KB, NX, bcut, has_b2):
    """Build the SPMD program for one core.

    EC: edge chunks (128 edges each) per core, multiple of 4.
    NB: node blocks (128 nodes each) per core, multiple of 4.
    KB: max edge chunks per node block (scatter schedule width).
    NX: number of rows of the replicated x (gather source).
    bcut: per node block, the number of edge superchunks that must be
          complete before its h2 rows exist (max over cores -> uniform).
    has_b2: emit the b2 (x) s_n rank-1 scatter correction (b2 cannot ride
          the h2 activation since the W2 stage keeps edges on partitions;
          it distributes through the mean as agg += b2 * [deg>0]).
    """
    EP = EC * P
    SC = EC // 4   # superchunks of 512 edges
    NSB = NB // 4  # superblocks of 512 nodes
    LOOKAHEAD = max(2, 12 // KB)  # h2-gather prefetch blocks (SBUF-bounded)
    SLACK = 1  # superchunks between a block's h2 completion and its scatter

    nc = bacc.Bacc("TRN2", target_bir_lowering=False, debug=False,
                   num_devices=NCORES)

    x_d = nc.dram_tensor("x", [NX, FN], BF16, kind="ExternalInput")
    rows_d = nc.dram_tensor("rows", [P, EC], I32, kind="ExternalInput")
    eaT_d = nc.dram_tensor("eaT", [FE, EP], BF16, kind="ExternalInput")
    W1_d = nc.dram_tensor("W1", [FN + FE, HID], BF16, kind="ExternalInput")
    W2_d = nc.dram_tensor("W2", [HID, HID], BF16, kind="ExternalInput")
    W3_d = nc.dram_tensor("W3", [FN + HID, FN + FE], BF16, kind="ExternalInput")
    W4_d = nc.dram_tensor("W4", [FN + FE, FN], BF16, kind="ExternalInput")
    b1_d = nc.dram_tensor("b1", [P, HID // P], F32, kind="ExternalInput")
    if has_b2:
        b2r_d = nc.dram_tensor("b2r", [1, HID], BF16, kind="ExternalInput")
        srow_d = nc.dram_tensor("srow", [1, NB * P], BF16,
                                kind="ExternalInput")
    b3_d = nc.dram_tensor("b3", [P, (FN + FE) // P], F32, kind="ExternalInput")
    b4_d = nc.dram_tensor("b4", [P, FN // P], F32, kind="ExternalInput")
    gid_d = nc.dram_tensor("gid", [P, NB * KB], I32, kind="ExternalInput")
    colb_d = nc.dram_tensor("colb", [P, NB * KB], BF16, kind="ExternalInput")
    invce_d = nc.dram_tensor("invce", [P, EC], BF16, kind="ExternalInput")
    xsT_d = nc.dram_tensor("xsT", [FN, NB * P], BF16, kind="ExternalInput")
    iota_d = nc.dram_tensor("iota", [P, P], BF16, kind="ExternalInput")
    ident_d = nc.dram_tensor("ident", [P, P], BF16, kind="ExternalInput")
    outT_d = nc.dram_tensor("outT", [FN, NB * P], F32, kind="ExternalOutput")
    h2_d = nc.dram_tensor("h2buf", [EP, HID], BF16)  # internal staging

    with tile.TileContext(nc) as tc, ExitStack() as ctx:
        cpool = ctx.enter_context(tc.tile_pool(name="const", bufs=1))
        wpool = ctx.enter_context(tc.tile_pool(name="wts", bufs=1))

        # Load order = sync-queue FIFO order: first the tensors the first
        # superchunk needs (ident for transposes, rows for the gathers, b1,
        # W1 split per contraction chunk), then everything else behind them.
        identt = cpool.tile([P, P], BF16)
        nc.sync.dma_start(identt[:], ident_d.ap()[:])
        rowst = cpool.tile([P, EC], I32)
        nc.sync.dma_start(rowst[:], rows_d.ap()[:])
        b1t = cpool.tile([P, HID // P], F32)
        nc.sync.dma_start(b1t[:], b1_d.ap()[:])
        W1r = W1_d.ap().rearrange("(ko ki) m -> ki ko m", ki=P)
        W1t = []
        for k in range(5):
            w = wpool.tile([P, HID], BF16, name=f"W1_{k}", tag=f"W1_{k}")
            nc.sync.dma_start(w[:], W1r[:, k, :])
            W1t.append(w)
        W2t = wpool.tile([P, 10, HID], BF16)
        W2r = W2_d.ap().rearrange("(ko ki) m -> ki ko m", ki=P)
        for k in range(10):
            nc.sync.dma_start(W2t[:, k, :], W2r[:, k, :])
        iotat = cpool.tile([P, P], BF16)
        nc.sync.dma_start(iotat[:], iota_d.ap()[:])
        b3t = cpool.tile([P, (FN + FE) // P], F32)
        nc.sync.dma_start(b3t[:], b3_d.ap()[:])
        b4t = cpool.tile([P, FN // P], F32)
        nc.sync.dma_start(b4t[:], b4_d.ap()[:])
        gidt = cpool.tile([P, NB * KB], I32)
        nc.sync.dma_start(gidt[:], gid_d.ap()[:])
        colbt = cpool.tile([P, NB * KB], BF16)
        nc.sync.dma_start(colbt[:], colb_d.ap()[:])
        invcet = cpool.tile([P, EC], BF16)
        nc.sync.dma_start(invcet[:], invce_d.ap()[:])
        if has_b2:
            b2rt = cpool.tile([1, HID], BF16)
            nc.sync.dma_start(b2rt[:], b2r_d.ap()[:])
            srt = cpool.tile([1, NB * P], BF16)
            nc.sync.dma_start(srt[:], srow_d.ap()[:])
        W3t = wpool.tile([P, 14, FN + FE], BF16)
        nc.sync.dma_start(
            W3t[:], W3_d.ap().rearrange("(ko ki) m -> ki ko m", ki=P))
        W4t = wpool.tile([P, 5, FN], BF16)
        nc.sync.dma_start(
            W4t[:], W4_d.ap().rearrange("(ko ki) m -> ki ko m", ki=P))

        # ---- pools (all coexist: phases are interleaved) ----
        # PSUM is 8 banks of 2 KB, allocated per tile name at bank
        # granularity: mm 2x[P,512]f32 = 2 banks; smp holds the scatter
        # half-accumulator [P,5,P]f32 (2 banks) and the 4-slot transpose
        # staging tile [P,512]bf16 (1 bank) -> 2 + 2*(2+1) = 8.
        mmp = ctx.enter_context(tc.tile_pool(name="mm", bufs=2, space="PSUM"))
        smp = ctx.enter_context(tc.tile_pool(name="smp", bufs=2, space="PSUM"))

        def pt4():
            return smp.tile([P, 512], BF16, name="pt4", tag="pt4")
        xgp = ctx.enter_context(tc.tile_pool(name="xg", bufs=2))
        xgTp = ctx.enter_context(tc.tile_pool(name="xgT", bufs=2))
        eap = ctx.enter_context(tc.tile_pool(name="ea", bufs=2))
        h1p = ctx.enter_context(tc.tile_pool(name="h1T", bufs=1))
        h2op = ctx.enter_context(tc.tile_pool(name="h2o", bufs=4))
        h2gp = ctx.enter_context(
            tc.tile_pool(name="h2g", bufs=(LOOKAHEAD + 1) * KB))
        Sp = ctx.enter_context(
            tc.tile_pool(name="Smat", bufs=(LOOKAHEAD + 1) * KB))
        aggTp = ctx.enter_context(tc.tile_pool(name="aggT", bufs=2))
        xsp = ctx.enter_context(tc.tile_pool(name="xs", bufs=2))
        h3p = ctx.enter_context(tc.tile_pool(name="h3T", bufs=1))
        oTp = ctx.enter_context(tc.tile_pool(name="oT", bufs=1))

        # ---------------- edge-phase helpers ----------------
        def issue_gather(sc):
            xgt = xgp.tile([P, 4, FN], BF16)
            for k in range(4):
                nc.gpsimd.indirect_dma_start(
                    out=xgt[:, k, :], out_offset=None, in_=x_d.ap()[:],
                    in_offset=bass.IndirectOffsetOnAxis(
                        ap=rowst[:, sc * 4 + k:sc * 4 + k + 1], axis=0))
            eat = eap.tile([P, 512], BF16)
            nc.sync.dma_start(
                eat[:], eaT_d.ap()[:, sc * 512:(sc + 1) * 512])
            return xgt, eat

        def entry_T2(xgt, xgTt, pairs):
            """Transpose up to 4 (f, k) entries through one pt4 staging tile."""
            pt = pt4()
            for i, (f, k) in enumerate(pairs):
                nc.tensor.transpose(
                    pt[:, i * P:(i + 1) * P],
                    xgt[:, k, f * P:(f + 1) * P], identt[:])
                nc.vector.tensor_copy(
                    xgTt[:, f, k * P:(k + 1) * P], pt[:, i * P:(i + 1) * P])

        # ---------------- scatter/node-phase helpers ----------------
        pend_gs = {}
        state = dict(g_next=0, b_next=0, s_next=0, sc_done=0,
                     xg_cur=None, ea_cur=None, xgT_cur=None, xst_cur=None)

        def gather_S(b):
            ext = max(bcut[b], 1) * 512  # h2 rows that exist by then
            lst = []
            for k in range(KB):
                c = b * KB + k
                # pad slots carry id 0 (not OOB-skip): every partition of the
                # tile gets written with finite data, so the zero one-hot
                # columns can never multiply stale NaN bit patterns.
                h2g = h2gp.tile([P, HID], BF16, name=f"h2g_{b}_{k}",
                                tag="h2g")
                nc.gpsimd.indirect_dma_start(
                    out=h2g[:], out_offset=None, in_=h2_d.ap()[:ext],
                    in_offset=bass.IndirectOffsetOnAxis(
                        ap=gidt[:, c:c + 1], axis=0))
                St = Sp.tile([P, P], BF16, name=f"S_{b}_{k}", tag="S")
                nc.vector.tensor_tensor(
                    St[:], colbt[:, c:c + 1].to_broadcast([P, P]),
                    iotat[:], op=mybir.AluOpType.is_equal)
                lst.append((h2g, St))
            pend_gs[b] = lst

        def try_gathers():
            while (state["g_next"] < NB
                   and bcut[state["g_next"]] <= state["sc_done"]
                   and state["g_next"] < state["b_next"] + LOOKAHEAD):
                gather_S(state["g_next"])
                state["g_next"] += 1

        def load_xst(s):
            xst = xsp.tile([P, 4, 512], BF16, name=f"xst_{s}", tag="xst")
            nc.sync.dma_start(
                xst[:],
                xsT_d.ap().rearrange("(fo fi) n -> fi fo n", fi=P)
                [:, :, s * 512:(s + 1) * 512])
            return xst

        outTr = outT_d.ap().rearrange("(fo fi) n -> fi fo n", fi=P)

        def emit_superblock(s):
            # scatter: accumulate aggT[hid, node] directly in PSUM with the
            # gathered h2 rows stationary and the one-hot S moving.
            aggTsb = aggTp.tile([P, 10, 512], BF16)
            for bb in range(4):
                b = s * 4 + bb
                gs = pend_gs.pop(b)
                # j-major: each 128-wide accumulation group's matmuls stay
                # consecutive (open groups must not interleave in a bank).
                for half in range(2):
                    pss = smp.tile([P, 5, P], F32, name="pss", tag="pss")
                    for j5 in range(5):
                        j = half * 5 + j5
                        for k, (h2g, St) in enumerate(gs):
                            nc.tensor.matmul(
                                pss[:, j5, :], h2g[:, j * P:(j + 1) * P],
                                St[:], start=(k == 0),
                                stop=(k == KB - 1 and not has_b2))
                        if has_b2:
                            # agg includes +b2 for nodes with deg>0: rank-1
                            # b2[j-slice] (x) s_row closes the group.
                            nc.tensor.matmul(
                                pss[:, j5, :], b2rt[:, j * P:(j + 1) * P],
                                srt[:, b * P:(b + 1) * P],
                                start=False, stop=True)
                    nc.vector.tensor_copy(
                        aggTsb[:, half * 5:(half + 1) * 5,
                               bb * P:(bb + 1) * P], pss[:])
                state["b_next"] = b + 1
                try_gathers()

            xst = state["xst_cur"]
            state["xst_cur"] = load_xst(s + 1) if s + 1 < NSB else None
            h3Tt = h3p.tile([P, 5, 512], BF16)
            for of in range(5):
                ps = mmp.tile([P, 512], F32)
                for k in range(4):
                    nc.tensor.matmul(
                        ps[:], W3t[:, k, of * P:(of + 1) * P],
                        xst[:, k, :], start=(k == 0), stop=False)
                for f in range(10):
                    nc.tensor.matmul(
                        ps[:], W3t[:, 4 + f, of * P:(of + 1) * P],
                        aggTsb[:, f, :], start=False, stop=(f == 9))
                nc.scalar.activation(h3Tt[:, of, :], ps[:], RELU,
                                     bias=b3t[:, of:of + 1])
            oTt = oTp.tile([P, 4, 512], F32)
            for of in range(4):
                ps = mmp.tile([P, 512], F32)
                for k in range(5):
                    nc.tensor.matmul(
                        ps[:], W4t[:, k, of * P:(of + 1) * P],
                        h3Tt[:, k, :], start=(k == 0), stop=(k == 4))
                nc.scalar.activation(
                    oTt[:, of, :], ps[:], IDENT, bias=b4t[:, of:of + 1])
            nc.sync.dma_start(outTr[:, :, s * 512:(s + 1) * 512], oTt[:])

        # ---------------- interleaved main loop ----------------
        # prologue: superchunk 0 input + its entry transposes + first xst
        xg_cur, ea_cur = issue_gather(0)
        xgT_cur = xgTp.tile([P, 4, 512], BF16)
        for f in range(4):
            entry_T2(xg_cur, xgT_cur, [(f, k) for k in range(4)])
        state["xst_cur"] = load_xst(0)

        for sc in range(SC):
            if sc + 1 < SC:
                xg_next, ea_next = issue_gather(sc + 1)
                xgT_next = xgTp.tile([P, 4, 512], BF16)
            else:
                xg_next = ea_next = xgT_next = None

            h1Tt = h1p.tile([P, 10, 512], BF16)
            for of in range(10):
                ps = mmp.tile([P, 512], F32)
                for k in range(5):
                    rhs = xgT_cur[:, k, :] if k < 4 else ea_cur[:]
                    nc.tensor.matmul(
                        ps[:], W1t[k][:, of * P:(of + 1) * P], rhs,
                        start=(k == 0), stop=(k == 4))
                nc.scalar.activation(h1Tt[:, of, :], ps[:], RELU,
                                     bias=b1t[:, of:of + 1])
            # W2 stage with h1T stationary and W2 moving: the product lands
            # directly in [edge, hid] layout -- no exit transposes.  The
            # per-edge 1/count of the destination is folded into the
            # PSUM->SBUF copy so the scatter can accumulate raw sums (b2,
            # which would vary along the free dim here, distributes through
            # the scatter-mean and is re-added there when nonzero).
            h2ot = [h2op.tile([P, HID], BF16, name=f"h2o_{sc}_{k}", tag="h2o")
                     for k in range(4)]
            gi = 0
            for kk in range(4):
                c = sc * 4 + kk
                for lo, w in ((0, 512), (512, 512), (1024, 256)):
                    ps = mmp.tile([P, 512], F32)
                    for k in range(10):
                        nc.tensor.matmul(
                            ps[:, :w], h1Tt[:, k, kk * P:(kk + 1) * P],
                            W2t[:, k, lo:lo + w],
                            start=(k == 0), stop=(k == 9))
                    nc.vector.tensor_tensor(
                        h2ot[kk][:, lo:lo + w], ps[:, :w],
                        invcet[:, c:c + 1].to_broadcast([P, w]),
                        op=mybir.AluOpType.mult)
                    # interleave: next superchunk's entry transposes
                    if xgT_next is not None and gi < 8:
                        entry_T2(xg_next, xgT_next,
                                 [(gi // 2, (gi % 2) * 2 + k) for k in range(2)])
                    gi += 1
            for k in range(4):
                r0 = sc * 512 + k * P
                nc.sync.dma_start(h2_d.ap()[r0:r0 + P, :], h2ot[k][:])
            xg_cur, ea_cur, xgT_cur = xg_next, ea_next, xgT_next

            state["sc_done"] = sc + 1
            try_gathers()
            while (state["s_next"] < NSB
                   and bcut[4 * (state["s_next"] + 1) - 1] + SLACK
                       <= state["sc_done"]):
                emit_superblock(state["s_next"])
                state["s_next"] += 1

        while state["s_next"] < NSB:
            emit_superblock(state["s_next"])
            state["s_next"] += 1

    nc.compile()
    return nc


def _prepare(x, row, col, ea):
    """Host-side sharding: sort edges by destination, split nodes into 8
    block-aligned edge-balanced shards, build per-core arrays."""
    N = x.shape[0]
    E = ea.shape[0]
    order = np.argsort(col, kind="stable")
    scol = col[order]
    srow = row[order]
    NBLK = (N + P - 1) // P
    NTOT = NBLK * P

    bounds = [0]
    for p in range(1, NCORES):
        if E > 0:
            t = int(scol[min((p * E) // NCORES, E - 1)])
        else:
            t = (p * NTOT) // NCORES
        b = int(round(t / P)) * P
        b = max(b, bounds[-1] + P)
        b = min(b, NTOT - P * (NCORES - p))
        bounds.append(b)
    bounds.append(NTOT)
    for p in range(1, NCORES + 1):
        assert bounds[p] > bounds[p - 1], f"degenerate shard bounds {bounds}"

    e_split = np.searchsorted(scol, bounds)
    Ec = np.diff(e_split)
    EC = max(4, math.ceil(int(Ec.max()) / P))
    EC = ((EC + 3) // 4) * 4
    EP = EC * P
    nblk = [(bounds[p + 1] - bounds[p]) // P for p in range(NCORES)]
    NB = max(4, ((max(nblk) + 3) // 4) * 4)
    NBP = NB * P
    blkdeg = np.bincount(scol // P, minlength=NBLK)
    KB = max(1, math.ceil(int(blkdeg.max()) / P))

    xbf = np.zeros((NTOT, FN), NPBF16)
    xbf[:N] = x.astype(NPBF16)
    xpadT = np.zeros((FN, NTOT + NBP), NPBF16)
    xpadT[:, :N] = xbf[:N].T

    cnt_all = np.bincount(scol, minlength=NTOT)

    cores = []
    bstarts = []
    for p in range(NCORES):
        s, e = int(e_split[p]), int(e_split[p + 1])
        n0 = bounds[p]
        ne = e - s
        tmp = np.zeros(EP, np.int32)
        tmp[:ne] = srow[s:e]
        rows_t = np.ascontiguousarray(tmp.reshape(EC, P).T)
        eaT = np.zeros((FE, EP), NPBF16)
        eaT[:, :ne] = ea[order[s:e]].T.astype(NPBF16)
        lcol = (scol[s:e] - n0).astype(np.int64)
        bstart = np.searchsorted(lcol, np.arange(NB + 1) * P)
        bstarts.append(bstart)
        gid = np.zeros((NB, KB, P), np.int32)
        colb = np.full((NB, KB, P), -1.0, np.float32)
        for b in range(NB):
            sb, eb = int(bstart[b]), int(bstart[b + 1])
            cnt = eb - sb
            assert cnt <= KB * P
            gid[b].reshape(-1)[:cnt] = np.arange(sb, eb, dtype=np.int32)
            colb[b].reshape(-1)[:cnt] = (lcol[sb:eb] - b * P)
        gid_t = np.ascontiguousarray(gid.reshape(NB * KB, P).T)
        colb_t = np.ascontiguousarray(
            colb.reshape(NB * KB, P).T.astype(NPBF16))
        # per-edge 1/count of the destination node (0 for pad edges so
        # their staged h2 rows are exactly zero)
        invce = np.zeros(EP, np.float32)
        invce[:ne] = 1.0 / np.maximum(cnt_all[scol[s:e]], 1.0)
        invce_t = np.ascontiguousarray(invce.reshape(EC, P).T.astype(NPBF16))
        xsT = np.ascontiguousarray(xpadT[:, n0:n0 + NBP])
        # per-node degree>0 indicator for the b2 scatter correction
        deg = np.zeros(NBP, np.float32)
        span = min(NBP, NTOT - n0)
        deg[:span] = cnt_all[n0:n0 + span]
        srow_t = (deg > 0).astype(NPBF16).reshape(1, NBP)
        cores.append(dict(rows=rows_t, eaT=eaT, gid=gid_t, colb=colb_t,
                          invce=invce_t, xsT=xsT, srow=srow_t))

    # uniform (max-over-cores) superchunk cut per node block: block b's h2
    # rows are complete once bcut[b] edge superchunks have run on every core
    bcut = tuple(
        int(max(math.ceil(bstarts[p][b + 1] / 512) for p in range(NCORES)))
        for b in range(NB))
    return cores, bounds, EC, NB, KB, xbf, bcut


def _run(inputs, trace=False):
    x = np.ascontiguousarray(np.asarray(inputs["x"], dtype=np.float32))
    ei = np.asarray(inputs["edge_index"])
    ea = np.ascontiguousarray(np.asarray(inputs["edge_attr"], dtype=np.float32))
    row = ei[0].astype(np.int64)
    col = ei[1].astype(np.int64)
    W1 = np.asarray(inputs["W1"], np.float32).astype(NPBF16)
    W2 = np.asarray(inputs["W2"], np.float32).astype(NPBF16)
    W3 = np.asarray(inputs["W3"], np.float32).astype(NPBF16)
    W4 = np.asarray(inputs["W4"], np.float32).astype(NPBF16)
    b1 = np.asarray(inputs["b1"], np.float32)
    b2 = np.asarray(inputs["b2"], np.float32)
    b3 = np.asarray(inputs["b3"], np.float32)
    b4 = np.asarray(inputs["b4"], np.float32)
    N = x.shape[0]

    cores, bounds, EC, NB, KB, xbf, bcut = _prepare(x, row, col, ea)
    has_b2 = bool(np.any(b2 != 0))

    key = (EC, NB, KB, xbf.shape[0], bcut, has_b2)
    if key not in _prog_cache:
        _prog_cache[key] = _build(EC, NB, KB, xbf.shape[0], bcut, has_b2)
    nc = _prog_cache[key]

    b1t = np.ascontiguousarray(b1.reshape(HID // P, P).T)
    b3t = np.ascontiguousarray(b3.reshape((FN + FE) // P, P).T)
    b4t = np.ascontiguousarray(b4.reshape(FN // P, P).T)
    iota = np.ascontiguousarray(
        np.broadcast_to(np.arange(P, dtype=np.float32), (P, P))).astype(NPBF16)
    ident = np.eye(P, dtype=np.float32).astype(NPBF16)

    in_maps = []
    for p in range(NCORES):
        c = cores[p]
        m = {
            "x": xbf, "rows": c["rows"], "eaT": c["eaT"],
            "W1": W1, "W2": W2, "W3": W3, "W4": W4,
            "b1": b1t, "b3": b3t, "b4": b4t,
            "gid": c["gid"], "colb": c["colb"], "invce": c["invce"],
            "xsT": c["xsT"], "iota": iota, "ident": ident,
        }
        if has_b2:
            m["b2r"] = np.ascontiguousarray(b2.reshape(1, HID).astype(NPBF16))
            m["srow"] = c["srow"]
        in_maps.append(m)

    res = run_bass_kernel_spmd(nc, in_maps, list(range(NCORES)), trace=trace)

    out = np.empty((N, FN), np.float32)
    for p in range(NCORES):
        n0, n1 = bounds[p], min(bounds[p + 1], N)
        if n1 > n0:
            out[n0:n1] = res.results[p]["outT"].T[:n1 - n0]
    return out, res


def kernel(**inputs) -> np.ndarray:
    out, _ = _run(inputs, trace=False)
    return out


# revision 45
# speedup vs baseline: 1.4114x; 1.0367x over previous
"""Trainium2 Bass kernel for nn_NodeModel (GNN message passing).

Reference computation:
    h   = relu(concat(x[row], edge_attr) @ W1 + b1) @ W2 + b2     # edge MLP
    agg = scatter_mean(h, col, N)                                  # per-dest mean
    out = relu(concat(x, agg) @ W3 + b3) @ W4 + b4                 # node MLP

Distribution strategy (8 cores, no collectives needed):
  - Sort edges by destination node; split destination nodes into 8
    block-aligned, edge-balanced shards.  Each core owns one node shard and
    ALL edges targeting it, so per-node sums are complete locally.
  - x is replicated; each core gathers x[row] for its edges with indirect
    DMA on-device.
  - All matmul operands are bf16 (fp32 PSUM accumulation): halves HBM
    traffic vs fp32r and speeds PE transposes 1.5x.
  - Edge MLP runs with weights stationary and activations kept transposed
    [feat, edge]; h2 rows (pre-scaled by 1/count of their destination) are
    staged to DRAM in bf16.
  - Scatter-mean per 128-node block: indirect-gather the block's h2 rows,
    build a one-hot selection matrix with is_equal against an iota, and
    matmul-accumulate h2^T @ S in PSUM -> aggT directly in [hid, node]
    layout (the mean's 1/count is pre-applied per-edge in the h2 exit copy).
  - The scatter + node-MLP work is INTERLEAVED into the edge phase: since
    edges are sorted by destination, node block b only needs the first
    bcut[b] edge superchunks.  A static schedule (max over cores, so the
    SPMD program is uniform) runs each superblock as soon as its edges are
    done, overlapping the h2 gather-back DMA with edge-MLP compute.
  - Node-MLP output stays transposed [feat, node]; un-transposed on host.
"""

import math
import sys
from contextlib import ExitStack

sys.path.insert(0, "/opt/trn_rl_repo")

import ml_dtypes
import numpy as np

import concourse.bass as bass
import concourse.tile as tile
from concourse import bacc, mybir
from concourse.bass_utils import run_bass_kernel_spmd

NCORES = 8
P = 128
FN = 512    # node feature dim
FE = 128    # edge feature dim
HID = 1280  # edge-MLP hidden/output dim
F32 = mybir.dt.float32
BF16 = mybir.dt.bfloat16
I32 = mybir.dt.int32
RELU = mybir.ActivationFunctionType.Relu
IDENT = mybir.ActivationFunctionType.Identity
NPBF16 = ml_dtypes.bfloat16

_prog_cache = {}


def _build(EC, NB, KB, USZ, bcut, has_b2):
    """Build the SPMD program for one core.

    EC: edge chunks (128 edges each) per core, multiple of 4.
    NB: node blocks (128 nodes each) per core, multiple of 4.
    KB: max edge chunks per node block (scatter schedule width).
    USZ: rows of the compacted per-core x source table (unique sources,
         int16-indexable so dma_gather's transpose path can be used).
    bcut: per node block, the number of edge superchunks that must be
          complete before its h2 rows exist (max over cores -> uniform).
    has_b2: emit the b2 (x) s_n rank-1 scatter correction (b2 cannot ride
          the h2 activation since the W2 stage keeps edges on partitions;
          it distributes through the mean as agg += b2 * [deg>0]).
    """
    EP = EC * P
    SC = EC // 4   # superchunks of 512 edges
    NSB = NB // 4  # superblocks of 512 nodes
    LOOKAHEAD = max(2, 12 // KB)  # h2-gather prefetch blocks (SBUF-bounded)
    SLACK = 1  # superchunks between a block's h2 completion and its scatter

    nc = bacc.Bacc("TRN2", target_bir_lowering=False, debug=False,
                   num_devices=NCORES)

    xsrc_d = nc.dram_tensor("xsrc", [USZ, FN], BF16, kind="ExternalInput")
    gidx_d = nc.dram_tensor("gidx", [P, SC * 32], mybir.dt.int16,
                            kind="ExternalInput")
    xgT0_d = nc.dram_tensor("xgT0", [P, 4, 512], BF16, kind="ExternalInput")
    eaT_d = nc.dram_tensor("eaT", [FE, EP], BF16, kind="ExternalInput")
    W1_d = nc.dram_tensor("W1", [FN + FE, HID], BF16, kind="ExternalInput")
    W2_d = nc.dram_tensor("W2", [HID, HID], BF16, kind="ExternalInput")
    W3_d = nc.dram_tensor("W3", [FN + HID, FN + FE], BF16, kind="ExternalInput")
    W4_d = nc.dram_tensor("W4", [FN + FE, FN], BF16, kind="ExternalInput")
    b1_d = nc.dram_tensor("b1", [P, HID // P], F32, kind="ExternalInput")
    if has_b2:
        b2r_d = nc.dram_tensor("b2r", [1, HID], BF16, kind="ExternalInput")
        srow_d = nc.dram_tensor("srow", [1, NB * P], BF16,
                                kind="ExternalInput")
    b3_d = nc.dram_tensor("b3", [P, (FN + FE) // P], F32, kind="ExternalInput")
    b4_d = nc.dram_tensor("b4", [P, FN // P], F32, kind="ExternalInput")
    gid_d = nc.dram_tensor("gid", [P, NB * KB], I32, kind="ExternalInput")
    colb_d = nc.dram_tensor("colb", [P, NB * KB], BF16, kind="ExternalInput")
    invce_d = nc.dram_tensor("invce", [P, EC], BF16, kind="ExternalInput")
    xsT_d = nc.dram_tensor("xsT", [FN, NB * P], BF16, kind="ExternalInput")
    iota_d = nc.dram_tensor("iota", [P, P], BF16, kind="ExternalInput")
    outT_d = nc.dram_tensor("outT", [FN, NB * P], F32, kind="ExternalOutput")
    h2_d = nc.dram_tensor("h2buf", [EP, HID], BF16)  # internal staging

    with tile.TileContext(nc) as tc, ExitStack() as ctx:
        cpool = ctx.enter_context(tc.tile_pool(name="const", bufs=1))
        wpool = ctx.enter_context(tc.tile_pool(name="wts", bufs=1))

        # Load order = sync-queue FIFO order: first the tensors the first
        # superchunk needs (its pre-transposed x rows are staged on host so
        # nothing waits on the SWDGE warm-up; b1; W1 split per contraction
        # chunk), then everything else behind them.
        b1t = cpool.tile([P, HID // P], F32)
        nc.sync.dma_start(b1t[:], b1_d.ap()[:])
        gidxt = cpool.tile([P, SC * 32], mybir.dt.int16)
        nc.sync.dma_start(gidxt[:], gidx_d.ap()[:])

        def load_weights():
            W1r = W1_d.ap().rearrange("(ko ki) m -> ki ko m", ki=P)
            W1t = []
            for k in range(5):
                w = wpool.tile([P, HID], BF16, name=f"W1_{k}", tag=f"W1_{k}")
                nc.sync.dma_start(w[:], W1r[:, k, :])
                W1t.append(w)
            W2t = wpool.tile([P, 10, HID], BF16)
            W2r = W2_d.ap().rearrange("(ko ki) m -> ki ko m", ki=P)
            for k in range(10):
                nc.sync.dma_start(W2t[:, k, :], W2r[:, k, :])
            iotat = cpool.tile([P, P], BF16)
            nc.sync.dma_start(iotat[:], iota_d.ap()[:])
            b3t = cpool.tile([P, (FN + FE) // P], F32)
            nc.sync.dma_start(b3t[:], b3_d.ap()[:])
            b4t = cpool.tile([P, FN // P], F32)
            nc.sync.dma_start(b4t[:], b4_d.ap()[:])
            gidt = cpool.tile([P, NB * KB], I32)
            nc.sync.dma_start(gidt[:], gid_d.ap()[:])
            colbt = cpool.tile([P, NB * KB], BF16)
            nc.sync.dma_start(colbt[:], colb_d.ap()[:])
            invcet = cpool.tile([P, EC], BF16)
            nc.sync.dma_start(invcet[:], invce_d.ap()[:])
            bsr = None
            if has_b2:
                b2rt = cpool.tile([1, HID], BF16)
                nc.sync.dma_start(b2rt[:], b2r_d.ap()[:])
                srt = cpool.tile([1, NB * P], BF16)
                nc.sync.dma_start(srt[:], srow_d.ap()[:])
                bsr = (b2rt, srt)
            W3t = wpool.tile([P, 14, FN + FE], BF16)
            nc.sync.dma_start(
                W3t[:], W3_d.ap().rearrange("(ko ki) m -> ki ko m", ki=P))
            W4t = wpool.tile([P, 5, FN], BF16)
            nc.sync.dma_start(
                W4t[:], W4_d.ap().rearrange("(ko ki) m -> ki ko m", ki=P))
            return W1t, W2t, W3t, W4t, iotat, b3t, b4t, gidt, colbt, invcet, bsr

        # ---- pools (all coexist: phases are interleaved) ----
        # PSUM is 8 banks of 2 KB, allocated per tile name at bank
        # granularity: mm 2x[P,512]f32 = 2 banks; smp holds the scatter
        # half-accumulator [P,5,P]f32 (2 banks per buf) -> 2 + 4 = 6.
        mmp = ctx.enter_context(tc.tile_pool(name="mm", bufs=2, space="PSUM"))
        smp = ctx.enter_context(tc.tile_pool(name="smp", bufs=2, space="PSUM"))

        xgTp = ctx.enter_context(tc.tile_pool(name="xgT", bufs=2))
        eap = ctx.enter_context(tc.tile_pool(name="ea", bufs=2))
        h1p = ctx.enter_context(tc.tile_pool(name="h1T", bufs=1))
        h2op = ctx.enter_context(tc.tile_pool(name="h2o", bufs=4))
        h2gp = ctx.enter_context(
            tc.tile_pool(name="h2g", bufs=(LOOKAHEAD + 1) * KB))
        Sp = ctx.enter_context(
            tc.tile_pool(name="Smat", bufs=(LOOKAHEAD + 1) * KB))
        aggTp = ctx.enter_context(tc.tile_pool(name="aggT", bufs=2))
        xsp = ctx.enter_context(tc.tile_pool(name="xs", bufs=2))
        h3p = ctx.enter_context(tc.tile_pool(name="h3T", bufs=1))
        oTp = ctx.enter_context(tc.tile_pool(name="oT", bufs=1))

        # ---------------- edge-phase helpers ----------------
        def issue_gather(sc):
            # dma_gather(transpose=True) delivers x rows already transposed
            # into [feat-chunk-partition, feat-chunk, edge] layout -- no PE
            # entry transposes.  Superchunk 0 is host-staged (plain DMA).
            xgTt = xgTp.tile([P, 4, 512], BF16)
            if sc == 0:
                nc.sync.dma_start(xgTt[:], xgT0_d.ap()[:])
            else:
                nc.gpsimd.dma_gather(
                    xgTt[:], xsrc_d.ap()[:],
                    gidxt[:, sc * 32:(sc + 1) * 32],
                    512, 512, FN, transpose=True)
            eat = eap.tile([P, 512], BF16)
            nc.sync.dma_start(
                eat[:], eaT_d.ap()[:, sc * 512:(sc + 1) * 512])
            return xgTt, eat

        # ---------------- scatter/node-phase helpers ----------------
        pend_gs = {}
        state = dict(g_next=0, b_next=0, s_next=0, sc_done=0,
                     xg_cur=None, ea_cur=None, xgT_cur=None, xst_cur=None)

        def gather_S(b):
            ext = max(bcut[b], 1) * 512  # h2 rows that exist by then
            lst = []
            for k in range(KB):
                c = b * KB + k
                # pad slots carry id 0 (not OOB-skip): every partition of the
                # tile gets written with finite data, so the zero one-hot
                # columns can never multiply stale NaN bit patterns.
                h2g = h2gp.tile([P, HID], BF16, name=f"h2g_{b}_{k}",
                                tag="h2g")
                nc.gpsimd.indirect_dma_start(
                    out=h2g[:], out_offset=None, in_=h2_d.ap()[:ext],
                    in_offset=bass.IndirectOffsetOnAxis(
                        ap=gidt[:, c:c + 1], axis=0))
                St = Sp.tile([P, P], BF16, name=f"S_{b}_{k}", tag="S")
                nc.vector.tensor_tensor(
                    St[:], colbt[:, c:c + 1].to_broadcast([P, P]),
                    iotat[:], op=mybir.AluOpType.is_equal)
                lst.append((h2g, St))
            pend_gs[b] = lst

        def try_gathers():
            while (state["g_next"] < NB
                   and bcut[state["g_next"]] <= state["sc_done"]
                   and state["g_next"] < state["b_next"] + LOOKAHEAD):
                gather_S(state["g_next"])
                state["g_next"] += 1

        def load_xst(s):
            xst = xsp.tile([P, 4, 512], BF16, name=f"xst_{s}", tag="xst")
            nc.sync.dma_start(
                xst[:],
                xsT_d.ap().rearrange("(fo fi) n -> fi fo n", fi=P)
                [:, :, s * 512:(s + 1) * 512])
            return xst

        outTr = outT_d.ap().rearrange("(fo fi) n -> fi fo n", fi=P)

        def emit_superblock(s):
            # scatter: accumulate aggT[hid, node] directly in PSUM with the
            # gathered h2 rows stationary and the one-hot S moving.
            aggTsb = aggTp.tile([P, 10, 512], BF16)
            for bb in range(4):
                b = s * 4 + bb
                gs = pend_gs.pop(b)
                # j-major: each 128-wide accumulation group's matmuls stay
                # consecutive (open groups must not interleave in a bank).
                for half in range(2):
                    pss = smp.tile([P, 5, P], F32, name="pss", tag="pss")
                    for j5 in range(5):
                        j = half * 5 + j5
                        for k, (h2g, St) in enumerate(gs):
                            nc.tensor.matmul(
                                pss[:, j5, :], h2g[:, j * P:(j + 1) * P],
                                St[:], start=(k == 0),
                                stop=(k == KB - 1 and not has_b2))
                        if has_b2:
                            # agg includes +b2 for nodes with deg>0: rank-1
                            # b2[j-slice] (x) s_row closes the group.
                            nc.tensor.matmul(
                                pss[:, j5, :], b2rt[:, j * P:(j + 1) * P],
                                srt[:, b * P:(b + 1) * P],
                                start=False, stop=True)
                    nc.vector.tensor_copy(
                        aggTsb[:, half * 5:(half + 1) * 5,
                               bb * P:(bb + 1) * P], pss[:])
                state["b_next"] = b + 1
                try_gathers()

            xst = state["xst_cur"]
            state["xst_cur"] = load_xst(s + 1) if s + 1 < NSB else None
            h3Tt = h3p.tile([P, 5, 512], BF16)
            for of in range(5):
                ps = mmp.tile([P, 512], F32)
                for k in range(4):
                    nc.tensor.matmul(
                        ps[:], W3t[:, k, of * P:(of + 1) * P],
                        xst[:, k, :], start=(k == 0), stop=False)
                for f in range(10):
                    nc.tensor.matmul(
                        ps[:], W3t[:, 4 + f, of * P:(of + 1) * P],
                        aggTsb[:, f, :], start=False, stop=(f == 9))
                nc.scalar.activation(h3Tt[:, of, :], ps[:], RELU,
                                     bias=b3t[:, of:of + 1])
            oTt = oTp.tile([P, 4, 512], F32)
            for of in range(4):
                ps = mmp.tile([P, 512], F32)
                for k in range(5):
                    nc.tensor.matmul(
                        ps[:], W4t[:, k, of * P:(of + 1) * P],
                        h3Tt[:, k, :], start=(k == 0), stop=(k == 4))
                nc.scalar.activation(
                    oTt[:, of, :], ps[:], IDENT, bias=b4t[:, of:of + 1])
            nc.sync.dma_start(outTr[:, :, s * 512:(s + 1) * 512], oTt[:])

        # ---------------- interleaved main loop ----------------
        # superchunk 0's inputs enter the DMA queue first; all weights and
        # scatter tables queue up behind them.
        xgT_cur, ea_cur = issue_gather(0)
        (W1t, W2t, W3t, W4t, iotat, b3t, b4t, gidt, colbt, invcet,
         bsr) = load_weights()
        if has_b2:
            b2rt, srt = bsr
        state["xst_cur"] = load_xst(0)

        for sc in range(SC):
            if sc + 1 < SC:
                xgT_next, ea_next = issue_gather(sc + 1)
            else:
                xgT_next = ea_next = None

            h1Tt = h1p.tile([P, 10, 512], BF16)
            for of in range(10):
                ps = mmp.tile([P, 512], F32)
                for k in range(5):
                    rhs = xgT_cur[:, k, :] if k < 4 else ea_cur[:]
                    nc.tensor.matmul(
                        ps[:], W1t[k][:, of * P:(of + 1) * P], rhs,
                        start=(k == 0), stop=(k == 4))
                nc.scalar.activation(h1Tt[:, of, :], ps[:], RELU,
                                     bias=b1t[:, of:of + 1])
            # W2 stage with h1T stationary and W2 moving: the product lands
            # directly in [edge, hid] layout -- no exit transposes.  The
            # per-edge 1/count of the destination is folded into the
            # PSUM->SBUF copy so the scatter can accumulate raw sums (b2,
            # which would vary along the free dim here, distributes through
            # the scatter-mean and is re-added there when nonzero).
            h2ot = [h2op.tile([P, HID], BF16, name=f"h2o_{sc}_{k}", tag="h2o")
                     for k in range(4)]
            for kk in range(4):
                c = sc * 4 + kk
                for lo, w in ((0, 512), (512, 512), (1024, 256)):
                    ps = mmp.tile([P, 512], F32)
                    for k in range(10):
                        nc.tensor.matmul(
                            ps[:, :w], h1Tt[:, k, kk * P:(kk + 1) * P],
                            W2t[:, k, lo:lo + w],
                            start=(k == 0), stop=(k == 9))
                    nc.vector.tensor_tensor(
                        h2ot[kk][:, lo:lo + w], ps[:, :w],
                        invcet[:, c:c + 1].to_broadcast([P, w]),
                        op=mybir.AluOpType.mult)
            for k in range(4):
                r0 = sc * 512 + k * P
                nc.sync.dma_start(h2_d.ap()[r0:r0 + P, :], h2ot[k][:])
            xgT_cur, ea_cur = xgT_next, ea_next

            state["sc_done"] = sc + 1
            try_gathers()
            while (state["s_next"] < NSB
                   and bcut[4 * (state["s_next"] + 1) - 1] + SLACK
                       <= state["sc_done"]):
                emit_superblock(state["s_next"])
                state["s_next"] += 1

        while state["s_next"] < NSB:
            emit_superblock(state["s_next"])
            state["s_next"] += 1

    nc.compile()
    return nc


def _prepare(x, row, col, ea):
    """Host-side sharding: sort edges by destination, split nodes into 8
    block-aligned edge-balanced shards, build per-core arrays."""
    N = x.shape[0]
    E = ea.shape[0]
    order = np.argsort(col, kind="stable")
    scol = col[order]
    srow = row[order]
    NBLK = (N + P - 1) // P
    NTOT = NBLK * P

    bounds = [0]
    for p in range(1, NCORES):
        if E > 0:
            t = int(scol[min((p * E) // NCORES, E - 1)])
        else:
            t = (p * NTOT) // NCORES
        b = int(round(t / P)) * P
        b = max(b, bounds[-1] + P)
        b = min(b, NTOT - P * (NCORES - p))
        bounds.append(b)
    bounds.append(NTOT)
    for p in range(1, NCORES + 1):
        assert bounds[p] > bounds[p - 1], f"degenerate shard bounds {bounds}"

    e_split = np.searchsorted(scol, bounds)
    Ec = np.diff(e_split)
    EC = max(4, math.ceil(int(Ec.max()) / P))
    EC = ((EC + 3) // 4) * 4
    EP = EC * P
    nblk = [(bounds[p + 1] - bounds[p]) // P for p in range(NCORES)]
    NB = max(4, ((max(nblk) + 3) // 4) * 4)
    NBP = NB * P
    blkdeg = np.bincount(scol // P, minlength=NBLK)
    KB = max(1, math.ceil(int(blkdeg.max()) / P))

    xbf = np.zeros((NTOT, FN), NPBF16)
    xbf[:N] = x.astype(NPBF16)
    xpadT = np.zeros((FN, NTOT + NBP), NPBF16)
    xpadT[:, :N] = xbf[:N].T

    cnt_all = np.bincount(scol, minlength=NTOT)

    cores = []
    bstarts = []
    for p in range(NCORES):
        s, e = int(e_split[p]), int(e_split[p + 1])
        n0 = bounds[p]
        ne = e - s
        tmp = np.zeros(EP, np.int64)
        tmp[:ne] = srow[s:e]
        # compacted source table + int16 remapped indices in dma_gather's
        # 16-partition-wrapped layout; superchunk 0 is staged pre-transposed
        uniq, ridx = np.unique(tmp, return_inverse=True)
        assert uniq.size <= 32767, "unique sources exceed int16 gather range"
        xsrc = xbf[uniq]
        ridx = ridx.astype(np.int16)
        SC = EC // 4
        gidx = np.tile(
            ridx.reshape(SC, 32, 16).transpose(2, 0, 1).reshape(16, SC * 32),
            (8, 1))
        xg0 = xsrc[ridx[:512]]  # [512 edges, FN]
        xgT0 = np.ascontiguousarray(
            xg0.T.reshape(4, P, 512).transpose(1, 0, 2))
        eaT = np.zeros((FE, EP), NPBF16)
        eaT[:, :ne] = ea[order[s:e]].T.astype(NPBF16)
        lcol = (scol[s:e] - n0).astype(np.int64)
        bstart = np.searchsorted(lcol, np.arange(NB + 1) * P)
        bstarts.append(bstart)
        gid = np.zeros((NB, KB, P), np.int32)
        colb = np.full((NB, KB, P), -1.0, np.float32)
        for b in range(NB):
            sb, eb = int(bstart[b]), int(bstart[b + 1])
            cnt = eb - sb
            assert cnt <= KB * P
            gid[b].reshape(-1)[:cnt] = np.arange(sb, eb, dtype=np.int32)
            colb[b].reshape(-1)[:cnt] = (lcol[sb:eb] - b * P)
        gid_t = np.ascontiguousarray(gid.reshape(NB * KB, P).T)
        colb_t = np.ascontiguousarray(
            colb.reshape(NB * KB, P).T.astype(NPBF16))
        # per-edge 1/count of the destination node (0 for pad edges so
        # their staged h2 rows are exactly zero)
        invce = np.zeros(EP, np.float32)
        invce[:ne] = 1.0 / np.maximum(cnt_all[scol[s:e]], 1.0)
        invce_t = np.ascontiguousarray(invce.reshape(EC, P).T.astype(NPBF16))
        xsT = np.ascontiguousarray(xpadT[:, n0:n0 + NBP])
        # per-node degree>0 indicator for the b2 scatter correction
        deg = np.zeros(NBP, np.float32)
        span = min(NBP, NTOT - n0)
        deg[:span] = cnt_all[n0:n0 + span]
        srow_t = (deg > 0).astype(NPBF16).reshape(1, NBP)
        cores.append(dict(xsrc=xsrc, gidx=gidx, xgT0=xgT0, eaT=eaT,
                          gid=gid_t, colb=colb_t,
                          invce=invce_t, xsT=xsT, srow=srow_t))

    # pad the compacted source tables to a uniform row count
    USZ = max(c["xsrc"].shape[0] for c in cores)
    for c in cores:
        u = c["xsrc"].shape[0]
        if u < USZ:
            c["xsrc"] = np.vstack([c["xsrc"], np.zeros((USZ - u, FN), NPBF16)])
        c["xsrc"] = np.ascontiguousarray(c["xsrc"])

    # uniform (max-over-cores) superchunk cut per node block: block b's h2
    # rows are complete once bcut[b] edge superchunks have run on every core
    bcut = tuple(
        int(max(math.ceil(bstarts[p][b + 1] / 512) for p in range(NCORES)))
        for b in range(NB))
    return cores, bounds, EC, NB, KB, USZ, bcut


def _run(inputs, trace=False):
    x = np.ascontiguousarray(np.asarray(inputs["x"], dtype=np.float32))
    ei = np.asarray(inputs["edge_index"])
    ea = np.ascontiguousarray(np.asarray(inputs["edge_attr"], dtype=np.float32))
    row = ei[0].astype(np.int64)
    col = ei[1].astype(np.int64)
    W1 = np.asarray(inputs["W1"], np.float32).astype(NPBF16)
    W2 = np.asarray(inputs["W2"], np.float32).astype(NPBF16)
    W3 = np.asarray(inputs["W3"], np.float32).astype(NPBF16)
    W4 = np.asarray(inputs["W4"], np.float32).astype(NPBF16)
    b1 = np.asarray(inputs["b1"], np.float32)
    b2 = np.asarray(inputs["b2"], np.float32)
    b3 = np.asarray(inputs["b3"], np.float32)
    b4 = np.asarray(inputs["b4"], np.float32)
    N = x.shape[0]

    cores, bounds, EC, NB, KB, USZ, bcut = _prepare(x, row, col, ea)
    has_b2 = bool(np.any(b2 != 0))

    key = (EC, NB, KB, USZ, bcut, has_b2)
    if key not in _prog_cache:
        _prog_cache[key] = _build(EC, NB, KB, USZ, bcut, has_b2)
    nc = _prog_cache[key]

    b1t = np.ascontiguousarray(b1.reshape(HID // P, P).T)
    b3t = np.ascontiguousarray(b3.reshape((FN + FE) // P, P).T)
    b4t = np.ascontiguousarray(b4.reshape(FN // P, P).T)
    iota = np.ascontiguousarray(
        np.broadcast_to(np.arange(P, dtype=np.float32), (P, P))).astype(NPBF16)

    in_maps = []
    for p in range(NCORES):
        c = cores[p]
        m = {
            "xsrc": c["xsrc"], "gidx": c["gidx"], "xgT0": c["xgT0"],
            "eaT": c["eaT"],
            "W1": W1, "W2": W2, "W3": W3, "W4": W4,
            "b1": b1t, "b3": b3t, "b4": b4t,
            "gid": c["gid"], "colb": c["colb"], "invce": c["invce"],
            "xsT": c["xsT"], "iota": iota,
        }
        if has_b2:
            m["b2r"] = np.ascontiguousarray(b2.reshape(1, HID).astype(NPBF16))
            m["srow"] = c["srow"]
        in_maps.append(m)

    res = run_bass_kernel_spmd(nc, in_maps, list(range(NCORES)), trace=trace)

    out = np.empty((N, FN), np.float32)
    for p in range(NCORES):
        n0, n1 = bounds[p], min(bounds[p + 1], N)
        if n1 > n0:
            out[n0:n1] = res.results[p]["outT"].T[:n1 - n0]
    return out, res


def kernel(**inputs) -> np.ndarray:
    out, _ = _run(inputs, trace=False)
    return out


# revision 46
# speedup vs baseline: 1.4150x; 1.0026x over previous
"""Trainium2 Bass kernel for nn_NodeModel (GNN message passing).

Reference computation:
    h   = relu(concat(x[row], edge_attr) @ W1 + b1) @ W2 + b2     # edge MLP
    agg = scatter_mean(h, col, N)                                  # per-dest mean
    out = relu(concat(x, agg) @ W3 + b3) @ W4 + b4                 # node MLP

Distribution strategy (8 cores, no collectives needed):
  - Sort edges by destination node; split destination nodes into 8
    block-aligned, edge-balanced shards.  Each core owns one node shard and
    ALL edges targeting it, so per-node sums are complete locally.
  - x is replicated; each core gathers x[row] for its edges with indirect
    DMA on-device.
  - All matmul operands are bf16 (fp32 PSUM accumulation): halves HBM
    traffic vs fp32r and speeds PE transposes 1.5x.
  - Edge MLP runs with weights stationary and activations kept transposed
    [feat, edge]; h2 rows (pre-scaled by 1/count of their destination) are
    staged to DRAM in bf16.
  - Scatter-mean per 128-node block: indirect-gather the block's h2 rows,
    build a one-hot selection matrix with is_equal against an iota, and
    matmul-accumulate h2^T @ S in PSUM -> aggT directly in [hid, node]
    layout (the mean's 1/count is pre-applied per-edge in the h2 exit copy).
  - The scatter + node-MLP work is INTERLEAVED into the edge phase: since
    edges are sorted by destination, node block b only needs the first
    bcut[b] edge superchunks.  A static schedule (max over cores, so the
    SPMD program is uniform) runs each superblock as soon as its edges are
    done, overlapping the h2 gather-back DMA with edge-MLP compute.
  - Node-MLP output stays transposed [feat, node]; un-transposed on host.
"""

import math
import sys
from contextlib import ExitStack

sys.path.insert(0, "/opt/trn_rl_repo")

import ml_dtypes
import numpy as np

import concourse.bass as bass
import concourse.tile as tile
from concourse import bacc, mybir
from concourse.bass_utils import run_bass_kernel_spmd

NCORES = 8
P = 128
FN = 512    # node feature dim
FE = 128    # edge feature dim
HID = 1280  # edge-MLP hidden/output dim
F32 = mybir.dt.float32
BF16 = mybir.dt.bfloat16
I32 = mybir.dt.int32
RELU = mybir.ActivationFunctionType.Relu
IDENT = mybir.ActivationFunctionType.Identity
NPBF16 = ml_dtypes.bfloat16

_prog_cache = {}


def _build(EC, NB, KB, USZ, bcut, has_b2):
    """Build the SPMD program for one core.

    EC: edge chunks (128 edges each) per core, multiple of 4.
    NB: node blocks (128 nodes each) per core, multiple of 4.
    KB: max edge chunks per node block (scatter schedule width).
    USZ: rows of the compacted per-core x source table (unique sources,
         int16-indexable so dma_gather's transpose path can be used).
    bcut: per node block, the number of edge superchunks that must be
          complete before its h2 rows exist (max over cores -> uniform).
    has_b2: emit the b2 (x) s_n rank-1 scatter correction (b2 cannot ride
          the h2 activation since the W2 stage keeps edges on partitions;
          it distributes through the mean as agg += b2 * [deg>0]).
    """
    EP = EC * P
    SC = EC // 4   # superchunks of 512 edges
    NSB = NB // 4  # superblocks of 512 nodes
    LOOKAHEAD = max(2, 12 // KB)  # h2-gather prefetch blocks (SBUF-bounded)
    SLACK = 1  # superchunks between a block's h2 completion and its scatter

    nc = bacc.Bacc("TRN2", target_bir_lowering=False, debug=False,
                   num_devices=NCORES)

    xsrc_d = nc.dram_tensor("xsrc", [USZ, FN], BF16, kind="ExternalInput")
    gidx_d = nc.dram_tensor("gidx", [P, SC * 32], mybir.dt.int16,
                            kind="ExternalInput")
    xgT0_d = nc.dram_tensor("xgT0", [P, 4, 512], BF16, kind="ExternalInput")
    eaT_d = nc.dram_tensor("eaT", [FE, EP], BF16, kind="ExternalInput")
    W1_d = nc.dram_tensor("W1", [FN + FE, HID], BF16, kind="ExternalInput")
    W2_d = nc.dram_tensor("W2", [HID, HID], BF16, kind="ExternalInput")
    W3_d = nc.dram_tensor("W3", [FN + HID, FN + FE], BF16, kind="ExternalInput")
    W4_d = nc.dram_tensor("W4", [FN + FE, FN], BF16, kind="ExternalInput")
    b1_d = nc.dram_tensor("b1", [P, HID // P], F32, kind="ExternalInput")
    if has_b2:
        b2r_d = nc.dram_tensor("b2r", [1, HID], BF16, kind="ExternalInput")
        srow_d = nc.dram_tensor("srow", [1, NB * P], BF16,
                                kind="ExternalInput")
    b3_d = nc.dram_tensor("b3", [P, (FN + FE) // P], F32, kind="ExternalInput")
    b4_d = nc.dram_tensor("b4", [P, FN // P], F32, kind="ExternalInput")
    gid_d = nc.dram_tensor("gid", [P, NB * KB], I32, kind="ExternalInput")
    colb_d = nc.dram_tensor("colb", [P, NB * KB], BF16, kind="ExternalInput")
    invce_d = nc.dram_tensor("invce", [P, EC], BF16, kind="ExternalInput")
    xsT_d = nc.dram_tensor("xsT", [FN, NB * P], BF16, kind="ExternalInput")
    iota_d = nc.dram_tensor("iota", [P, P], BF16, kind="ExternalInput")
    outT_d = nc.dram_tensor("outT", [FN, NB * P], F32, kind="ExternalOutput")
    h2_d = nc.dram_tensor("h2buf", [EP, HID], BF16)  # internal staging

    with tile.TileContext(nc) as tc, ExitStack() as ctx:
        cpool = ctx.enter_context(tc.tile_pool(name="const", bufs=1))
        wpool = ctx.enter_context(tc.tile_pool(name="wts", bufs=1))

        # Load order = sync-queue FIFO order: first the tensors the first
        # superchunk needs (its pre-transposed x rows are staged on host so
        # nothing waits on the SWDGE warm-up; b1; W1 split per contraction
        # chunk), then everything else behind them.
        b1t = cpool.tile([P, HID // P], F32)
        nc.sync.dma_start(b1t[:], b1_d.ap()[:])
        gidxt = cpool.tile([P, SC * 32], mybir.dt.int16)
        nc.sync.dma_start(gidxt[:], gidx_d.ap()[:])

        def load_weights():
            W1r = W1_d.ap().rearrange("(ko ki) m -> ki ko m", ki=P)
            W1t = []
            for k in range(5):
                w = wpool.tile([P, HID], BF16, name=f"W1_{k}", tag=f"W1_{k}")
                nc.sync.dma_start(w[:], W1r[:, k, :])
                W1t.append(w)
            W2t = wpool.tile([P, 10, HID], BF16)
            W2r = W2_d.ap().rearrange("(ko ki) m -> ki ko m", ki=P)
            for k in range(10):
                nc.sync.dma_start(W2t[:, k, :], W2r[:, k, :])
            iotat = cpool.tile([P, P], BF16)
            nc.sync.dma_start(iotat[:], iota_d.ap()[:])
            b3t = cpool.tile([P, (FN + FE) // P], F32)
            nc.sync.dma_start(b3t[:], b3_d.ap()[:])
            b4t = cpool.tile([P, FN // P], F32)
            nc.sync.dma_start(b4t[:], b4_d.ap()[:])
            gidt = cpool.tile([P, NB * KB], I32)
            nc.sync.dma_start(gidt[:], gid_d.ap()[:])
            colbt = cpool.tile([P, NB * KB], BF16)
            nc.sync.dma_start(colbt[:], colb_d.ap()[:])
            invcet = cpool.tile([P, EC], BF16)
            nc.sync.dma_start(invcet[:], invce_d.ap()[:])
            bsr = None
            if has_b2:
                b2rt = cpool.tile([1, HID], BF16)
                nc.sync.dma_start(b2rt[:], b2r_d.ap()[:])
                srt = cpool.tile([1, NB * P], BF16)
                nc.sync.dma_start(srt[:], srow_d.ap()[:])
                bsr = (b2rt, srt)
            W3t = wpool.tile([P, 14, FN + FE], BF16)
            nc.sync.dma_start(
                W3t[:], W3_d.ap().rearrange("(ko ki) m -> ki ko m", ki=P))
            W4t = wpool.tile([P, 5, FN], BF16)
            nc.sync.dma_start(
                W4t[:], W4_d.ap().rearrange("(ko ki) m -> ki ko m", ki=P))
            return W1t, W2t, W3t, W4t, iotat, b3t, b4t, gidt, colbt, invcet, bsr

        # ---- pools (all coexist: phases are interleaved) ----
        # PSUM is 8 banks of 2 KB, allocated per tile name at bank
        # granularity: mm 4x[P,512]f32 = 4 banks; smp holds the scatter
        # half-accumulator [P,5,P]f32 (2 banks per buf) -> 4 + 4 = 8.
        mmp = ctx.enter_context(tc.tile_pool(name="mm", bufs=4, space="PSUM"))
        smp = ctx.enter_context(tc.tile_pool(name="smp", bufs=2, space="PSUM"))

        xgTp = ctx.enter_context(tc.tile_pool(name="xgT", bufs=2))
        eap = ctx.enter_context(tc.tile_pool(name="ea", bufs=2))
        h1p = ctx.enter_context(tc.tile_pool(name="h1T", bufs=2))
        h2op = ctx.enter_context(tc.tile_pool(name="h2o", bufs=6))
        h2gp = ctx.enter_context(
            tc.tile_pool(name="h2g", bufs=(LOOKAHEAD + 1) * KB))
        Sp = ctx.enter_context(
            tc.tile_pool(name="Smat", bufs=(LOOKAHEAD + 1) * KB))
        aggTp = ctx.enter_context(tc.tile_pool(name="aggT", bufs=2))
        xsp = ctx.enter_context(tc.tile_pool(name="xs", bufs=2))
        h3p = ctx.enter_context(tc.tile_pool(name="h3T", bufs=1))
        oTp = ctx.enter_context(tc.tile_pool(name="oT", bufs=1))

        # ---------------- edge-phase helpers ----------------
        def issue_gather(sc):
            # dma_gather(transpose=True) delivers x rows already transposed
            # into [feat-chunk-partition, feat-chunk, edge] layout -- no PE
            # entry transposes.  Superchunk 0 is host-staged (plain DMA).
            xgTt = xgTp.tile([P, 4, 512], BF16)
            if sc == 0:
                nc.sync.dma_start(xgTt[:], xgT0_d.ap()[:])
            else:
                nc.gpsimd.dma_gather(
                    xgTt[:], xsrc_d.ap()[:],
                    gidxt[:, sc * 32:(sc + 1) * 32],
                    512, 512, FN, transpose=True)
            eat = eap.tile([P, 512], BF16)
            nc.sync.dma_start(
                eat[:], eaT_d.ap()[:, sc * 512:(sc + 1) * 512])
            return xgTt, eat

        # ---------------- scatter/node-phase helpers ----------------
        pend_gs = {}
        state = dict(g_next=0, b_next=0, s_next=0, sc_done=0,
                     xg_cur=None, ea_cur=None, xgT_cur=None, xst_cur=None)

        def gather_S(b):
            ext = max(bcut[b], 1) * 512  # h2 rows that exist by then
            lst = []
            for k in range(KB):
                c = b * KB + k
                # pad slots carry id 0 (not OOB-skip): every partition of the
                # tile gets written with finite data, so the zero one-hot
                # columns can never multiply stale NaN bit patterns.
                h2g = h2gp.tile([P, HID], BF16, name=f"h2g_{b}_{k}",
                                tag="h2g")
                nc.gpsimd.indirect_dma_start(
                    out=h2g[:], out_offset=None, in_=h2_d.ap()[:ext],
                    in_offset=bass.IndirectOffsetOnAxis(
                        ap=gidt[:, c:c + 1], axis=0))
                St = Sp.tile([P, P], BF16, name=f"S_{b}_{k}", tag="S")
                nc.vector.tensor_tensor(
                    St[:], colbt[:, c:c + 1].to_broadcast([P, P]),
                    iotat[:], op=mybir.AluOpType.is_equal)
                lst.append((h2g, St))
            pend_gs[b] = lst

        def try_gathers():
            while (state["g_next"] < NB
                   and bcut[state["g_next"]] <= state["sc_done"]
                   and state["g_next"] < state["b_next"] + LOOKAHEAD):
                gather_S(state["g_next"])
                state["g_next"] += 1

        def load_xst(s):
            xst = xsp.tile([P, 4, 512], BF16, name=f"xst_{s}", tag="xst")
            nc.sync.dma_start(
                xst[:],
                xsT_d.ap().rearrange("(fo fi) n -> fi fo n", fi=P)
                [:, :, s * 512:(s + 1) * 512])
            return xst

        outTr = outT_d.ap().rearrange("(fo fi) n -> fi fo n", fi=P)

        def emit_superblock(s):
            # scatter: accumulate aggT[hid, node] directly in PSUM with the
            # gathered h2 rows stationary and the one-hot S moving.
            aggTsb = aggTp.tile([P, 10, 512], BF16)
            for bb in range(4):
                b = s * 4 + bb
                gs = pend_gs.pop(b)
                # j-major: each 128-wide accumulation group's matmuls stay
                # consecutive (open groups must not interleave in a bank).
                for half in range(2):
                    pss = smp.tile([P, 5, P], F32, name="pss", tag="pss")
                    for j5 in range(5):
                        j = half * 5 + j5
                        for k, (h2g, St) in enumerate(gs):
                            nc.tensor.matmul(
                                pss[:, j5, :], h2g[:, j * P:(j + 1) * P],
                                St[:], start=(k == 0),
                                stop=(k == KB - 1 and not has_b2))
                        if has_b2:
                            # agg includes +b2 for nodes with deg>0: rank-1
                            # b2[j-slice] (x) s_row closes the group.
                            nc.tensor.matmul(
                                pss[:, j5, :], b2rt[:, j * P:(j + 1) * P],
                                srt[:, b * P:(b + 1) * P],
                                start=False, stop=True)
                    nc.vector.tensor_copy(
                        aggTsb[:, half * 5:(half + 1) * 5,
                               bb * P:(bb + 1) * P], pss[:])
                state["b_next"] = b + 1
                try_gathers()

            xst = state["xst_cur"]
            state["xst_cur"] = load_xst(s + 1) if s + 1 < NSB else None
            h3Tt = h3p.tile([P, 5, 512], BF16)
            for of in range(5):
                ps = mmp.tile([P, 512], F32)
                for k in range(4):
                    nc.tensor.matmul(
                        ps[:], W3t[:, k, of * P:(of + 1) * P],
                        xst[:, k, :], start=(k == 0), stop=False)
                for f in range(10):
                    nc.tensor.matmul(
                        ps[:], W3t[:, 4 + f, of * P:(of + 1) * P],
                        aggTsb[:, f, :], start=False, stop=(f == 9))
                nc.scalar.activation(h3Tt[:, of, :], ps[:], RELU,
                                     bias=b3t[:, of:of + 1])
            oTt = oTp.tile([P, 4, 512], F32)
            for of in range(4):
                ps = mmp.tile([P, 512], F32)
                for k in range(5):
                    nc.tensor.matmul(
                        ps[:], W4t[:, k, of * P:(of + 1) * P],
                        h3Tt[:, k, :], start=(k == 0), stop=(k == 4))
                nc.scalar.activation(
                    oTt[:, of, :], ps[:], IDENT, bias=b4t[:, of:of + 1])
            nc.sync.dma_start(outTr[:, :, s * 512:(s + 1) * 512], oTt[:])

        # ---------------- interleaved main loop ----------------
        # superchunk 0's inputs enter the DMA queue first; all weights and
        # scatter tables queue up behind them.
        xgT_cur, ea_cur = issue_gather(0)
        (W1t, W2t, W3t, W4t, iotat, b3t, b4t, gidt, colbt, invcet,
         bsr) = load_weights()
        if has_b2:
            b2rt, srt = bsr
        state["xst_cur"] = load_xst(0)

        for sc in range(SC):
            if sc + 1 < SC:
                xgT_next, ea_next = issue_gather(sc + 1)
            else:
                xgT_next = ea_next = None

            h1Tt = h1p.tile([P, 10, 512], BF16)
            for of in range(10):
                ps = mmp.tile([P, 512], F32)
                for k in range(5):
                    rhs = xgT_cur[:, k, :] if k < 4 else ea_cur[:]
                    nc.tensor.matmul(
                        ps[:], W1t[k][:, of * P:(of + 1) * P], rhs,
                        start=(k == 0), stop=(k == 4))
                nc.scalar.activation(h1Tt[:, of, :], ps[:], RELU,
                                     bias=b1t[:, of:of + 1])
            # W2 stage with h1T stationary and W2 moving: the product lands
            # directly in [edge, hid] layout -- no exit transposes.  The
            # per-edge 1/count of the destination is folded into the
            # PSUM->SBUF copy so the scatter can accumulate raw sums (b2,
            # which would vary along the free dim here, distributes through
            # the scatter-mean and is re-added there when nonzero).
            h2ot = [h2op.tile([P, HID], BF16, name=f"h2o_{sc}_{k}", tag="h2o")
                     for k in range(4)]
            for kk in range(4):
                c = sc * 4 + kk
                for lo, w in ((0, 512), (512, 512), (1024, 256)):
                    ps = mmp.tile([P, 512], F32)
                    for k in range(10):
                        nc.tensor.matmul(
                            ps[:, :w], h1Tt[:, k, kk * P:(kk + 1) * P],
                            W2t[:, k, lo:lo + w],
                            start=(k == 0), stop=(k == 9))
                    nc.vector.tensor_tensor(
                        h2ot[kk][:, lo:lo + w], ps[:, :w],
                        invcet[:, c:c + 1].to_broadcast([P, w]),
                        op=mybir.AluOpType.mult)
            for k in range(4):
                r0 = sc * 512 + k * P
                nc.sync.dma_start(h2_d.ap()[r0:r0 + P, :], h2ot[k][:])
            xgT_cur, ea_cur = xgT_next, ea_next

            state["sc_done"] = sc + 1
            try_gathers()
            while (state["s_next"] < NSB
                   and bcut[4 * (state["s_next"] + 1) - 1] + SLACK
                       <= state["sc_done"]):
                emit_superblock(state["s_next"])
                state["s_next"] += 1

        while state["s_next"] < NSB:
            emit_superblock(state["s_next"])
            state["s_next"] += 1

    nc.compile()
    return nc


def _prepare(x, row, col, ea):
    """Host-side sharding: sort edges by destination, split nodes into 8
    block-aligned edge-balanced shards, build per-core arrays."""
    N = x.shape[0]
    E = ea.shape[0]
    order = np.argsort(col, kind="stable")
    scol = col[order]
    srow = row[order]
    NBLK = (N + P - 1) // P
    NTOT = NBLK * P

    bounds = [0]
    for p in range(1, NCORES):
        if E > 0:
            t = int(scol[min((p * E) // NCORES, E - 1)])
        else:
            t = (p * NTOT) // NCORES
        b = int(round(t / P)) * P
        b = max(b, bounds[-1] + P)
        b = min(b, NTOT - P * (NCORES - p))
        bounds.append(b)
    bounds.append(NTOT)
    for p in range(1, NCORES + 1):
        assert bounds[p] > bounds[p - 1], f"degenerate shard bounds {bounds}"

    e_split = np.searchsorted(scol, bounds)
    Ec = np.diff(e_split)
    EC = max(4, math.ceil(int(Ec.max()) / P))
    EC = ((EC + 3) // 4) * 4
    EP = EC * P
    nblk = [(bounds[p + 1] - bounds[p]) // P for p in range(NCORES)]
    NB = max(4, ((max(nblk) + 3) // 4) * 4)
    NBP = NB * P
    blkdeg = np.bincount(scol // P, minlength=NBLK)
    KB = max(1, math.ceil(int(blkdeg.max()) / P))

    xbf = np.zeros((NTOT, FN), NPBF16)
    xbf[:N] = x.astype(NPBF16)
    xpadT = np.zeros((FN, NTOT + NBP), NPBF16)
    xpadT[:, :N] = xbf[:N].T

    cnt_all = np.bincount(scol, minlength=NTOT)

    cores = []
    bstarts = []
    for p in range(NCORES):
        s, e = int(e_split[p]), int(e_split[p + 1])
        n0 = bounds[p]
        ne = e - s
        tmp = np.zeros(EP, np.int64)
        tmp[:ne] = srow[s:e]
        # compacted source table + int16 remapped indices in dma_gather's
        # 16-partition-wrapped layout; superchunk 0 is staged pre-transposed
        uniq, ridx = np.unique(tmp, return_inverse=True)
        assert uniq.size <= 32767, "unique sources exceed int16 gather range"
        xsrc = xbf[uniq]
        ridx = ridx.astype(np.int16)
        SC = EC // 4
        gidx = np.tile(
            ridx.reshape(SC, 32, 16).transpose(2, 0, 1).reshape(16, SC * 32),
            (8, 1))
        xg0 = xsrc[ridx[:512]]  # [512 edges, FN]
        xgT0 = np.ascontiguousarray(
            xg0.T.reshape(4, P, 512).transpose(1, 0, 2))
        eaT = np.zeros((FE, EP), NPBF16)
        eaT[:, :ne] = ea[order[s:e]].T.astype(NPBF16)
        lcol = (scol[s:e] - n0).astype(np.int64)
        bstart = np.searchsorted(lcol, np.arange(NB + 1) * P)
        bstarts.append(bstart)
        gid = np.zeros((NB, KB, P), np.int32)
        colb = np.full((NB, KB, P), -1.0, np.float32)
        for b in range(NB):
            sb, eb = int(bstart[b]), int(bstart[b + 1])
            cnt = eb - sb
            assert cnt <= KB * P
            gid[b].reshape(-1)[:cnt] = np.arange(sb, eb, dtype=np.int32)
            colb[b].reshape(-1)[:cnt] = (lcol[sb:eb] - b * P)
        gid_t = np.ascontiguousarray(gid.reshape(NB * KB, P).T)
        colb_t = np.ascontiguousarray(
            colb.reshape(NB * KB, P).T.astype(NPBF16))
        # per-edge 1/count of the destination node (0 for pad edges so
        # their staged h2 rows are exactly zero)
        invce = np.zeros(EP, np.float32)
        invce[:ne] = 1.0 / np.maximum(cnt_all[scol[s:e]], 1.0)
        invce_t = np.ascontiguousarray(invce.reshape(EC, P).T.astype(NPBF16))
        xsT = np.ascontiguousarray(xpadT[:, n0:n0 + NBP])
        # per-node degree>0 indicator for the b2 scatter correction
        deg = np.zeros(NBP, np.float32)
        span = min(NBP, NTOT - n0)
        deg[:span] = cnt_all[n0:n0 + span]
        srow_t = (deg > 0).astype(NPBF16).reshape(1, NBP)
        cores.append(dict(xsrc=xsrc, gidx=gidx, xgT0=xgT0, eaT=eaT,
                          gid=gid_t, colb=colb_t,
                          invce=invce_t, xsT=xsT, srow=srow_t))

    # pad the compacted source tables to a uniform row count
    USZ = max(c["xsrc"].shape[0] for c in cores)
    for c in cores:
        u = c["xsrc"].shape[0]
        if u < USZ:
            c["xsrc"] = np.vstack([c["xsrc"], np.zeros((USZ - u, FN), NPBF16)])
        c["xsrc"] = np.ascontiguousarray(c["xsrc"])

    # uniform (max-over-cores) superchunk cut per node block: block b's h2
    # rows are complete once bcut[b] edge superchunks have run on every core
    bcut = tuple(
        int(max(math.ceil(bstarts[p][b + 1] / 512) for p in range(NCORES)))
        for b in range(NB))
    return cores, bounds, EC, NB, KB, USZ, bcut


def _run(inputs, trace=False):
    x = np.ascontiguousarray(np.asarray(inputs["x"], dtype=np.float32))
    ei = np.asarray(inputs["edge_index"])
    ea = np.ascontiguousarray(np.asarray(inputs["edge_attr"], dtype=np.float32))
    row = ei[0].astype(np.int64)
    col = ei[1].astype(np.int64)
    W1 = np.asarray(inputs["W1"], np.float32).astype(NPBF16)
    W2 = np.asarray(inputs["W2"], np.float32).astype(NPBF16)
    W3 = np.asarray(inputs["W3"], np.float32).astype(NPBF16)
    W4 = np.asarray(inputs["W4"], np.float32).astype(NPBF16)
    b1 = np.asarray(inputs["b1"], np.float32)
    b2 = np.asarray(inputs["b2"], np.float32)
    b3 = np.asarray(inputs["b3"], np.float32)
    b4 = np.asarray(inputs["b4"], np.float32)
    N = x.shape[0]

    cores, bounds, EC, NB, KB, USZ, bcut = _prepare(x, row, col, ea)
    has_b2 = bool(np.any(b2 != 0))

    key = (EC, NB, KB, USZ, bcut, has_b2)
    if key not in _prog_cache:
        _prog_cache[key] = _build(EC, NB, KB, USZ, bcut, has_b2)
    nc = _prog_cache[key]

    b1t = np.ascontiguousarray(b1.reshape(HID // P, P).T)
    b3t = np.ascontiguousarray(b3.reshape((FN + FE) // P, P).T)
    b4t = np.ascontiguousarray(b4.reshape(FN // P, P).T)
    iota = np.ascontiguousarray(
        np.broadcast_to(np.arange(P, dtype=np.float32), (P, P))).astype(NPBF16)

    in_maps = []
    for p in range(NCORES):
        c = cores[p]
        m = {
            "xsrc": c["xsrc"], "gidx": c["gidx"], "xgT0": c["xgT0"],
            "eaT": c["eaT"],
            "W1": W1, "W2": W2, "W3": W3, "W4": W4,
            "b1": b1t, "b3": b3t, "b4": b4t,
            "gid": c["gid"], "colb": c["colb"], "invce": c["invce"],
            "xsT": c["xsT"], "iota": iota,
        }
        if has_b2:
            m["b2r"] = np.ascontiguousarray(b2.reshape(1, HID).astype(NPBF16))
            m["srow"] = c["srow"]
        in_maps.append(m)

    res = run_bass_kernel_spmd(nc, in_maps, list(range(NCORES)), trace=trace)

    out = np.empty((N, FN), np.float32)
    for p in range(NCORES):
        n0, n1 = bounds[p], min(bounds[p + 1], N)
        if n1 > n0:
            out[n0:n1] = res.results[p]["outT"].T[:n1 - n0]
    return out, res


def kernel(**inputs) -> np.ndarray:
    out, _ = _run(inputs, trace=False)
    return out


# revision 49
# speedup vs baseline: 1.4160x; 1.0007x over previous
"""Trainium2 Bass kernel for nn_NodeModel (GNN message passing).

Reference computation:
    h   = relu(concat(x[row], edge_attr) @ W1 + b1) @ W2 + b2     # edge MLP
    agg = scatter_mean(h, col, N)                                  # per-dest mean
    out = relu(concat(x, agg) @ W3 + b3) @ W4 + b4                 # node MLP

Distribution strategy (8 cores, no collectives needed):
  - Sort edges by destination node; split destination nodes into 8
    block-aligned, edge-balanced shards.  Each core owns one node shard and
    ALL edges targeting it, so per-node sums are complete locally.
  - x is replicated; each core gathers x[row] for its edges with indirect
    DMA on-device.
  - All matmul operands are bf16 (fp32 PSUM accumulation): halves HBM
    traffic vs fp32r and speeds PE transposes 1.5x.
  - Edge MLP runs with weights stationary and activations kept transposed
    [feat, edge]; h2 rows (pre-scaled by 1/count of their destination) are
    staged to DRAM in bf16.
  - Scatter-mean per 128-node block: indirect-gather the block's h2 rows,
    build a one-hot selection matrix with is_equal against an iota, and
    matmul-accumulate h2^T @ S in PSUM -> aggT directly in [hid, node]
    layout (the mean's 1/count is pre-applied per-edge in the h2 exit copy).
  - The scatter + node-MLP work is INTERLEAVED into the edge phase: since
    edges are sorted by destination, node block b only needs the first
    bcut[b] edge superchunks.  A static schedule (max over cores, so the
    SPMD program is uniform) runs each superblock as soon as its edges are
    done, overlapping the h2 gather-back DMA with edge-MLP compute.
  - Node-MLP output stays transposed [feat, node]; un-transposed on host.
"""

import math
import sys
from contextlib import ExitStack

sys.path.insert(0, "/opt/trn_rl_repo")

import ml_dtypes
import numpy as np

import concourse.bass as bass
import concourse.tile as tile
from concourse import bacc, mybir
from concourse.bass_utils import run_bass_kernel_spmd

NCORES = 8
P = 128
FN = 512    # node feature dim
FE = 128    # edge feature dim
HID = 1280  # edge-MLP hidden/output dim
F32 = mybir.dt.float32
BF16 = mybir.dt.bfloat16
I32 = mybir.dt.int32
RELU = mybir.ActivationFunctionType.Relu
IDENT = mybir.ActivationFunctionType.Identity
NPBF16 = ml_dtypes.bfloat16

_prog_cache = {}


def _build(EC, NB, KB, USZ, bcut, has_b2):
    """Build the SPMD program for one core.

    EC: edge chunks (128 edges each) per core, multiple of 4.
    NB: node blocks (128 nodes each) per core, multiple of 4.
    KB: max edge chunks per node block (scatter schedule width).
    USZ: rows of the compacted per-core x source table (unique sources,
         int16-indexable so dma_gather's transpose path can be used).
    bcut: per node block, the number of edge superchunks that must be
          complete before its h2 rows exist (max over cores -> uniform).
    has_b2: emit the b2 (x) s_n rank-1 scatter correction (b2 cannot ride
          the h2 activation since the W2 stage keeps edges on partitions;
          it distributes through the mean as agg += b2 * [deg>0]).
    """
    EP = EC * P
    SC = EC // 4   # superchunks of 512 edges
    NSB = NB // 4  # superblocks of 512 nodes
    LOOKAHEAD = max(2, 12 // KB)  # h2-gather prefetch blocks (SBUF-bounded)
    SLACK = 1  # superchunks between a block's h2 completion and its scatter

    nc = bacc.Bacc("TRN2", target_bir_lowering=False, debug=False,
                   num_devices=NCORES)

    xsrc_d = nc.dram_tensor("xsrc", [USZ, FN], BF16, kind="ExternalInput")
    gidx_d = nc.dram_tensor("gidx", [P, SC * 32], mybir.dt.int16,
                            kind="ExternalInput")
    xgT0_d = nc.dram_tensor("xgT0", [P, 4, 512], BF16, kind="ExternalInput")
    eaT_d = nc.dram_tensor("eaT", [FE, EP], BF16, kind="ExternalInput")
    W1_d = nc.dram_tensor("W1", [FN + FE, HID], BF16, kind="ExternalInput")
    W2_d = nc.dram_tensor("W2", [HID, HID], BF16, kind="ExternalInput")
    W3_d = nc.dram_tensor("W3", [FN + HID, FN + FE], BF16, kind="ExternalInput")
    W4_d = nc.dram_tensor("W4", [FN + FE, FN], BF16, kind="ExternalInput")
    b1_d = nc.dram_tensor("b1", [P, HID // P], F32, kind="ExternalInput")
    if has_b2:
        b2r_d = nc.dram_tensor("b2r", [1, HID], BF16, kind="ExternalInput")
        srow_d = nc.dram_tensor("srow", [1, NB * P], BF16,
                                kind="ExternalInput")
    b3_d = nc.dram_tensor("b3", [P, (FN + FE) // P], F32, kind="ExternalInput")
    b4_d = nc.dram_tensor("b4", [P, FN // P], F32, kind="ExternalInput")
    gid_d = nc.dram_tensor("gid", [P, NB * KB], I32, kind="ExternalInput")
    colb_d = nc.dram_tensor("colb", [P, NB * KB], BF16, kind="ExternalInput")
    invce_d = nc.dram_tensor("invce", [P, EC], BF16, kind="ExternalInput")
    xsT_d = nc.dram_tensor("xsT", [FN, NB * P], BF16, kind="ExternalInput")
    iota_d = nc.dram_tensor("iota", [P, P], BF16, kind="ExternalInput")
    outT_d = nc.dram_tensor("outT", [FN, NB * P], F32, kind="ExternalOutput")
    h2_d = nc.dram_tensor("h2buf", [EP, HID], BF16)  # internal staging

    with tile.TileContext(nc) as tc, ExitStack() as ctx:
        cpool = ctx.enter_context(tc.tile_pool(name="const", bufs=1))
        wpool = ctx.enter_context(tc.tile_pool(name="wts", bufs=1))

        # Load order = sync-queue FIFO order: first the tensors the first
        # superchunk needs (its pre-transposed x rows are staged on host so
        # nothing waits on the SWDGE warm-up; b1; W1 split per contraction
        # chunk), then everything else behind them.
        b1t = cpool.tile([P, HID // P], F32)
        nc.sync.dma_start(b1t[:], b1_d.ap()[:])
        gidxt = cpool.tile([P, SC * 32], mybir.dt.int16)
        nc.sync.dma_start(gidxt[:], gidx_d.ap()[:])

        def load_weights():
            W1r = W1_d.ap().rearrange("(ko ki) m -> ki ko m", ki=P)
            W1t = [wpool.tile([P, HID], BF16, name=f"W1_{k}", tag=f"W1_{k}")
                   for k in range(5)]
            for k in (4, 0, 1, 2, 3):  # ea chunk's weights first
                nc.sync.dma_start(W1t[k][:], W1r[:, k, :])
            W2t = wpool.tile([P, 10, HID], BF16)
            W2r = W2_d.ap().rearrange("(ko ki) m -> ki ko m", ki=P)
            for k in range(10):
                nc.sync.dma_start(W2t[:, k, :], W2r[:, k, :])
            iotat = cpool.tile([P, P], BF16)
            nc.sync.dma_start(iotat[:], iota_d.ap()[:])
            b3t = cpool.tile([P, (FN + FE) // P], F32)
            nc.sync.dma_start(b3t[:], b3_d.ap()[:])
            b4t = cpool.tile([P, FN // P], F32)
            nc.sync.dma_start(b4t[:], b4_d.ap()[:])
            gidt = cpool.tile([P, NB * KB], I32)
            nc.sync.dma_start(gidt[:], gid_d.ap()[:])
            colbt = cpool.tile([P, NB * KB], BF16)
            nc.sync.dma_start(colbt[:], colb_d.ap()[:])
            invcet = cpool.tile([P, EC], BF16)
            nc.sync.dma_start(invcet[:], invce_d.ap()[:])
            bsr = None
            if has_b2:
                b2rt = cpool.tile([1, HID], BF16)
                nc.sync.dma_start(b2rt[:], b2r_d.ap()[:])
                srt = cpool.tile([1, NB * P], BF16)
                nc.sync.dma_start(srt[:], srow_d.ap()[:])
                bsr = (b2rt, srt)
            W3t = wpool.tile([P, 14, FN + FE], BF16)
            nc.sync.dma_start(
                W3t[:], W3_d.ap().rearrange("(ko ki) m -> ki ko m", ki=P))
            W4t = wpool.tile([P, 5, FN], BF16)
            nc.sync.dma_start(
                W4t[:], W4_d.ap().rearrange("(ko ki) m -> ki ko m", ki=P))
            return W1t, W2t, W3t, W4t, iotat, b3t, b4t, gidt, colbt, invcet, bsr

        # ---- pools (all coexist: phases are interleaved) ----
        # PSUM is 8 banks of 2 KB, allocated per tile name at bank
        # granularity: mm 4x[P,512]f32 = 4 banks; smp holds the scatter
        # half-accumulator [P,5,P]f32 (2 banks per buf) -> 4 + 4 = 8.
        mmp = ctx.enter_context(tc.tile_pool(name="mm", bufs=4, space="PSUM"))
        smp = ctx.enter_context(tc.tile_pool(name="smp", bufs=2, space="PSUM"))

        xgTp = ctx.enter_context(tc.tile_pool(name="xgT", bufs=2))
        eap = ctx.enter_context(tc.tile_pool(name="ea", bufs=2))
        h1p = ctx.enter_context(tc.tile_pool(name="h1T", bufs=2))
        h2op = ctx.enter_context(tc.tile_pool(name="h2o", bufs=6))
        h2gp = ctx.enter_context(
            tc.tile_pool(name="h2g", bufs=(LOOKAHEAD + 1) * KB))
        Sp = ctx.enter_context(
            tc.tile_pool(name="Smat", bufs=(LOOKAHEAD + 1) * KB))
        aggTp = ctx.enter_context(tc.tile_pool(name="aggT", bufs=2))
        xsp = ctx.enter_context(tc.tile_pool(name="xs", bufs=2))
        h3p = ctx.enter_context(tc.tile_pool(name="h3T", bufs=1))
        oTp = ctx.enter_context(tc.tile_pool(name="oT", bufs=1))

        # ---------------- edge-phase helpers ----------------
        def issue_gather(sc):
            # dma_gather(transpose=True) delivers x rows already transposed
            # into [feat-chunk-partition, feat-chunk, edge] layout -- no PE
            # entry transposes.  Superchunk 0 is host-staged (plain DMA).
            xgTt = xgTp.tile([P, 4, 512], BF16)
            eat = eap.tile([P, 512], BF16)
            if sc == 0:
                # ea first: the W1 group starts on the ea chunk, so the very
                # first matmul only waits for this small load plus W1_4.
                nc.sync.dma_start(eat[:], eaT_d.ap()[:, :512])
                nc.sync.dma_start(xgTt[:], xgT0_d.ap()[:])
            else:
                nc.gpsimd.dma_gather(
                    xgTt[:], xsrc_d.ap()[:],
                    gidxt[:, sc * 32:(sc + 1) * 32],
                    512, 512, FN, transpose=True)
                nc.sync.dma_start(
                    eat[:], eaT_d.ap()[:, sc * 512:(sc + 1) * 512])
            return xgTt, eat

        # ---------------- scatter/node-phase helpers ----------------
        pend_gs = {}
        state = dict(g_next=0, b_next=0, s_next=0, sc_done=0,
                     xg_cur=None, ea_cur=None, xgT_cur=None, xst_cur=None)

        def gather_S(b):
            ext = max(bcut[b], 1) * 512  # h2 rows that exist by then
            lst = []
            for k in range(KB):
                c = b * KB + k
                # pad slots carry id 0 (not OOB-skip): every partition of the
                # tile gets written with finite data, so the zero one-hot
                # columns can never multiply stale NaN bit patterns.
                h2g = h2gp.tile([P, HID], BF16, name=f"h2g_{b}_{k}",
                                tag="h2g")
                nc.gpsimd.indirect_dma_start(
                    out=h2g[:], out_offset=None, in_=h2_d.ap()[:ext],
                    in_offset=bass.IndirectOffsetOnAxis(
                        ap=gidt[:, c:c + 1], axis=0))
                St = Sp.tile([P, P], BF16, name=f"S_{b}_{k}", tag="S")
                nc.vector.tensor_tensor(
                    St[:], colbt[:, c:c + 1].to_broadcast([P, P]),
                    iotat[:], op=mybir.AluOpType.is_equal)
                lst.append((h2g, St))
            pend_gs[b] = lst

        def try_gathers():
            while (state["g_next"] < NB
                   and bcut[state["g_next"]] <= state["sc_done"]
                   and state["g_next"] < state["b_next"] + LOOKAHEAD):
                gather_S(state["g_next"])
                state["g_next"] += 1

        def load_xst(s):
            xst = xsp.tile([P, 4, 512], BF16, name=f"xst_{s}", tag="xst")
            nc.sync.dma_start(
                xst[:],
                xsT_d.ap().rearrange("(fo fi) n -> fi fo n", fi=P)
                [:, :, s * 512:(s + 1) * 512])
            return xst

        outTr = outT_d.ap().rearrange("(fo fi) n -> fi fo n", fi=P)

        def emit_superblock(s):
            # scatter: accumulate aggT[hid, node] directly in PSUM with the
            # gathered h2 rows stationary and the one-hot S moving.
            aggTsb = aggTp.tile([P, 10, 512], BF16)
            for bb in range(4):
                b = s * 4 + bb
                gs = pend_gs.pop(b)
                # j-major: each 128-wide accumulation group's matmuls stay
                # consecutive (open groups must not interleave in a bank).
                for half in range(2):
                    pss = smp.tile([P, 5, P], F32, name="pss", tag="pss")
                    for j5 in range(5):
                        j = half * 5 + j5
                        for k, (h2g, St) in enumerate(gs):
                            nc.tensor.matmul(
                                pss[:, j5, :], h2g[:, j * P:(j + 1) * P],
                                St[:], start=(k == 0),
                                stop=(k == KB - 1 and not has_b2))
                        if has_b2:
                            # agg includes +b2 for nodes with deg>0: rank-1
                            # b2[j-slice] (x) s_row closes the group.
                            nc.tensor.matmul(
                                pss[:, j5, :], b2rt[:, j * P:(j + 1) * P],
                                srt[:, b * P:(b + 1) * P],
                                start=False, stop=True)
                    nc.vector.tensor_copy(
                        aggTsb[:, half * 5:(half + 1) * 5,
                               bb * P:(bb + 1) * P], pss[:])
                state["b_next"] = b + 1
                try_gathers()

            xst = state["xst_cur"]
            state["xst_cur"] = load_xst(s + 1) if s + 1 < NSB else None
            h3Tt = h3p.tile([P, 5, 512], BF16)
            for of in range(5):
                ps = mmp.tile([P, 512], F32)
                for k in range(4):
                    nc.tensor.matmul(
                        ps[:], W3t[:, k, of * P:(of + 1) * P],
                        xst[:, k, :], start=(k == 0), stop=False)
                for f in range(10):
                    nc.tensor.matmul(
                        ps[:], W3t[:, 4 + f, of * P:(of + 1) * P],
                        aggTsb[:, f, :], start=False, stop=(f == 9))
                nc.scalar.activation(h3Tt[:, of, :], ps[:], RELU,
                                     bias=b3t[:, of:of + 1])
            oTt = oTp.tile([P, 4, 512], F32)
            for of in range(4):
                ps = mmp.tile([P, 512], F32)
                for k in range(5):
                    nc.tensor.matmul(
                        ps[:], W4t[:, k, of * P:(of + 1) * P],
                        h3Tt[:, k, :], start=(k == 0), stop=(k == 4))
                nc.scalar.activation(
                    oTt[:, of, :], ps[:], IDENT, bias=b4t[:, of:of + 1])
            nc.sync.dma_start(outTr[:, :, s * 512:(s + 1) * 512], oTt[:])

        # ---------------- interleaved main loop ----------------
        # superchunk 0's inputs enter the DMA queue first; all weights and
        # scatter tables queue up behind them.
        xgT_cur, ea_cur = issue_gather(0)
        (W1t, W2t, W3t, W4t, iotat, b3t, b4t, gidt, colbt, invcet,
         bsr) = load_weights()
        if has_b2:
            b2rt, srt = bsr
        state["xst_cur"] = load_xst(0)

        for sc in range(SC):
            if sc + 1 < SC:
                xgT_next, ea_next = issue_gather(sc + 1)
            else:
                xgT_next = ea_next = None

            h1Tt = h1p.tile([P, 10, 512], BF16)
            for of in range(10):
                ps = mmp.tile([P, 512], F32)
                for i, k in enumerate((4, 0, 1, 2, 3)):
                    rhs = xgT_cur[:, k, :] if k < 4 else ea_cur[:]
                    nc.tensor.matmul(
                        ps[:], W1t[k][:, of * P:(of + 1) * P], rhs,
                        start=(i == 0), stop=(i == 4))
                nc.scalar.activation(h1Tt[:, of, :], ps[:], RELU,
                                     bias=b1t[:, of:of + 1])
            # W2 stage with h1T stationary and W2 moving: the product lands
            # directly in [edge, hid] layout -- no exit transposes.  The
            # per-edge 1/count of the destination is folded into the
            # PSUM->SBUF copy so the scatter can accumulate raw sums (b2,
            # which would vary along the free dim here, distributes through
            # the scatter-mean and is re-added there when nonzero).
            h2ot = [h2op.tile([P, HID], BF16, name=f"h2o_{sc}_{k}", tag="h2o")
                     for k in range(4)]
            for kk in range(4):
                c = sc * 4 + kk
                for lo, w in ((0, 512), (512, 512), (1024, 256)):
                    ps = mmp.tile([P, 512], F32)
                    for k in range(10):
                        nc.tensor.matmul(
                            ps[:, :w], h1Tt[:, k, kk * P:(kk + 1) * P],
                            W2t[:, k, lo:lo + w],
                            start=(k == 0), stop=(k == 9))
                    nc.vector.tensor_tensor(
                        h2ot[kk][:, lo:lo + w], ps[:, :w],
                        invcet[:, c:c + 1].to_broadcast([P, w]),
                        op=mybir.AluOpType.mult)
            for k in range(4):
                r0 = sc * 512 + k * P
                nc.sync.dma_start(h2_d.ap()[r0:r0 + P, :], h2ot[k][:])
            xgT_cur, ea_cur = xgT_next, ea_next

            state["sc_done"] = sc + 1
            try_gathers()
            while (state["s_next"] < NSB
                   and bcut[4 * (state["s_next"] + 1) - 1] + SLACK
                       <= state["sc_done"]):
                emit_superblock(state["s_next"])
                state["s_next"] += 1

        while state["s_next"] < NSB:
            emit_superblock(state["s_next"])
            state["s_next"] += 1

    nc.compile()
    return nc


def _prepare(x, row, col, ea):
    """Host-side sharding: sort edges by destination, split nodes into 8
    block-aligned edge-balanced shards, build per-core arrays."""
    N = x.shape[0]
    E = ea.shape[0]
    order = np.argsort(col, kind="stable")
    scol = col[order]
    srow = row[order]
    NBLK = (N + P - 1) // P
    NTOT = NBLK * P

    bounds = [0]
    for p in range(1, NCORES):
        if E > 0:
            t = int(scol[min((p * E) // NCORES, E - 1)])
        else:
            t = (p * NTOT) // NCORES
        b = int(round(t / P)) * P
        b = max(b, bounds[-1] + P)
        b = min(b, NTOT - P * (NCORES - p))
        bounds.append(b)
    bounds.append(NTOT)
    for p in range(1, NCORES + 1):
        assert bounds[p] > bounds[p - 1], f"degenerate shard bounds {bounds}"

    e_split = np.searchsorted(scol, bounds)
    Ec = np.diff(e_split)
    EC = max(4, math.ceil(int(Ec.max()) / P))
    EC = ((EC + 3) // 4) * 4
    EP = EC * P
    nblk = [(bounds[p + 1] - bounds[p]) // P for p in range(NCORES)]
    NB = max(4, ((max(nblk) + 3) // 4) * 4)
    NBP = NB * P
    blkdeg = np.bincount(scol // P, minlength=NBLK)
    KB = max(1, math.ceil(int(blkdeg.max()) / P))

    xbf = np.zeros((NTOT, FN), NPBF16)
    xbf[:N] = x.astype(NPBF16)
    xpadT = np.zeros((FN, NTOT + NBP), NPBF16)
    xpadT[:, :N] = xbf[:N].T

    cnt_all = np.bincount(scol, minlength=NTOT)

    cores = []
    bstarts = []
    for p in range(NCORES):
        s, e = int(e_split[p]), int(e_split[p + 1])
        n0 = bounds[p]
        ne = e - s
        tmp = np.zeros(EP, np.int64)
        tmp[:ne] = srow[s:e]
        # compacted source table + int16 remapped indices in dma_gather's
        # 16-partition-wrapped layout; superchunk 0 is staged pre-transposed
        uniq, ridx = np.unique(tmp, return_inverse=True)
        assert uniq.size <= 32767, "unique sources exceed int16 gather range"
        xsrc = xbf[uniq]
        ridx = ridx.astype(np.int16)
        SC = EC // 4
        gidx = np.tile(
            ridx.reshape(SC, 32, 16).transpose(2, 0, 1).reshape(16, SC * 32),
            (8, 1))
        xg0 = xsrc[ridx[:512]]  # [512 edges, FN]
        xgT0 = np.ascontiguousarray(
            xg0.T.reshape(4, P, 512).transpose(1, 0, 2))
        eaT = np.zeros((FE, EP), NPBF16)
        eaT[:, :ne] = ea[order[s:e]].T.astype(NPBF16)
        lcol = (scol[s:e] - n0).astype(np.int64)
        bstart = np.searchsorted(lcol, np.arange(NB + 1) * P)
        bstarts.append(bstart)
        gid = np.zeros((NB, KB, P), np.int32)
        colb = np.full((NB, KB, P), -1.0, np.float32)
        for b in range(NB):
            sb, eb = int(bstart[b]), int(bstart[b + 1])
            cnt = eb - sb
            assert cnt <= KB * P
            gid[b].reshape(-1)[:cnt] = np.arange(sb, eb, dtype=np.int32)
            colb[b].reshape(-1)[:cnt] = (lcol[sb:eb] - b * P)
        gid_t = np.ascontiguousarray(gid.reshape(NB * KB, P).T)
        colb_t = np.ascontiguousarray(
            colb.reshape(NB * KB, P).T.astype(NPBF16))
        # per-edge 1/count of the destination node (0 for pad edges so
        # their staged h2 rows are exactly zero)
        invce = np.zeros(EP, np.float32)
        invce[:ne] = 1.0 / np.maximum(cnt_all[scol[s:e]], 1.0)
        invce_t = np.ascontiguousarray(invce.reshape(EC, P).T.astype(NPBF16))
        xsT = np.ascontiguousarray(xpadT[:, n0:n0 + NBP])
        # per-node degree>0 indicator for the b2 scatter correction
        deg = np.zeros(NBP, np.float32)
        span = min(NBP, NTOT - n0)
        deg[:span] = cnt_all[n0:n0 + span]
        srow_t = (deg > 0).astype(NPBF16).reshape(1, NBP)
        cores.append(dict(xsrc=xsrc, gidx=gidx, xgT0=xgT0, eaT=eaT,
                          gid=gid_t, colb=colb_t,
                          invce=invce_t, xsT=xsT, srow=srow_t))

    # pad the compacted source tables to a uniform row count
    USZ = max(c["xsrc"].shape[0] for c in cores)
    for c in cores:
        u = c["xsrc"].shape[0]
        if u < USZ:
            c["xsrc"] = np.vstack([c["xsrc"], np.zeros((USZ - u, FN), NPBF16)])
        c["xsrc"] = np.ascontiguousarray(c["xsrc"])

    # uniform (max-over-cores) superchunk cut per node block: block b's h2
    # rows are complete once bcut[b] edge superchunks have run on every core
    bcut = tuple(
        int(max(math.ceil(bstarts[p][b + 1] / 512) for p in range(NCORES)))
        for b in range(NB))
    return cores, bounds, EC, NB, KB, USZ, bcut


def _run(inputs, trace=False):
    x = np.ascontiguousarray(np.asarray(inputs["x"], dtype=np.float32))
    ei = np.asarray(inputs["edge_index"])
    ea = np.ascontiguousarray(np.asarray(inputs["edge_attr"], dtype=np.float32))
    row = ei[0].astype(np.int64)
    col = ei[1].astype(np.int64)
    W1 = np.asarray(inputs["W1"], np.float32).astype(NPBF16)
    W2 = np.asarray(inputs["W2"], np.float32).astype(NPBF16)
    W3 = np.asarray(inputs["W3"], np.float32).astype(NPBF16)
    W4 = np.asarray(inputs["W4"], np.float32).astype(NPBF16)
    b1 = np.asarray(inputs["b1"], np.float32)
    b2 = np.asarray(inputs["b2"], np.float32)
    b3 = np.asarray(inputs["b3"], np.float32)
    b4 = np.asarray(inputs["b4"], np.float32)
    N = x.shape[0]

    cores, bounds, EC, NB, KB, USZ, bcut = _prepare(x, row, col, ea)
    has_b2 = bool(np.any(b2 != 0))

    key = (EC, NB, KB, USZ, bcut, has_b2)
    if key not in _prog_cache:
        _prog_cache[key] = _build(EC, NB, KB, USZ, bcut, has_b2)
    nc = _prog_cache[key]

    b1t = np.ascontiguousarray(b1.reshape(HID // P, P).T)
    b3t = np.ascontiguousarray(b3.reshape((FN + FE) // P, P).T)
    b4t = np.ascontiguousarray(b4.reshape(FN // P, P).T)
    iota = np.ascontiguousarray(
        np.broadcast_to(np.arange(P, dtype=np.float32), (P, P))).astype(NPBF16)

    in_maps = []
    for p in range(NCORES):
        c = cores[p]
        m = {
            "xsrc": c["xsrc"], "gidx": c["gidx"], "xgT0": c["xgT0"],
            "eaT": c["eaT"],
            "W1": W1, "W2": W2, "W3": W3, "W4": W4,
            "b1": b1t, "b3": b3t, "b4": b4t,
            "gid": c["gid"], "colb": c["colb"], "invce": c["invce"],
            "xsT": c["xsT"], "iota": iota,
        }
        if has_b2:
            m["b2r"] = np.ascontiguousarray(b2.reshape(1, HID).astype(NPBF16))
            m["srow"] = c["srow"]
        in_maps.append(m)

    res = run_bass_kernel_spmd(nc, in_maps, list(range(NCORES)), trace=trace)

    out = np.empty((N, FN), np.float32)
    for p in range(NCORES):
        n0, n1 = bounds[p], min(bounds[p + 1], N)
        if n1 > n0:
            out[n0:n1] = res.results[p]["outT"].T[:n1 - n0]
    return out, res


def kernel(**inputs) -> np.ndarray:
    out, _ = _run(inputs, trace=False)
    return out


# revision 61
# speedup vs baseline: 1.6512x; 1.1661x over previous
"""Trainium2 Bass kernel for nn_NodeModel (GNN message passing).

Reference computation:
    h   = relu(concat(x[row], edge_attr) @ W1 + b1) @ W2 + b2     # edge MLP
    agg = scatter_mean(h, col, N)                                  # per-dest mean
    out = relu(concat(x, agg) @ W3 + b3) @ W4 + b4                 # node MLP

Distribution strategy (8 cores, no collectives needed):
  - Sort edges by destination node; split destination nodes into 8
    block-aligned, edge-balanced shards.  Each core owns one node shard and
    ALL edges targeting it, so per-node sums are complete locally.
  - x is replicated; each core gathers x[row] for its edges with indirect
    DMA on-device.
  - All matmul operands are bf16 (fp32 PSUM accumulation): halves HBM
    traffic vs fp32r and speeds PE transposes 1.5x.
  - Edge MLP runs with weights stationary and activations kept transposed
    [feat, edge]; h2 rows (pre-scaled by 1/count of their destination) are
    staged to DRAM in bf16.
  - Scatter-mean per 128-node block: indirect-gather the block's h2 rows,
    build a one-hot selection matrix with is_equal against an iota, and
    matmul-accumulate h2^T @ S in PSUM -> aggT directly in [hid, node]
    layout (the mean's 1/count is pre-applied per-edge in the h2 exit copy).
  - The scatter + node-MLP work is INTERLEAVED into the edge phase: since
    edges are sorted by destination, node block b only needs the first
    bcut[b] edge superchunks.  A static schedule (max over cores, so the
    SPMD program is uniform) runs each superblock as soon as its edges are
    done, overlapping the h2 gather-back DMA with edge-MLP compute.
  - Node-MLP output stays transposed [feat, node]; un-transposed on host.
"""

import math
import sys
from contextlib import ExitStack

sys.path.insert(0, "/opt/trn_rl_repo")

import ml_dtypes
import numpy as np

import concourse.bass as bass
import concourse.tile as tile
from concourse import bacc, mybir
from concourse.bass_utils import run_bass_kernel_spmd

NCORES = 8
P = 128
FN = 512    # node feature dim
FE = 128    # edge feature dim
HID = 1280  # edge-MLP hidden/output dim
F32 = mybir.dt.float32
BF16 = mybir.dt.bfloat16
I32 = mybir.dt.int32
RELU = mybir.ActivationFunctionType.Relu
IDENT = mybir.ActivationFunctionType.Identity
NPBF16 = ml_dtypes.bfloat16

_prog_cache = {}


def _build(EC, NB, KB, USZ, bcut, has_b2):
    """Build the SPMD program for one core.

    EC: edge chunks (128 edges each) per core, multiple of 4.
    NB: node blocks (128 nodes each) per core, multiple of 4.
    KB: max edge chunks per node block (scatter schedule width).
    USZ: rows of the compacted per-core x source table (unique sources,
         int16-indexable so dma_gather's transpose path can be used).
    bcut: per node block, the number of edge superchunks that must be
          complete before its h2 rows exist (max over cores -> uniform).
    has_b2: emit the b2 (x) s_n rank-1 scatter correction (b2 cannot ride
          the h2 activation since the W2 stage keeps edges on partitions;
          it distributes through the mean as agg += b2 * [deg>0]).
    """
    EP = EC * P
    SC = EC // 4   # superchunks of 512 edges
    NSB = NB // 4  # superblocks of 512 nodes
    LOOKAHEAD = max(2, 12 // KB)  # h2-gather prefetch blocks (SBUF-bounded)
    SLACK = 1  # superchunks between a block's h2 completion and its scatter

    nc = bacc.Bacc("TRN2", target_bir_lowering=False, debug=False,
                   num_devices=NCORES)

    NF8 = 6  # leading W2 contraction chunks carried in fp8 DoubleRow pairs
    F8 = mybir.dt.float8e4

    xsrc_d = nc.dram_tensor("xsrc", [USZ, FN], BF16, kind="ExternalInput")
    gidx_d = nc.dram_tensor("gidx", [P, SC * 32], mybir.dt.int16,
                            kind="ExternalInput")
    xgT0_d = nc.dram_tensor("xgT0", [P, 4, 512], BF16, kind="ExternalInput")
    eaT_d = nc.dram_tensor("eaT", [FE, EP], BF16, kind="ExternalInput")
    W1_d = nc.dram_tensor("W1", [FN + FE, HID], BF16, kind="ExternalInput")
    W2a_d = nc.dram_tensor("W2a", [NF8 * P, HID], F8, kind="ExternalInput")
    W2b_d = nc.dram_tensor("W2b", [HID - NF8 * P, HID], BF16,
                           kind="ExternalInput")
    W3_d = nc.dram_tensor("W3", [FN + HID, FN + FE], BF16, kind="ExternalInput")
    W4_d = nc.dram_tensor("W4", [FN + FE, FN], BF16, kind="ExternalInput")
    b1_d = nc.dram_tensor("b1", [P, HID // P], F32, kind="ExternalInput")
    if has_b2:
        b2r_d = nc.dram_tensor("b2r", [1, HID], BF16, kind="ExternalInput")
        srow_d = nc.dram_tensor("srow", [1, NB * P], BF16,
                                kind="ExternalInput")
    b3_d = nc.dram_tensor("b3", [P, (FN + FE) // P], F32, kind="ExternalInput")
    b4_d = nc.dram_tensor("b4", [P, FN // P], F32, kind="ExternalInput")
    gid_d = nc.dram_tensor("gid", [P, NB * KB], I32, kind="ExternalInput")
    colb_d = nc.dram_tensor("colb", [P, NB * KB], BF16, kind="ExternalInput")
    invce_d = nc.dram_tensor("invce", [P, EC], BF16, kind="ExternalInput")
    xsT_d = nc.dram_tensor("xsT", [FN, NB * P], BF16, kind="ExternalInput")
    iota_d = nc.dram_tensor("iota", [P, P], BF16, kind="ExternalInput")
    outT_d = nc.dram_tensor("outT", [FN, NB * P], F32, kind="ExternalOutput")
    h2_d = nc.dram_tensor("h2buf", [EP, HID], BF16)  # internal staging

    with tile.TileContext(nc) as tc, ExitStack() as ctx:
        cpool = ctx.enter_context(tc.tile_pool(name="const", bufs=1))
        wpool = ctx.enter_context(tc.tile_pool(name="wts", bufs=1))

        # Load order = sync-queue FIFO order: first the tensors the first
        # superchunk needs (its pre-transposed x rows are staged on host so
        # nothing waits on the SWDGE warm-up; b1; W1 split per contraction
        # chunk), then everything else behind them.
        b1t = cpool.tile([P, HID // P], F32)
        nc.sync.dma_start(b1t[:], b1_d.ap()[:])
        gidxt = cpool.tile([P, SC * 32], mybir.dt.int16)
        nc.sync.dma_start(gidxt[:], gidx_d.ap()[:])

        def load_weights():
            W1r = W1_d.ap().rearrange("(ko ki) m -> ki ko m", ki=P)
            W1t = [wpool.tile([P, HID], BF16, name=f"W1_{k}", tag=f"W1_{k}")
                   for k in range(5)]
            for k in (4, 0, 1, 2, 3):  # ea chunk's weights first
                nc.sync.dma_start(W1t[k][:], W1r[:, k, :])
            W2at = wpool.tile([P, NF8, HID], F8)
            W2ar = W2a_d.ap().rearrange("(ko ki) m -> ki ko m", ki=P)
            for k in range(NF8):
                nc.sync.dma_start(W2at[:, k, :], W2ar[:, k, :])
            W2bt = wpool.tile([P, 10 - NF8, HID], BF16)
            W2br = W2b_d.ap().rearrange("(ko ki) m -> ki ko m", ki=P)
            for k in range(10 - NF8):
                nc.sync.dma_start(W2bt[:, k, :], W2br[:, k, :])
            iotat = cpool.tile([P, P], BF16)
            nc.sync.dma_start(iotat[:], iota_d.ap()[:])
            b3t = cpool.tile([P, (FN + FE) // P], F32)
            nc.sync.dma_start(b3t[:], b3_d.ap()[:])
            b4t = cpool.tile([P, FN // P], F32)
            nc.sync.dma_start(b4t[:], b4_d.ap()[:])
            gidt = cpool.tile([P, NB * KB], I32)
            nc.sync.dma_start(gidt[:], gid_d.ap()[:])
            colbt = cpool.tile([P, NB * KB], BF16)
            nc.sync.dma_start(colbt[:], colb_d.ap()[:])
            invcet = cpool.tile([P, EC], BF16)
            nc.sync.dma_start(invcet[:], invce_d.ap()[:])
            bsr = None
            if has_b2:
                b2rt = cpool.tile([1, HID], BF16)
                nc.sync.dma_start(b2rt[:], b2r_d.ap()[:])
                srt = cpool.tile([1, NB * P], BF16)
                nc.sync.dma_start(srt[:], srow_d.ap()[:])
                bsr = (b2rt, srt)
            W3t = wpool.tile([P, 14, FN + FE], BF16)
            nc.sync.dma_start(
                W3t[:], W3_d.ap().rearrange("(ko ki) m -> ki ko m", ki=P))
            W4t = wpool.tile([P, 5, FN], BF16)
            nc.sync.dma_start(
                W4t[:], W4_d.ap().rearrange("(ko ki) m -> ki ko m", ki=P))
            return (W1t, W2at, W2bt, W3t, W4t, iotat, b3t, b4t, gidt, colbt,
                    invcet, bsr)

        # ---- pools (all coexist: phases are interleaved) ----
        # PSUM is 8 banks of 2 KB, allocated per tile name at bank
        # granularity: mm 4x[P,512]f32 = 4 banks; smp holds the scatter
        # half-accumulator [P,5,P]f32 (2 banks per buf) -> 4 + 4 = 8.
        mmp = ctx.enter_context(tc.tile_pool(name="mm", bufs=4, space="PSUM"))
        smp = ctx.enter_context(tc.tile_pool(name="smp", bufs=2, space="PSUM"))

        xgTp = ctx.enter_context(tc.tile_pool(name="xgT", bufs=2))
        eap = ctx.enter_context(tc.tile_pool(name="ea", bufs=2))
        h1p = ctx.enter_context(tc.tile_pool(name="h1T", bufs=2))
        h2op = ctx.enter_context(tc.tile_pool(name="h2o", bufs=6))
        h2gp = ctx.enter_context(
            tc.tile_pool(name="h2g", bufs=(LOOKAHEAD + 1) * KB))
        Sp = ctx.enter_context(
            tc.tile_pool(name="Smat", bufs=(LOOKAHEAD + 1) * KB))
        aggTp = ctx.enter_context(tc.tile_pool(name="aggT", bufs=2))
        xsp = ctx.enter_context(tc.tile_pool(name="xs", bufs=2))
        h3p = ctx.enter_context(tc.tile_pool(name="h3T", bufs=1))
        oTp = ctx.enter_context(tc.tile_pool(name="oT", bufs=1))

        # ---------------- edge-phase helpers ----------------
        def issue_gather(sc):
            # dma_gather(transpose=True) delivers x rows already transposed
            # into [feat-chunk-partition, feat-chunk, edge] layout -- no PE
            # entry transposes.  Superchunk 0 is host-staged (plain DMA).
            xgTt = xgTp.tile([P, 4, 512], BF16)
            eat = eap.tile([P, 512], BF16)
            if sc == 0:
                # ea first: the W1 group starts on the ea chunk, so the very
                # first matmul only waits for this small load plus W1_4.
                nc.sync.dma_start(eat[:], eaT_d.ap()[:, :512])
                nc.sync.dma_start(xgTt[:], xgT0_d.ap()[:])
            else:
                nc.gpsimd.dma_gather(
                    xgTt[:], xsrc_d.ap()[:],
                    gidxt[:, sc * 32:(sc + 1) * 32],
                    512, 512, FN, transpose=True)
                nc.sync.dma_start(
                    eat[:], eaT_d.ap()[:, sc * 512:(sc + 1) * 512])
            return xgTt, eat

        # ---------------- scatter/node-phase helpers ----------------
        pend_gs = {}
        state = dict(g_next=0, b_next=0, s_next=0, sc_done=0,
                     xg_cur=None, ea_cur=None, xgT_cur=None, xst_cur=None)

        def gather_S(b):
            ext = max(bcut[b], 1) * 512  # h2 rows that exist by then
            lst = []
            for k in range(KB):
                c = b * KB + k
                # pad slots carry id 0 (not OOB-skip): every partition of the
                # tile gets written with finite data, so the zero one-hot
                # columns can never multiply stale NaN bit patterns.
                h2g = h2gp.tile([P, HID], BF16, name=f"h2g_{b}_{k}",
                                tag="h2g")
                nc.gpsimd.indirect_dma_start(
                    out=h2g[:], out_offset=None, in_=h2_d.ap()[:ext],
                    in_offset=bass.IndirectOffsetOnAxis(
                        ap=gidt[:, c:c + 1], axis=0))
                St = Sp.tile([P, P], BF16, name=f"S_{b}_{k}", tag="S")
                nc.vector.tensor_tensor(
                    St[:], colbt[:, c:c + 1].to_broadcast([P, P]),
                    iotat[:], op=mybir.AluOpType.is_equal)
                lst.append((h2g, St))
            pend_gs[b] = lst

        def try_gathers():
            while (state["g_next"] < NB
                   and bcut[state["g_next"]] <= state["sc_done"]
                   and state["g_next"] < state["b_next"] + LOOKAHEAD):
                gather_S(state["g_next"])
                state["g_next"] += 1

        def load_xst(s):
            xst = xsp.tile([P, 4, 512], BF16, name=f"xst_{s}", tag="xst")
            nc.sync.dma_start(
                xst[:],
                xsT_d.ap().rearrange("(fo fi) n -> fi fo n", fi=P)
                [:, :, s * 512:(s + 1) * 512])
            return xst

        outTr = outT_d.ap().rearrange("(fo fi) n -> fi fo n", fi=P)

        def emit_superblock(s):
            # scatter: accumulate aggT[hid, node] directly in PSUM with the
            # gathered h2 rows stationary and the one-hot S moving.
            aggTsb = aggTp.tile([P, 10, 512], BF16)
            for bb in range(4):
                b = s * 4 + bb
                gs = pend_gs.pop(b)
                # j-major: each 128-wide accumulation group's matmuls stay
                # consecutive (open groups must not interleave in a bank).
                for half in range(2):
                    pss = smp.tile([P, 5, P], F32, name="pss", tag="pss")
                    for j5 in range(5):
                        j = half * 5 + j5
                        for k, (h2g, St) in enumerate(gs):
                            nc.tensor.matmul(
                                pss[:, j5, :], h2g[:, j * P:(j + 1) * P],
                                St[:], start=(k == 0),
                                stop=(k == KB - 1 and not has_b2))
                        if has_b2:
                            # agg includes +b2 for nodes with deg>0: rank-1
                            # b2[j-slice] (x) s_row closes the group.
                            nc.tensor.matmul(
                                pss[:, j5, :], b2rt[:, j * P:(j + 1) * P],
                                srt[:, b * P:(b + 1) * P],
                                start=False, stop=True)
                    nc.vector.tensor_copy(
                        aggTsb[:, half * 5:(half + 1) * 5,
                               bb * P:(bb + 1) * P], pss[:])
                state["b_next"] = b + 1
                try_gathers()

            xst = state["xst_cur"]
            state["xst_cur"] = load_xst(s + 1) if s + 1 < NSB else None
            h3Tt = h3p.tile([P, 5, 512], BF16)
            for of in range(5):
                ps = mmp.tile([P, 512], F32)
                for k in range(4):
                    nc.tensor.matmul(
                        ps[:], W3t[:, k, of * P:(of + 1) * P],
                        xst[:, k, :], start=(k == 0), stop=False)
                for f in range(10):
                    nc.tensor.matmul(
                        ps[:], W3t[:, 4 + f, of * P:(of + 1) * P],
                        aggTsb[:, f, :], start=False, stop=(f == 9))
                nc.scalar.activation(h3Tt[:, of, :], ps[:], RELU,
                                     bias=b3t[:, of:of + 1])
            oTt = oTp.tile([P, 4, 512], F32)
            for of in range(4):
                ps = mmp.tile([P, 512], F32)
                for k in range(5):
                    nc.tensor.matmul(
                        ps[:], W4t[:, k, of * P:(of + 1) * P],
                        h3Tt[:, k, :], start=(k == 0), stop=(k == 4))
                nc.scalar.activation(
                    oTt[:, of, :], ps[:], IDENT, bias=b4t[:, of:of + 1])
            nc.sync.dma_start(outTr[:, :, s * 512:(s + 1) * 512], oTt[:])

        # ---------------- interleaved main loop ----------------
        # superchunk 0's inputs enter the DMA queue first; all weights and
        # scatter tables queue up behind them.
        xgT_cur, ea_cur = issue_gather(0)
        (W1t, W2at, W2bt, W3t, W4t, iotat, b3t, b4t, gidt, colbt, invcet,
         bsr) = load_weights()
        if has_b2:
            b2rt, srt = bsr
        state["xst_cur"] = load_xst(0)

        for sc in range(SC):
            if sc + 1 < SC:
                xgT_next, ea_next = issue_gather(sc + 1)
            else:
                xgT_next = ea_next = None

            h1a = h1p.tile([P, NF8, 512], F8, name="h1a", tag="h1a")
            h1b = h1p.tile([P, 10 - NF8, 512], BF16, name="h1b", tag="h1b")
            for of in range(10):
                ps = mmp.tile([P, 512], F32)
                for i, k in enumerate((4, 0, 1, 2, 3)):
                    rhs = xgT_cur[:, k, :] if k < 4 else ea_cur[:]
                    nc.tensor.matmul(
                        ps[:], W1t[k][:, of * P:(of + 1) * P], rhs,
                        start=(i == 0), stop=(i == 4))
                dst = (h1a[:, of, :] if of < NF8
                       else h1b[:, of - NF8, :])
                nc.scalar.activation(dst, ps[:], RELU,
                                     bias=b1t[:, of:of + 1])
            # W2 stage with h1T stationary and W2 moving: the product lands
            # directly in [edge, hid] layout -- no exit transposes.  The
            # per-edge 1/count of the destination is folded into the
            # PSUM->SBUF copy so the scatter can accumulate raw sums (b2,
            # which would vary along the free dim here, distributes through
            # the scatter-mean and is re-added there when nonzero).
            h2ot = [h2op.tile([P, HID], BF16, name=f"h2o_{sc}_{k}", tag="h2o")
                     for k in range(4)]
            for kk in range(4):
                c = sc * 4 + kk
                for lo, w in ((0, 512), (512, 512), (1024, 256)):
                    ps = mmp.tile([P, 512], F32)
                    # fp8 DoubleRow pairs: two contraction chunks per matmul
                    for g in range(NF8 // 2):
                        nc.tensor.matmul(
                            ps[:, :w],
                            h1a[:, 2 * g:2 * g + 2, kk * P:(kk + 1) * P],
                            W2at[:, 2 * g:2 * g + 2, lo:lo + w],
                            start=(g == 0), stop=False,
                            perf_mode=mybir.MatmulPerfMode.DoubleRow)
                    for k in range(10 - NF8):
                        nc.tensor.matmul(
                            ps[:, :w], h1b[:, k, kk * P:(kk + 1) * P],
                            W2bt[:, k, lo:lo + w],
                            start=False, stop=(k == 10 - NF8 - 1))
                    nc.vector.tensor_tensor(
                        h2ot[kk][:, lo:lo + w], ps[:, :w],
                        invcet[:, c:c + 1].to_broadcast([P, w]),
                        op=mybir.AluOpType.mult)
            for k in range(4):
                r0 = sc * 512 + k * P
                nc.sync.dma_start(h2_d.ap()[r0:r0 + P, :], h2ot[k][:])
            xgT_cur, ea_cur = xgT_next, ea_next

            state["sc_done"] = sc + 1
            try_gathers()
            while (state["s_next"] < NSB
                   and bcut[4 * (state["s_next"] + 1) - 1] + SLACK
                       <= state["sc_done"]):
                emit_superblock(state["s_next"])
                state["s_next"] += 1

        while state["s_next"] < NSB:
            emit_superblock(state["s_next"])
            state["s_next"] += 1

    nc.compile()
    return nc


def _prepare(x, row, col, ea, w2_scale):
    """Host-side sharding: sort edges by destination, split nodes into 8
    block-aligned edge-balanced shards, build per-core arrays."""
    N = x.shape[0]
    E = ea.shape[0]
    order = np.argsort(col, kind="stable")
    scol = col[order]
    srow = row[order]
    NBLK = (N + P - 1) // P
    NTOT = NBLK * P

    bounds = [0]
    for p in range(1, NCORES):
        if E > 0:
            t = int(scol[min((p * E) // NCORES, E - 1)])
        else:
            t = (p * NTOT) // NCORES
        b = int(round(t / P)) * P
        b = max(b, bounds[-1] + P)
        b = min(b, NTOT - P * (NCORES - p))
        bounds.append(b)
    bounds.append(NTOT)
    for p in range(1, NCORES + 1):
        assert bounds[p] > bounds[p - 1], f"degenerate shard bounds {bounds}"

    e_split = np.searchsorted(scol, bounds)
    Ec = np.diff(e_split)
    EC = max(4, math.ceil(int(Ec.max()) / P))
    EC = ((EC + 3) // 4) * 4
    EP = EC * P
    nblk = [(bounds[p + 1] - bounds[p]) // P for p in range(NCORES)]
    NB = max(4, ((max(nblk) + 3) // 4) * 4)
    NBP = NB * P
    blkdeg = np.bincount(scol // P, minlength=NBLK)
    KB = max(1, math.ceil(int(blkdeg.max()) / P))

    xbf = np.zeros((NTOT, FN), NPBF16)
    xbf[:N] = x.astype(NPBF16)
    xpadT = np.zeros((FN, NTOT + NBP), NPBF16)
    xpadT[:, :N] = xbf[:N].T

    cnt_all = np.bincount(scol, minlength=NTOT)

    cores = []
    bstarts = []
    for p in range(NCORES):
        s, e = int(e_split[p]), int(e_split[p + 1])
        n0 = bounds[p]
        ne = e - s
        tmp = np.zeros(EP, np.int64)
        tmp[:ne] = srow[s:e]
        # compacted source table + int16 remapped indices in dma_gather's
        # 16-partition-wrapped layout; superchunk 0 is staged pre-transposed
        uniq, ridx = np.unique(tmp, return_inverse=True)
        assert uniq.size <= 32767, "unique sources exceed int16 gather range"
        xsrc = xbf[uniq]
        ridx = ridx.astype(np.int16)
        SC = EC // 4
        gidx = np.tile(
            ridx.reshape(SC, 32, 16).transpose(2, 0, 1).reshape(16, SC * 32),
            (8, 1))
        xg0 = xsrc[ridx[:512]]  # [512 edges, FN]
        xgT0 = np.ascontiguousarray(
            xg0.T.reshape(4, P, 512).transpose(1, 0, 2))
        eaT = np.zeros((FE, EP), NPBF16)
        eaT[:, :ne] = ea[order[s:e]].T.astype(NPBF16)
        lcol = (scol[s:e] - n0).astype(np.int64)
        bstart = np.searchsorted(lcol, np.arange(NB + 1) * P)
        bstarts.append(bstart)
        gid = np.zeros((NB, KB, P), np.int32)
        colb = np.full((NB, KB, P), -1.0, np.float32)
        for b in range(NB):
            sb, eb = int(bstart[b]), int(bstart[b + 1])
            cnt = eb - sb
            assert cnt <= KB * P
            gid[b].reshape(-1)[:cnt] = np.arange(sb, eb, dtype=np.int32)
            colb[b].reshape(-1)[:cnt] = (lcol[sb:eb] - b * P)
        gid_t = np.ascontiguousarray(gid.reshape(NB * KB, P).T)
        colb_t = np.ascontiguousarray(
            colb.reshape(NB * KB, P).T.astype(NPBF16))
        # per-edge 1/count of the destination node (0 for pad edges so
        # their staged h2 rows are exactly zero)
        invce = np.zeros(EP, np.float32)
        # 1/deg of the destination, divided by the W2 fp8 scale (the whole
        # W2 product is computed scaled; this copy unscales it)
        invce[:ne] = 1.0 / np.maximum(cnt_all[scol[s:e]], 1.0) / w2_scale
        invce_t = np.ascontiguousarray(invce.reshape(EC, P).T.astype(NPBF16))
        xsT = np.ascontiguousarray(xpadT[:, n0:n0 + NBP])
        # per-node degree>0 indicator for the b2 scatter correction
        deg = np.zeros(NBP, np.float32)
        span = min(NBP, NTOT - n0)
        deg[:span] = cnt_all[n0:n0 + span]
        srow_t = (deg > 0).astype(NPBF16).reshape(1, NBP)
        cores.append(dict(xsrc=xsrc, gidx=gidx, xgT0=xgT0, eaT=eaT,
                          gid=gid_t, colb=colb_t,
                          invce=invce_t, xsT=xsT, srow=srow_t))

    # pad the compacted source tables to a uniform row count
    USZ = max(c["xsrc"].shape[0] for c in cores)
    for c in cores:
        u = c["xsrc"].shape[0]
        if u < USZ:
            c["xsrc"] = np.vstack([c["xsrc"], np.zeros((USZ - u, FN), NPBF16)])
        c["xsrc"] = np.ascontiguousarray(c["xsrc"])

    # uniform (max-over-cores) superchunk cut per node block: block b's h2
    # rows are complete once bcut[b] edge superchunks have run on every core
    bcut = tuple(
        int(max(math.ceil(bstarts[p][b + 1] / 512) for p in range(NCORES)))
        for b in range(NB))
    return cores, bounds, EC, NB, KB, USZ, bcut


def _run(inputs, trace=False):
    x = np.ascontiguousarray(np.asarray(inputs["x"], dtype=np.float32))
    ei = np.asarray(inputs["edge_index"])
    ea = np.ascontiguousarray(np.asarray(inputs["edge_attr"], dtype=np.float32))
    row = ei[0].astype(np.int64)
    col = ei[1].astype(np.int64)
    W1 = np.asarray(inputs["W1"], np.float32).astype(NPBF16)
    W2f = np.asarray(inputs["W2"], np.float32)
    W3 = np.asarray(inputs["W3"], np.float32).astype(NPBF16)
    W4 = np.asarray(inputs["W4"], np.float32).astype(NPBF16)
    # W2 split: leading chunks as fp8 (e4m3, max 240) DoubleRow pairs, rest
    # bf16; both pre-scaled so small weights stay out of the subnormal range
    NF8P = 6 * P
    w2_scale = float(224.0 / max(np.abs(W2f).max(), 1e-30))
    W2s = W2f * w2_scale
    W2a = np.ascontiguousarray(W2s[:NF8P].astype(ml_dtypes.float8_e4m3))
    W2b = np.ascontiguousarray(W2s[NF8P:].astype(NPBF16))
    b1 = np.asarray(inputs["b1"], np.float32)
    b2 = np.asarray(inputs["b2"], np.float32)
    b3 = np.asarray(inputs["b3"], np.float32)
    b4 = np.asarray(inputs["b4"], np.float32)
    N = x.shape[0]

    cores, bounds, EC, NB, KB, USZ, bcut = _prepare(x, row, col, ea, w2_scale)
    has_b2 = bool(np.any(b2 != 0))

    key = (EC, NB, KB, USZ, bcut, has_b2)
    if key not in _prog_cache:
        _prog_cache[key] = _build(EC, NB, KB, USZ, bcut, has_b2)
    nc = _prog_cache[key]

    b1t = np.ascontiguousarray(b1.reshape(HID // P, P).T)
    b3t = np.ascontiguousarray(b3.reshape((FN + FE) // P, P).T)
    b4t = np.ascontiguousarray(b4.reshape(FN // P, P).T)
    iota = np.ascontiguousarray(
        np.broadcast_to(np.arange(P, dtype=np.float32), (P, P))).astype(NPBF16)

    in_maps = []
    for p in range(NCORES):
        c = cores[p]
        m = {
            "xsrc": c["xsrc"], "gidx": c["gidx"], "xgT0": c["xgT0"],
            "eaT": c["eaT"],
            "W1": W1, "W2a": W2a, "W2b": W2b, "W3": W3, "W4": W4,
            "b1": b1t, "b3": b3t, "b4": b4t,
            "gid": c["gid"], "colb": c["colb"], "invce": c["invce"],
            "xsT": c["xsT"], "iota": iota,
        }
        if has_b2:
            m["b2r"] = np.ascontiguousarray(b2.reshape(1, HID).astype(NPBF16))
            m["srow"] = c["srow"]
        in_maps.append(m)

    res = run_bass_kernel_spmd(nc, in_maps, list(range(NCORES)), trace=trace)

    out = np.empty((N, FN), np.float32)
    for p in range(NCORES):
        n0, n1 = bounds[p], min(bounds[p + 1], N)
        if n1 > n0:
            out[n0:n1] = res.results[p]["outT"].T[:n1 - n0]
    return out, res


def kernel(**inputs) -> np.ndarray:
    out, _ = _run(inputs, trace=False)
    return out


# revision 63
# speedup vs baseline: 1.7477x; 1.0584x over previous
"""Trainium2 Bass kernel for nn_NodeModel (GNN message passing).

Reference computation:
    h   = relu(concat(x[row], edge_attr) @ W1 + b1) @ W2 + b2     # edge MLP
    agg = scatter_mean(h, col, N)                                  # per-dest mean
    out = relu(concat(x, agg) @ W3 + b3) @ W4 + b4                 # node MLP

Distribution strategy (8 cores, no collectives needed):
  - Sort edges by destination node; split destination nodes into 8
    block-aligned, edge-balanced shards.  Each core owns one node shard and
    ALL edges targeting it, so per-node sums are complete locally.
  - x is replicated; each core gathers x[row] for its edges with indirect
    DMA on-device.
  - All matmul operands are bf16 (fp32 PSUM accumulation): halves HBM
    traffic vs fp32r and speeds PE transposes 1.5x.
  - Edge MLP runs with weights stationary and activations kept transposed
    [feat, edge]; h2 rows (pre-scaled by 1/count of their destination) are
    staged to DRAM in bf16.
  - Scatter-mean per 128-node block: indirect-gather the block's h2 rows,
    build a one-hot selection matrix with is_equal against an iota, and
    matmul-accumulate h2^T @ S in PSUM -> aggT directly in [hid, node]
    layout (the mean's 1/count is pre-applied per-edge in the h2 exit copy).
  - The scatter + node-MLP work is INTERLEAVED into the edge phase: since
    edges are sorted by destination, node block b only needs the first
    bcut[b] edge superchunks.  A static schedule (max over cores, so the
    SPMD program is uniform) runs each superblock as soon as its edges are
    done, overlapping the h2 gather-back DMA with edge-MLP compute.
  - Node-MLP output stays transposed [feat, node]; un-transposed on host.
"""

import math
import sys
from contextlib import ExitStack

sys.path.insert(0, "/opt/trn_rl_repo")

import ml_dtypes
import numpy as np

import concourse.bass as bass
import concourse.tile as tile
from concourse import bacc, mybir
from concourse.bass_utils import run_bass_kernel_spmd

NCORES = 8
P = 128
FN = 512    # node feature dim
FE = 128    # edge feature dim
HID = 1280  # edge-MLP hidden/output dim
F32 = mybir.dt.float32
BF16 = mybir.dt.bfloat16
I32 = mybir.dt.int32
RELU = mybir.ActivationFunctionType.Relu
IDENT = mybir.ActivationFunctionType.Identity
NPBF16 = ml_dtypes.bfloat16

_prog_cache = {}


def _build(EC, NB, KB, USZ, bcut, has_b2):
    """Build the SPMD program for one core.

    EC: edge chunks (128 edges each) per core, multiple of 4.
    NB: node blocks (128 nodes each) per core, multiple of 4.
    KB: max edge chunks per node block (scatter schedule width).
    USZ: rows of the compacted per-core x source table (unique sources,
         int16-indexable so dma_gather's transpose path can be used).
    bcut: per node block, the number of edge superchunks that must be
          complete before its h2 rows exist (max over cores -> uniform).
    has_b2: emit the b2 (x) s_n rank-1 scatter correction (b2 cannot ride
          the h2 activation since the W2 stage keeps edges on partitions;
          it distributes through the mean as agg += b2 * [deg>0]).
    """
    EP = EC * P
    SC = EC // 4   # superchunks of 512 edges
    NSB = NB // 4  # superblocks of 512 nodes
    LOOKAHEAD = max(2, 12 // KB)  # h2-gather prefetch blocks (SBUF-bounded)
    SLACK = 1  # superchunks between a block's h2 completion and its scatter

    nc = bacc.Bacc("TRN2", target_bir_lowering=False, debug=False,
                   num_devices=NCORES)

    NF8 = 8  # leading W2 contraction chunks carried in fp8 DoubleRow pairs
    F8 = mybir.dt.float8e4

    xsrc_d = nc.dram_tensor("xsrc", [USZ, FN], BF16, kind="ExternalInput")
    gidx_d = nc.dram_tensor("gidx", [P, SC * 32], mybir.dt.int16,
                            kind="ExternalInput")
    xgT0_d = nc.dram_tensor("xgT0", [P, 4, 512], BF16, kind="ExternalInput")
    eaT_d = nc.dram_tensor("eaT", [FE, EP], BF16, kind="ExternalInput")
    W1_d = nc.dram_tensor("W1", [FN + FE, HID], BF16, kind="ExternalInput")
    W2a_d = nc.dram_tensor("W2a", [NF8 * P, HID], F8, kind="ExternalInput")
    W2b_d = nc.dram_tensor("W2b", [HID - NF8 * P, HID], BF16,
                           kind="ExternalInput")
    W3_d = nc.dram_tensor("W3", [FN + HID, FN + FE], BF16, kind="ExternalInput")
    W4_d = nc.dram_tensor("W4", [FN + FE, FN], BF16, kind="ExternalInput")
    b1_d = nc.dram_tensor("b1", [P, HID // P], F32, kind="ExternalInput")
    if has_b2:
        b2r_d = nc.dram_tensor("b2r", [1, HID], BF16, kind="ExternalInput")
        srow_d = nc.dram_tensor("srow", [1, NB * P], BF16,
                                kind="ExternalInput")
    b3_d = nc.dram_tensor("b3", [P, (FN + FE) // P], F32, kind="ExternalInput")
    b4_d = nc.dram_tensor("b4", [P, FN // P], F32, kind="ExternalInput")
    gid_d = nc.dram_tensor("gid", [P, NB * KB], I32, kind="ExternalInput")
    colb_d = nc.dram_tensor("colb", [P, NB * KB], BF16, kind="ExternalInput")
    invce_d = nc.dram_tensor("invce", [P, EC], BF16, kind="ExternalInput")
    xsT_d = nc.dram_tensor("xsT", [FN, NB * P], BF16, kind="ExternalInput")
    iota_d = nc.dram_tensor("iota", [P, P], BF16, kind="ExternalInput")
    outT_d = nc.dram_tensor("outT", [FN, NB * P], F32, kind="ExternalOutput")
    h2_d = nc.dram_tensor("h2buf", [EP, HID], BF16)  # internal staging

    with tile.TileContext(nc) as tc, ExitStack() as ctx:
        cpool = ctx.enter_context(tc.tile_pool(name="const", bufs=1))
        wpool = ctx.enter_context(tc.tile_pool(name="wts", bufs=1))

        # Load order = sync-queue FIFO order: first the tensors the first
        # superchunk needs (its pre-transposed x rows are staged on host so
        # nothing waits on the SWDGE warm-up; b1; W1 split per contraction
        # chunk), then everything else behind them.
        b1t = cpool.tile([P, HID // P], F32)
        nc.sync.dma_start(b1t[:], b1_d.ap()[:])
        gidxt = cpool.tile([P, SC * 32], mybir.dt.int16)
        nc.sync.dma_start(gidxt[:], gidx_d.ap()[:])

        def load_weights():
            W1r = W1_d.ap().rearrange("(ko ki) m -> ki ko m", ki=P)
            W1t = [wpool.tile([P, HID], BF16, name=f"W1_{k}", tag=f"W1_{k}")
                   for k in range(5)]
            for k in (4, 0, 1, 2, 3):  # ea chunk's weights first
                nc.sync.dma_start(W1t[k][:], W1r[:, k, :])
            W2at = wpool.tile([P, NF8, HID], F8)
            W2ar = W2a_d.ap().rearrange("(ko ki) m -> ki ko m", ki=P)
            for k in range(NF8):
                nc.sync.dma_start(W2at[:, k, :], W2ar[:, k, :])
            W2bt = wpool.tile([P, 10 - NF8, HID], BF16)
            W2br = W2b_d.ap().rearrange("(ko ki) m -> ki ko m", ki=P)
            for k in range(10 - NF8):
                nc.sync.dma_start(W2bt[:, k, :], W2br[:, k, :])
            iotat = cpool.tile([P, P], BF16)
            nc.sync.dma_start(iotat[:], iota_d.ap()[:])
            b3t = cpool.tile([P, (FN + FE) // P], F32)
            nc.sync.dma_start(b3t[:], b3_d.ap()[:])
            b4t = cpool.tile([P, FN // P], F32)
            nc.sync.dma_start(b4t[:], b4_d.ap()[:])
            gidt = cpool.tile([P, NB * KB], I32)
            nc.sync.dma_start(gidt[:], gid_d.ap()[:])
            colbt = cpool.tile([P, NB * KB], BF16)
            nc.sync.dma_start(colbt[:], colb_d.ap()[:])
            invcet = cpool.tile([P, EC], BF16)
            nc.sync.dma_start(invcet[:], invce_d.ap()[:])
            bsr = None
            if has_b2:
                b2rt = cpool.tile([1, HID], BF16)
                nc.sync.dma_start(b2rt[:], b2r_d.ap()[:])
                srt = cpool.tile([1, NB * P], BF16)
                nc.sync.dma_start(srt[:], srow_d.ap()[:])
                bsr = (b2rt, srt)
            W3t = wpool.tile([P, 14, FN + FE], BF16)
            nc.sync.dma_start(
                W3t[:], W3_d.ap().rearrange("(ko ki) m -> ki ko m", ki=P))
            W4t = wpool.tile([P, 5, FN], BF16)
            nc.sync.dma_start(
                W4t[:], W4_d.ap().rearrange("(ko ki) m -> ki ko m", ki=P))
            return (W1t, W2at, W2bt, W3t, W4t, iotat, b3t, b4t, gidt, colbt,
                    invcet, bsr)

        # ---- pools (all coexist: phases are interleaved) ----
        # PSUM is 8 banks of 2 KB, allocated per tile name at bank
        # granularity: mm 4x[P,512]f32 = 4 banks; smp holds the scatter
        # half-accumulator [P,5,P]f32 (2 banks per buf) -> 4 + 4 = 8.
        mmp = ctx.enter_context(tc.tile_pool(name="mm", bufs=4, space="PSUM"))
        smp = ctx.enter_context(tc.tile_pool(name="smp", bufs=2, space="PSUM"))

        xgTp = ctx.enter_context(tc.tile_pool(name="xgT", bufs=2))
        eap = ctx.enter_context(tc.tile_pool(name="ea", bufs=2))
        h1p = ctx.enter_context(tc.tile_pool(name="h1T", bufs=2))
        h2op = ctx.enter_context(tc.tile_pool(name="h2o", bufs=6))
        h2gp = ctx.enter_context(
            tc.tile_pool(name="h2g", bufs=(LOOKAHEAD + 1) * KB))
        Sp = ctx.enter_context(
            tc.tile_pool(name="Smat", bufs=(LOOKAHEAD + 1) * KB))
        aggTp = ctx.enter_context(tc.tile_pool(name="aggT", bufs=2))
        xsp = ctx.enter_context(tc.tile_pool(name="xs", bufs=2))
        h3p = ctx.enter_context(tc.tile_pool(name="h3T", bufs=1))
        oTp = ctx.enter_context(tc.tile_pool(name="oT", bufs=1))

        # ---------------- edge-phase helpers ----------------
        def issue_gather(sc):
            # dma_gather(transpose=True) delivers x rows already transposed
            # into [feat-chunk-partition, feat-chunk, edge] layout -- no PE
            # entry transposes.  Superchunk 0 is host-staged (plain DMA).
            xgTt = xgTp.tile([P, 4, 512], BF16)
            eat = eap.tile([P, 512], BF16)
            if sc == 0:
                # ea first: the W1 group starts on the ea chunk, so the very
                # first matmul only waits for this small load plus W1_4.
                nc.sync.dma_start(eat[:], eaT_d.ap()[:, :512])
                nc.sync.dma_start(xgTt[:], xgT0_d.ap()[:])
            else:
                nc.gpsimd.dma_gather(
                    xgTt[:], xsrc_d.ap()[:],
                    gidxt[:, sc * 32:(sc + 1) * 32],
                    512, 512, FN, transpose=True)
                nc.sync.dma_start(
                    eat[:], eaT_d.ap()[:, sc * 512:(sc + 1) * 512])
            return xgTt, eat

        # ---------------- scatter/node-phase helpers ----------------
        pend_gs = {}
        state = dict(g_next=0, b_next=0, s_next=0, sc_done=0,
                     xg_cur=None, ea_cur=None, xgT_cur=None, xst_cur=None)

        def gather_S(b):
            ext = max(bcut[b], 1) * 512  # h2 rows that exist by then
            lst = []
            for k in range(KB):
                c = b * KB + k
                # pad slots carry id 0 (not OOB-skip): every partition of the
                # tile gets written with finite data, so the zero one-hot
                # columns can never multiply stale NaN bit patterns.
                h2g = h2gp.tile([P, HID], BF16, name=f"h2g_{b}_{k}",
                                tag="h2g")
                nc.gpsimd.indirect_dma_start(
                    out=h2g[:], out_offset=None, in_=h2_d.ap()[:ext],
                    in_offset=bass.IndirectOffsetOnAxis(
                        ap=gidt[:, c:c + 1], axis=0))
                St = Sp.tile([P, P], BF16, name=f"S_{b}_{k}", tag="S")
                nc.vector.tensor_tensor(
                    St[:], colbt[:, c:c + 1].to_broadcast([P, P]),
                    iotat[:], op=mybir.AluOpType.is_equal)
                lst.append((h2g, St))
            pend_gs[b] = lst

        def try_gathers():
            while (state["g_next"] < NB
                   and bcut[state["g_next"]] <= state["sc_done"]
                   and state["g_next"] < state["b_next"] + LOOKAHEAD):
                gather_S(state["g_next"])
                state["g_next"] += 1

        def load_xst(s):
            xst = xsp.tile([P, 4, 512], BF16, name=f"xst_{s}", tag="xst")
            nc.sync.dma_start(
                xst[:],
                xsT_d.ap().rearrange("(fo fi) n -> fi fo n", fi=P)
                [:, :, s * 512:(s + 1) * 512])
            return xst

        outTr = outT_d.ap().rearrange("(fo fi) n -> fi fo n", fi=P)

        def emit_superblock(s):
            # scatter: accumulate aggT[hid, node] directly in PSUM with the
            # gathered h2 rows stationary and the one-hot S moving.
            aggTsb = aggTp.tile([P, 10, 512], BF16)
            for bb in range(4):
                b = s * 4 + bb
                gs = pend_gs.pop(b)
                # j-major: each 128-wide accumulation group's matmuls stay
                # consecutive (open groups must not interleave in a bank).
                for half in range(2):
                    pss = smp.tile([P, 5, P], F32, name="pss", tag="pss")
                    for j5 in range(5):
                        j = half * 5 + j5
                        for k, (h2g, St) in enumerate(gs):
                            nc.tensor.matmul(
                                pss[:, j5, :], h2g[:, j * P:(j + 1) * P],
                                St[:], start=(k == 0),
                                stop=(k == KB - 1 and not has_b2))
                        if has_b2:
                            # agg includes +b2 for nodes with deg>0: rank-1
                            # b2[j-slice] (x) s_row closes the group.
                            nc.tensor.matmul(
                                pss[:, j5, :], b2rt[:, j * P:(j + 1) * P],
                                srt[:, b * P:(b + 1) * P],
                                start=False, stop=True)
                    nc.vector.tensor_copy(
                        aggTsb[:, half * 5:(half + 1) * 5,
                               bb * P:(bb + 1) * P], pss[:])
                state["b_next"] = b + 1
                try_gathers()

            xst = state["xst_cur"]
            state["xst_cur"] = load_xst(s + 1) if s + 1 < NSB else None
            h3Tt = h3p.tile([P, 5, 512], BF16)
            for of in range(5):
                ps = mmp.tile([P, 512], F32)
                for k in range(4):
                    nc.tensor.matmul(
                        ps[:], W3t[:, k, of * P:(of + 1) * P],
                        xst[:, k, :], start=(k == 0), stop=False)
                for f in range(10):
                    nc.tensor.matmul(
                        ps[:], W3t[:, 4 + f, of * P:(of + 1) * P],
                        aggTsb[:, f, :], start=False, stop=(f == 9))
                nc.scalar.activation(h3Tt[:, of, :], ps[:], RELU,
                                     bias=b3t[:, of:of + 1])
            oTt = oTp.tile([P, 4, 512], F32)
            for of in range(4):
                ps = mmp.tile([P, 512], F32)
                for k in range(5):
                    nc.tensor.matmul(
                        ps[:], W4t[:, k, of * P:(of + 1) * P],
                        h3Tt[:, k, :], start=(k == 0), stop=(k == 4))
                nc.scalar.activation(
                    oTt[:, of, :], ps[:], IDENT, bias=b4t[:, of:of + 1])
            nc.sync.dma_start(outTr[:, :, s * 512:(s + 1) * 512], oTt[:])

        # ---------------- interleaved main loop ----------------
        # superchunk 0's inputs enter the DMA queue first; all weights and
        # scatter tables queue up behind them.
        xgT_cur, ea_cur = issue_gather(0)
        (W1t, W2at, W2bt, W3t, W4t, iotat, b3t, b4t, gidt, colbt, invcet,
         bsr) = load_weights()
        if has_b2:
            b2rt, srt = bsr
        state["xst_cur"] = load_xst(0)

        for sc in range(SC):
            if sc + 1 < SC:
                xgT_next, ea_next = issue_gather(sc + 1)
            else:
                xgT_next = ea_next = None

            h1a = h1p.tile([P, NF8, 512], F8, name="h1a", tag="h1a")
            h1b = h1p.tile([P, 10 - NF8, 512], BF16, name="h1b", tag="h1b")
            for of in range(10):
                ps = mmp.tile([P, 512], F32)
                for i, k in enumerate((4, 0, 1, 2, 3)):
                    rhs = xgT_cur[:, k, :] if k < 4 else ea_cur[:]
                    nc.tensor.matmul(
                        ps[:], W1t[k][:, of * P:(of + 1) * P], rhs,
                        start=(i == 0), stop=(i == 4))
                dst = (h1a[:, of, :] if of < NF8
                       else h1b[:, of - NF8, :])
                nc.scalar.activation(dst, ps[:], RELU,
                                     bias=b1t[:, of:of + 1])
            # W2 stage with h1T stationary and W2 moving: the product lands
            # directly in [edge, hid] layout -- no exit transposes.  The
            # per-edge 1/count of the destination is folded into the
            # PSUM->SBUF copy so the scatter can accumulate raw sums (b2,
            # which would vary along the free dim here, distributes through
            # the scatter-mean and is re-added there when nonzero).
            h2ot = [h2op.tile([P, HID], BF16, name=f"h2o_{sc}_{k}", tag="h2o")
                     for k in range(4)]
            for kk in range(4):
                c = sc * 4 + kk
                for lo, w in ((0, 512), (512, 512), (1024, 256)):
                    ps = mmp.tile([P, 512], F32)
                    # fp8 DoubleRow pairs: two contraction chunks per matmul
                    for g in range(NF8 // 2):
                        nc.tensor.matmul(
                            ps[:, :w],
                            h1a[:, 2 * g:2 * g + 2, kk * P:(kk + 1) * P],
                            W2at[:, 2 * g:2 * g + 2, lo:lo + w],
                            start=(g == 0), stop=False,
                            perf_mode=mybir.MatmulPerfMode.DoubleRow)
                    for k in range(10 - NF8):
                        nc.tensor.matmul(
                            ps[:, :w], h1b[:, k, kk * P:(kk + 1) * P],
                            W2bt[:, k, lo:lo + w],
                            start=False, stop=(k == 10 - NF8 - 1))
                    nc.vector.tensor_tensor(
                        h2ot[kk][:, lo:lo + w], ps[:, :w],
                        invcet[:, c:c + 1].to_broadcast([P, w]),
                        op=mybir.AluOpType.mult)
            for k in range(4):
                r0 = sc * 512 + k * P
                nc.sync.dma_start(h2_d.ap()[r0:r0 + P, :], h2ot[k][:])
            xgT_cur, ea_cur = xgT_next, ea_next

            state["sc_done"] = sc + 1
            try_gathers()
            while (state["s_next"] < NSB
                   and bcut[4 * (state["s_next"] + 1) - 1] + SLACK
                       <= state["sc_done"]):
                emit_superblock(state["s_next"])
                state["s_next"] += 1

        while state["s_next"] < NSB:
            emit_superblock(state["s_next"])
            state["s_next"] += 1

    nc.compile()
    return nc


def _prepare(x, row, col, ea, w2_scale):
    """Host-side sharding: sort edges by destination, split nodes into 8
    block-aligned edge-balanced shards, build per-core arrays."""
    N = x.shape[0]
    E = ea.shape[0]
    order = np.argsort(col, kind="stable")
    scol = col[order]
    srow = row[order]
    NBLK = (N + P - 1) // P
    NTOT = NBLK * P

    bounds = [0]
    for p in range(1, NCORES):
        if E > 0:
            t = int(scol[min((p * E) // NCORES, E - 1)])
        else:
            t = (p * NTOT) // NCORES
        b = int(round(t / P)) * P
        b = max(b, bounds[-1] + P)
        b = min(b, NTOT - P * (NCORES - p))
        bounds.append(b)
    bounds.append(NTOT)
    for p in range(1, NCORES + 1):
        assert bounds[p] > bounds[p - 1], f"degenerate shard bounds {bounds}"

    e_split = np.searchsorted(scol, bounds)
    Ec = np.diff(e_split)
    EC = max(4, math.ceil(int(Ec.max()) / P))
    EC = ((EC + 3) // 4) * 4
    EP = EC * P
    nblk = [(bounds[p + 1] - bounds[p]) // P for p in range(NCORES)]
    NB = max(4, ((max(nblk) + 3) // 4) * 4)
    NBP = NB * P
    blkdeg = np.bincount(scol // P, minlength=NBLK)
    KB = max(1, math.ceil(int(blkdeg.max()) / P))

    xbf = np.zeros((NTOT, FN), NPBF16)
    xbf[:N] = x.astype(NPBF16)
    xpadT = np.zeros((FN, NTOT + NBP), NPBF16)
    xpadT[:, :N] = xbf[:N].T

    cnt_all = np.bincount(scol, minlength=NTOT)

    cores = []
    bstarts = []
    for p in range(NCORES):
        s, e = int(e_split[p]), int(e_split[p + 1])
        n0 = bounds[p]
        ne = e - s
        tmp = np.zeros(EP, np.int64)
        tmp[:ne] = srow[s:e]
        # compacted source table + int16 remapped indices in dma_gather's
        # 16-partition-wrapped layout; superchunk 0 is staged pre-transposed
        uniq, ridx = np.unique(tmp, return_inverse=True)
        assert uniq.size <= 32767, "unique sources exceed int16 gather range"
        xsrc = xbf[uniq]
        ridx = ridx.astype(np.int16)
        SC = EC // 4
        gidx = np.tile(
            ridx.reshape(SC, 32, 16).transpose(2, 0, 1).reshape(16, SC * 32),
            (8, 1))
        xg0 = xsrc[ridx[:512]]  # [512 edges, FN]
        xgT0 = np.ascontiguousarray(
            xg0.T.reshape(4, P, 512).transpose(1, 0, 2))
        eaT = np.zeros((FE, EP), NPBF16)
        eaT[:, :ne] = ea[order[s:e]].T.astype(NPBF16)
        lcol = (scol[s:e] - n0).astype(np.int64)
        bstart = np.searchsorted(lcol, np.arange(NB + 1) * P)
        bstarts.append(bstart)
        gid = np.zeros((NB, KB, P), np.int32)
        colb = np.full((NB, KB, P), -1.0, np.float32)
        for b in range(NB):
            sb, eb = int(bstart[b]), int(bstart[b + 1])
            cnt = eb - sb
            assert cnt <= KB * P
            gid[b].reshape(-1)[:cnt] = np.arange(sb, eb, dtype=np.int32)
            colb[b].reshape(-1)[:cnt] = (lcol[sb:eb] - b * P)
        gid_t = np.ascontiguousarray(gid.reshape(NB * KB, P).T)
        colb_t = np.ascontiguousarray(
            colb.reshape(NB * KB, P).T.astype(NPBF16))
        # per-edge 1/count of the destination node (0 for pad edges so
        # their staged h2 rows are exactly zero)
        invce = np.zeros(EP, np.float32)
        # 1/deg of the destination, divided by the W2 fp8 scale (the whole
        # W2 product is computed scaled; this copy unscales it)
        invce[:ne] = 1.0 / np.maximum(cnt_all[scol[s:e]], 1.0) / w2_scale
        invce_t = np.ascontiguousarray(invce.reshape(EC, P).T.astype(NPBF16))
        xsT = np.ascontiguousarray(xpadT[:, n0:n0 + NBP])
        # per-node degree>0 indicator for the b2 scatter correction
        deg = np.zeros(NBP, np.float32)
        span = min(NBP, NTOT - n0)
        deg[:span] = cnt_all[n0:n0 + span]
        srow_t = (deg > 0).astype(NPBF16).reshape(1, NBP)
        cores.append(dict(xsrc=xsrc, gidx=gidx, xgT0=xgT0, eaT=eaT,
                          gid=gid_t, colb=colb_t,
                          invce=invce_t, xsT=xsT, srow=srow_t))

    # pad the compacted source tables to a uniform row count
    USZ = max(c["xsrc"].shape[0] for c in cores)
    for c in cores:
        u = c["xsrc"].shape[0]
        if u < USZ:
            c["xsrc"] = np.vstack([c["xsrc"], np.zeros((USZ - u, FN), NPBF16)])
        c["xsrc"] = np.ascontiguousarray(c["xsrc"])

    # uniform (max-over-cores) superchunk cut per node block: block b's h2
    # rows are complete once bcut[b] edge superchunks have run on every core
    bcut = tuple(
        int(max(math.ceil(bstarts[p][b + 1] / 512) for p in range(NCORES)))
        for b in range(NB))
    return cores, bounds, EC, NB, KB, USZ, bcut


def _run(inputs, trace=False):
    x = np.ascontiguousarray(np.asarray(inputs["x"], dtype=np.float32))
    ei = np.asarray(inputs["edge_index"])
    ea = np.ascontiguousarray(np.asarray(inputs["edge_attr"], dtype=np.float32))
    row = ei[0].astype(np.int64)
    col = ei[1].astype(np.int64)
    W1 = np.asarray(inputs["W1"], np.float32).astype(NPBF16)
    W2f = np.asarray(inputs["W2"], np.float32)
    W3 = np.asarray(inputs["W3"], np.float32).astype(NPBF16)
    W4 = np.asarray(inputs["W4"], np.float32).astype(NPBF16)
    # W2 split: leading chunks as fp8 (e4m3, max 240) DoubleRow pairs, rest
    # bf16; both pre-scaled so small weights stay out of the subnormal range
    NF8P = 8 * P
    w2_scale = float(224.0 / max(np.abs(W2f).max(), 1e-30))
    W2s = W2f * w2_scale
    W2a = np.ascontiguousarray(W2s[:NF8P].astype(ml_dtypes.float8_e4m3))
    W2b = np.ascontiguousarray(W2s[NF8P:].astype(NPBF16))
    b1 = np.asarray(inputs["b1"], np.float32)
    b2 = np.asarray(inputs["b2"], np.float32)
    b3 = np.asarray(inputs["b3"], np.float32)
    b4 = np.asarray(inputs["b4"], np.float32)
    N = x.shape[0]

    cores, bounds, EC, NB, KB, USZ, bcut = _prepare(x, row, col, ea, w2_scale)
    has_b2 = bool(np.any(b2 != 0))

    key = (EC, NB, KB, USZ, bcut, has_b2)
    if key not in _prog_cache:
        _prog_cache[key] = _build(EC, NB, KB, USZ, bcut, has_b2)
    nc = _prog_cache[key]

    b1t = np.ascontiguousarray(b1.reshape(HID // P, P).T)
    b3t = np.ascontiguousarray(b3.reshape((FN + FE) // P, P).T)
    b4t = np.ascontiguousarray(b4.reshape(FN // P, P).T)
    iota = np.ascontiguousarray(
        np.broadcast_to(np.arange(P, dtype=np.float32), (P, P))).astype(NPBF16)

    in_maps = []
    for p in range(NCORES):
        c = cores[p]
        m = {
            "xsrc": c["xsrc"], "gidx": c["gidx"], "xgT0": c["xgT0"],
            "eaT": c["eaT"],
            "W1": W1, "W2a": W2a, "W2b": W2b, "W3": W3, "W4": W4,
            "b1": b1t, "b3": b3t, "b4": b4t,
            "gid": c["gid"], "colb": c["colb"], "invce": c["invce"],
            "xsT": c["xsT"], "iota": iota,
        }
        if has_b2:
            m["b2r"] = np.ascontiguousarray(b2.reshape(1, HID).astype(NPBF16))
            m["srow"] = c["srow"]
        in_maps.append(m)

    res = run_bass_kernel_spmd(nc, in_maps, list(range(NCORES)), trace=trace)

    out = np.empty((N, FN), np.float32)
    for p in range(NCORES):
        n0, n1 = bounds[p], min(bounds[p + 1], N)
        if n1 > n0:
            out[n0:n1] = res.results[p]["outT"].T[:n1 - n0]
    return out, res


def kernel(**inputs) -> np.ndarray:
    out, _ = _run(inputs, trace=False)
    return out
